# revision 1
# baseline (speedup 1.0000x reference)
"""CrossGraphAttentionModel on 8 Trainium2 NeuronCores (Bass/Tile, SPMD).

Sharding: nodes/edges of both graphs are sharded 8 ways by (dst-sorted) node
range; 64-dim weights replicated. Per GINE layer each core dma_gathers x[src]
for its edge shard from an AllGathered copy of x in HBM, forms messages on
DVE/ACT, and scatter-adds them with one-hot matmuls on the PE (PSUM
accumulation), then runs the node MLP on its node shard and AllGathers the new
x. Cross-graph attention shards the query axis: scores are computed twice on
PE - once [q,k] for an exact row max, once [k,q] with the max folded into the
contraction via an appended ones row - so softmax needs only a single ACT exp
pass, and the exp tiles feed the wV matmul directly as lhsT with a ones column
in V producing the softmax denominator for free. Graph pooling is a one-hot
matmul with 1/count weights, AllReduced, followed by the tiny output MLP.

All floating point math runs on device in fp32; the host only sorts/pads
integer index structures and transposes/replicates input layouts.
"""

import numpy as np

R = 8
HID = 64
B = 32
HEADS = 4
HD = 16
N_MOL, N_PROT = 2048, 4096
E_MOL, E_PROT = 32768, 131072
NC_MOL, NC_PROT = N_MOL // R, N_PROT // R          # 256, 512
NBLK_MOL, NBLK_PROT = NC_MOL // 128, NC_PROT // 128  # 2, 4

_CACHE = {}
last_results = None


# ----------------------------------------------------------------- host prep

def _prep_edges(edge_index, eattr, nblk):
    """Sort edges by dst, partition into R cores x nblk 128-node windows,
    pad every window to T_blk tiles of 128 edges. Returns device layouts."""
    src, dst = np.asarray(edge_index[0]), np.asarray(edge_index[1])
    eattr = np.asarray(eattr, np.float32)
    order = np.argsort(dst, kind="stable")
    src_s, dst_s, ea_s = src[order], dst[order], eattr[order]
    nblocks = R * nblk
    blk = dst_s // 128
    counts = np.bincount(blk, minlength=nblocks)
    T_blk = int(np.ceil(counts.max() / 128))
    T_total = nblk * T_blk
    E_core = T_total * 128
    D = eattr.shape[1]

    gidx = np.zeros((R, E_core), np.int64)
    dstoff = np.full((R, E_core), -1.0, np.float32)
    ea_pad = np.zeros((R, E_core, D), np.float32)
    starts = np.concatenate([[0], np.cumsum(counts)])
    for c in range(R):
        for b in range(nblk):
            g = c * nblk + b
            cnt = counts[g]
            lo = starts[g]
            off = b * T_blk * 128
            gidx[c, off:off + cnt] = src_s[lo:lo + cnt]
            dstoff[c, off:off + cnt] = (dst_s[lo:lo + cnt] - g * 128)
            ea_pad[c, off:off + cnt] = ea_s[lo:lo + cnt]

    # gather indices wrapped [128, E_core/16] (i -> p=i%16, col=i//16), x8 replicated
    cols = E_core // 16
    w = gidx.reshape(R, cols, 16).transpose(0, 2, 1).astype(np.int16)
    gidx_sb = np.tile(w, (1, 8, 1)).copy()
    # dstoff [128, T_total]
    dstoff_sb = np.ascontiguousarray(
        dstoff.reshape(R, T_total, 128).transpose(0, 2, 1))
    # eattr^T with ones row: [11, E_core]
    eaT_packed = np.ascontiguousarray(np.concatenate(
        [ea_pad.transpose(0, 2, 1),
         np.ones((R, 1, E_core), np.float32)], axis=1))
    return dict(T_blk=T_blk, T_total=T_total, E_core=E_core, D=D,
                gidx_sb=gidx_sb, dstoff_sb=dstoff_sb, eaT_packed=eaT_packed)


def _prep_host(inp):
    """All integer/layout preprocessing. Returns (meta, per_core_inputs)."""
    mol = _prep_edges(inp["mol_edge_index"], inp["mol_eattr"], NBLK_MOL)
    prot = _prep_edges(inp["prot_edge_index"], inp["prot_eattr"], NBLK_PROT)

    # pool matrices with 1/count entries
    def pmat(batch, ncore):
        batch = np.asarray(batch)
        cnt = np.bincount(batch, minlength=B).astype(np.float32)
        inv = 1.0 / np.maximum(cnt, 1.0)
        m = np.zeros((R, ncore, B), np.float32)
        for c in range(R):
            sl = batch[c * ncore:(c + 1) * ncore]
            m[c, np.arange(ncore), sl] = inv[sl]
        return m

    mol_pmat = pmat(inp["mol_batch"], NC_MOL)
    prot_pmat = pmat(inp["prot_batch"], NC_PROT)

    # node features transposed per core with ones row
    def xt(x, ncore):
        x = np.asarray(x, np.float32)
        d = x.shape[1]
        out = np.zeros((R, d + 1, ncore), np.float32)
        for c in range(R):
            out[c, :d] = x[c * ncore:(c + 1) * ncore].T
            out[c, d] = 1.0
        return out

    mol_xT = xt(inp["mol_x"], NC_MOL)        # [R, 12, 256]
    prot_xT = xt(inp["prot_x"], NC_PROT)     # [R, 16, 512]

    iota = np.tile(np.arange(128, dtype=np.float32), (128, 1))
    ident = np.eye(128, dtype=np.float32)

    # attn K-bias as per-head columns [16, 4]
    def bcols(b):  # [64] -> [16, 4]
        return np.ascontiguousarray(np.asarray(b, np.float32).reshape(4, 16).T)

    percore = []
    for c in range(R):
        m = {
            "mol_xT": mol_xT[c], "prot_xT": prot_xT[c],
            "mol_eaT": mol["eaT_packed"][c], "prot_eaT": prot["eaT_packed"][c],
            "mol_gidx": mol["gidx_sb"][c], "prot_gidx": prot["gidx_sb"][c],
            "mol_dstoff": mol["dstoff_sb"][c], "prot_dstoff": prot["dstoff_sb"][c],
            "mol_pmat": mol_pmat[c], "prot_pmat": prot_pmat[c],
            "iota": iota, "ident": ident,
            "bk_mp_cols": bcols(np.asarray(inp["attn_mp_b"])[1]),
            "bk_pm_cols": bcols(np.asarray(inp["attn_pm_b"])[1]),
        }
        for k in ("node_lin_mol_W", "node_lin_mol_b", "node_lin_prot_W",
                  "node_lin_prot_b", "edge_lin_mol_W", "edge_lin_mol_b",
                  "edge_lin_prot_W", "edge_lin_prot_b",
                  "mol_conv_W1", "mol_conv_b1", "mol_conv_W2", "mol_conv_b2",
                  "prot_conv_W1", "prot_conv_b1", "prot_conv_W2", "prot_conv_b2",
                  "attn_mp_W", "attn_mp_b", "attn_pm_W", "attn_pm_b",
                  "fc1_W", "fc1_b", "fc2_W", "fc2_b"):
            m[k] = np.asarray(inp[k], np.float32)
        percore.append(m)

    meta = dict(mol_T_blk=mol["T_blk"], mol_T_total=mol["T_total"],
                mol_E_core=mol["E_core"],
                prot_T_blk=prot["T_blk"], prot_T_total=prot["T_total"],
                prot_E_core=prot["E_core"])
    return meta, percore


# ------------------------------------------------------------- device build

def _build(meta):
    import concourse.bacc as bacc
    import concourse.mybir as mybir
    import concourse.tile as tile

    F32 = mybir.dt.float32
    I16 = mybir.dt.int16
    AF = mybir.ActivationFunctionType
    ALU = mybir.AluOpType

    nc = bacc.Bacc("TRN2", target_bir_lowering=False, debug=False,
                   num_devices=R)

    # ---- I/O declarations
    dram = {}

    def din(name, shape, dtype=F32):
        dram[name] = nc.dram_tensor(name, list(shape), dtype,
                                    kind="ExternalInput")
        return dram[name]

    mT, mE = meta["mol_T_total"], meta["mol_E_core"]
    pT, pE = meta["prot_T_total"], meta["prot_E_core"]

    din("mol_xT", [12, NC_MOL]); din("prot_xT", [16, NC_PROT])
    din("mol_eaT", [11, mE]); din("prot_eaT", [11, pE])
    din("mol_gidx", [128, mE // 16], I16); din("prot_gidx", [128, pE // 16], I16)
    din("mol_dstoff", [128, mT]); din("prot_dstoff", [128, pT])
    din("mol_pmat", [NC_MOL, B]); din("prot_pmat", [NC_PROT, B])
    din("iota", [128, 128]); din("ident", [128, 128])
    din("bk_mp_cols", [16, 4]); din("bk_pm_cols", [16, 4])
    din("node_lin_mol_W", [11, 64]); din("node_lin_mol_b", [64])
    din("node_lin_prot_W", [15, 64]); din("node_lin_prot_b", [64])
    din("edge_lin_mol_W", [10, 64]); din("edge_lin_mol_b", [64])
    din("edge_lin_prot_W", [10, 64]); din("edge_lin_prot_b", [64])
    for s in ("mol", "prot"):
        din(f"{s}_conv_W1", [3, 64, 64]); din(f"{s}_conv_b1", [3, 64])
        din(f"{s}_conv_W2", [3, 64, 64]); din(f"{s}_conv_b2", [3, 64])
    din("attn_mp_W", [3, 64, 64]); din("attn_mp_b", [3, 64])
    din("attn_pm_W", [3, 64, 64]); din("attn_pm_b", [3, 64])
    din("fc1_W", [128, 64]); din("fc1_b", [64])
    din("fc2_W", [64, 1]); din("fc2_b", [1])

    out_d = nc.dram_tensor("out", [1, B], F32, kind="ExternalOutput")

    sides = {
        "mol": dict(N=N_MOL, NC=NC_MOL, nblk=NBLK_MOL, T_blk=meta["mol_T_blk"],
                    T_total=mT, E_core=mE, D=10),
        "prot": dict(N=N_PROT, NC=NC_PROT, nblk=NBLK_PROT,
                     T_blk=meta["prot_T_blk"], T_total=pT, E_core=pE, D=10),
    }

    with tile.TileContext(nc) as tc:
        # ---------------- persistent SBUF constants
        const = tc.alloc_tile_pool(name="const", bufs=1)

        def load_const(name, shape, dtype=F32, src=None):
            t = const.tile(list(shape), dtype, name=f"c_{name}")
            nc.sync.dma_start(t[:], (dram[name] if src is None else src)[:])
            return t

        iota_sb = load_const("iota", [128, 128])
        ident_sb = load_const("ident", [128, 128])

        def wcat(name_w, name_b, din_, dout, wslice=None, bslice=None):
            t = const.tile([din_ + 1, dout], F32, name=f"w_{name_w}_{wslice}")
            wsrc = dram[name_w] if wslice is None else dram[name_w][wslice]
            bsrc = dram[name_b] if bslice is None else dram[name_b][bslice]
            nc.sync.dma_start(t[0:din_, :], wsrc[:, :] if wslice is None else wsrc)
            nc.sync.dma_start(t[din_:din_ + 1, :], bsrc[None, :])
            return t

        Wn = {"mol": wcat("node_lin_mol_W", "node_lin_mol_b", 11, 64),
              "prot": wcat("node_lin_prot_W", "node_lin_prot_b", 15, 64)}
        We = {"mol": wcat("edge_lin_mol_W", "edge_lin_mol_b", 10, 64),
              "prot": wcat("edge_lin_prot_W", "edge_lin_prot_b", 10, 64)}
        W1 = {s: [wcat(f"{s}_conv_W1", f"{s}_conv_b1", 64, 64, l, l)
                  for l in range(3)] for s in ("mol", "prot")}
        W2 = {s: [wcat(f"{s}_conv_W2", f"{s}_conv_b2", 64, 64, l, l)
                  for l in range(3)] for s in ("mol", "prot")}

        sb_idx, sb_dstoff = {}, {}
        for s in sides:
            sd = sides[s]
            sb_idx[s] = load_const(f"{s}_gidx", [128, sd["E_core"] // 16], I16)
            sb_dstoff[s] = load_const(f"{s}_dstoff", [128, sd["T_total"]])
        sb_xTin = {"mol": load_const("mol_xT", [12, NC_MOL]),
                   "prot": load_const("prot_xT", [16, NC_PROT])}
        sb_pmat = {}
        for s in sides:
            sd = sides[s]
            t = const.tile([128, sd["nblk"], B], F32, name=f"pmat_{s}")
            nc.sync.dma_start(
                t[:], dram[f"{s}_pmat"].rearrange("(t p) g -> p t g", p=128))
            sb_pmat[s] = t

        # ---------------- DRAM internals
        dpool = tc.alloc_tile_pool(name="dram", bufs=1, space="DRAM")
        x_sh_d = {s: [dpool.tile([sides[s]["NC"], 64], F32,
                                 name=f"xsh_{s}_{l}") for l in range(4)]
                  for s in sides}
        x_full_d = {s: [dpool.tile([sides[s]["N"], 64], F32,
                                   addr_space="Shared", name=f"xfull_{s}_{l}")
                        for l in range(4)] for s in sides}

        # ---------------- long-lived x pools, then GINE-scoped pools
        xT_pool = tc.alloc_tile_pool(name="xT", bufs=2)
        xnf_pool = tc.alloc_tile_pool(name="xnf", bufs=2)
        gmem = tc.alloc_tile_pool(name="gmem", bufs=1)
        empp = tc.alloc_tile_pool(name="empp", bufs=2, space="PSUM")
        aggps = tc.alloc_tile_pool(name="aggps", bufs=2, space="PSUM")
        mlpps = tc.alloc_tile_pool(name="mlpps", bufs=2, space="PSUM")
        trps = tc.alloc_tile_pool(name="trps", bufs=2, space="PSUM")

        # edge features em = [eattr;1] @ [We;be], edge-major [128, T, 64],
        # eattr^T streamed from DRAM per block
        ea_stream = tc.alloc_tile_pool(name="ea_stream", bufs=2)
        em_sb = {}
        for s in sides:
            sd = sides[s]
            T_total, T_blk, nblk, D = (sd["T_total"], sd["T_blk"], sd["nblk"],
                                       sd["D"])
            em = gmem.tile([128, T_total, 64], F32, name=f"em_{s}")
            for b in range(nblk):
                ch = ea_stream.tile([11, T_blk * 128], F32, name="ea_chunk")
                nc.sync.dma_start(
                    ch[:],
                    dram[f"{s}_eaT"][:, b * T_blk * 128:(b + 1) * T_blk * 128])
                for t0 in range(0, T_blk, 8):
                    ng = min(8, T_blk - t0)
                    ps = empp.tile([128, 8, 64], F32, name="em_ps")
                    for j in range(ng):
                        nc.tensor.matmul(
                            ps[:, j, :],
                            ch[0:D + 1, (t0 + j) * 128:(t0 + j + 1) * 128],
                            We[s][:], start=True, stop=True)
                    nc.vector.tensor_copy(
                        em[:, b * T_blk + t0:b * T_blk + t0 + ng, :],
                        ps[:, 0:ng, :])
            em_sb[s] = em
        ea_stream.release()
        xg_pool = tc.alloc_tile_pool(name="xg", bufs=2)
        oh_pool = tc.alloc_tile_pool(name="oh", bufs=2)

        # initial node features x0
        xT_cur = {}
        xnf_cur = {}
        for s in sides:
            sd = sides[s]
            NCs, nblk = sd["NC"], sd["nblk"]
            ps = mlpps.tile([64, 512], F32, name="mlp_ps")
            nc.tensor.matmul(ps[:, 0:NCs], Wn[s][:], sb_xTin[s][:],
                             start=True, stop=True)
            xT = xT_pool.tile([65, NCs], F32, name=f"xT_{s}")
            nc.vector.tensor_copy(xT[0:64, :], ps[:, 0:NCs])
            nc.vector.memset(xT[64:65, :], 1.0)
            xnf = xnf_pool.tile([128, nblk, 64], F32, name=f"xnf_{s}")
            for b in range(nblk):
                tp = trps.tile([128, 64], F32, name="tr_ps")
                nc.tensor.transpose(tp[:], xT[0:64, b * 128:(b + 1) * 128],
                                    ident_sb[0:64, 0:64])
                nc.vector.tensor_copy(xnf[:, b, :], tp[:])
            nc.sync.dma_start(
                x_sh_d[s][0][:].rearrange("(t p) f -> p t f", p=128), xnf[:])
            nc.gpsimd.collective_compute(
                "AllGather", ALU.bypass, replica_groups=[list(range(R))],
                ins=[x_sh_d[s][0][:].opt()], outs=[x_full_d[s][0][:].opt()])
            xT_cur[s] = xT
            xnf_cur[s] = xnf

        # GINE layers
        for l in range(3):
            for s in ("prot", "mol"):
                sd = sides[s]
                NCs, nblk, T_blk = sd["NC"], sd["nblk"], sd["T_blk"]
                xT_prev = xT_cur[s]
                hT = gmem.tile([65, NCs], F32, name=f"hT_{s}_{l}", bufs=2,
                               tag=f"hT_{s}")
                for b in range(nblk):
                    nE = T_blk * 128
                    xg = xg_pool.tile([128, T_blk, 64], F32, name="xg")
                    nc.gpsimd.dma_gather(
                        xg[:], x_full_d[s][l][:],
                        sb_idx[s][:, b * T_blk * 8:(b + 1) * T_blk * 8],
                        nE, nE, 64, single_packet=False)
                    msg = xg_pool.tile([128, T_blk, 64], F32, name="msg")
                    nc.vector.tensor_add(
                        msg[:], xg[:], em_sb[s][:, b * T_blk:(b + 1) * T_blk, :])
                    nc.scalar.activation(msg[:], msg[:], AF.Relu)
                    oh = oh_pool.tile([128, T_blk, 128], F32, name="oh")
                    nc.vector.tensor_tensor(
                        oh[:],
                        iota_sb[:, :].unsqueeze(1).broadcast_to([128, T_blk, 128]),
                        sb_dstoff[s][:, b * T_blk:(b + 1) * T_blk]
                            .unsqueeze(2).broadcast_to([128, T_blk, 128]),
                        ALU.is_equal)
                    agg = aggps.tile([64, 128], F32, name="agg_ps")
                    for t in range(T_blk):
                        nc.tensor.matmul(agg[:], msg[:, t, :], oh[:, t, :],
                                         start=(t == 0), stop=(t == T_blk - 1))
                    nc.vector.tensor_add(hT[0:64, b * 128:(b + 1) * 128],
                                         xT_prev[0:64, b * 128:(b + 1) * 128],
                                         agg[:])
                nc.vector.memset(hT[64:65, :], 1.0)
                ps1 = mlpps.tile([64, 512], F32, name="mlp_ps")
                nc.tensor.matmul(ps1[:, 0:NCs], W1[s][l][:], hT[:],
                                 start=True, stop=True)
                r1 = gmem.tile([65, NCs], F32, name=f"r1_{s}_{l}", bufs=2,
                               tag=f"r1_{s}")
                nc.scalar.activation(r1[0:64, :], ps1[:, 0:NCs], AF.Relu)
                nc.vector.memset(r1[64:65, :], 1.0)
                ps2 = mlpps.tile([64, 512], F32, name="mlp_ps")
                nc.tensor.matmul(ps2[:, 0:NCs], W2[s][l][:], r1[:],
                                 start=True, stop=True)
                xT = xT_pool.tile([65, NCs], F32, name=f"xT_{s}")
                nc.scalar.activation(xT[0:64, :], ps2[:, 0:NCs], AF.Relu)
                nc.vector.memset(xT[64:65, :], 1.0)
                xnf = xnf_pool.tile([128, nblk, 64], F32, name=f"xnf_{s}")
                for b in range(nblk):
                    tp = trps.tile([128, 64], F32, name="tr_ps")
                    nc.tensor.transpose(tp[:], xT[0:64, b * 128:(b + 1) * 128],
                                        ident_sb[0:64, 0:64])
                    nc.vector.tensor_copy(xnf[:, b, :], tp[:])
                nc.sync.dma_start(
                    x_sh_d[s][l + 1][:].rearrange("(t p) f -> p t f", p=128),
                    xnf[:])
                nc.gpsimd.collective_compute(
                    "AllGather", ALU.bypass, replica_groups=[list(range(R))],
                    ins=[x_sh_d[s][l + 1][:].opt()],
                    outs=[x_full_d[s][l + 1][:].opt()])
                xT_cur[s] = xT
                xnf_cur[s] = xnf

        # close GINE-scoped pools (LIFO per space)
        oh_pool.release()
        xg_pool.release()
        for p in (trps, mlpps, aggps, empp):
            p.release()
        gmem.release()

        # ---------------- attention phase
        a_sb = tc.alloc_tile_pool(name="attn_sb", bufs=1)
        smallps = tc.alloc_tile_pool(name="smallps", bufs=2, space="PSUM")
        s12ps = tc.alloc_tile_pool(name="s12ps", bufs=2, space="PSUM")
        ops = tc.alloc_tile_pool(name="ops", bufs=4, space="PSUM")
        exp_pool = tc.alloc_tile_pool(name="expt", bufs=10)
        WAVE = 8

        # full x (both sides), transposed with ones row
        xT_full = {}
        for s in sides:
            sd = sides[s]
            Ns = sd["N"]
            nt = Ns // 128
            xT_f = a_sb.tile([65, Ns], F32, name=f"xTfull_{s}")
            for t in range(nt):
                xf_nf = a_sb.tile([128, 64], F32, name="xf_nf", bufs=3,
                                  tag="xf_nf")
                nc.sync.dma_start(
                    xf_nf[:], x_full_d[s][3][t * 128:(t + 1) * 128, :])
                tp = smallps.tile([128, 512], F32, name="small_ps")
                nc.tensor.transpose(tp[0:64, 0:128], xf_nf[:], ident_sb[:])
                nc.vector.tensor_copy(xT_f[0:64, t * 128:(t + 1) * 128],
                                      tp[0:64, 0:128])
            nc.vector.memset(xT_f[64:65, :], 1.0)
            xT_full[s] = xT_f

        H_sb = {}
        for dirn, (qs, ks) in (("mp", ("mol", "prot")), ("pm", ("prot", "mol"))):
            qd, kd = sides[qs], sides[ks]
            NCq, Nk = qd["NC"], kd["N"]
            n_qt = NCq // 128
            n_k512 = Nk // 512
            n_k128 = Nk // 128
            Wd = dram[f"attn_{dirn}_W"]
            bd = dram[f"attn_{dirn}_b"]

            Wq = a_sb.tile([65, 64], F32, name=f"Wq_{dirn}")
            nc.sync.dma_start(Wq[0:64, :], Wd[0])
            nc.sync.dma_start(Wq[64:65, :], bd[0][None, :])
            Wv = a_sb.tile([65, 64], F32, name=f"Wv_{dirn}")
            nc.sync.dma_start(Wv[0:64, :], Wd[2])
            nc.sync.dma_start(Wv[64:65, :], bd[2][None, :])
            Wk_raw = a_sb.tile([64, 64], F32, name=f"Wkraw_{dirn}")
            nc.sync.dma_start(Wk_raw[:], Wd[1])
            bk_cols = a_sb.tile([16, 4], F32, name=f"bkcols_{dirn}")
            nc.sync.dma_start(bk_cols[:], dram[f"bk_{dirn}_cols"][:])

            # rhs0_h = [R_h ; c_h]: folded K-side coefficients per head.
            # s^T chunk = rhs0_h^T @ xT_full  gives [q, k] scores (pass 1);
            # with row 64 -= m_h it gives s~^T in [k, q] (pass 2).
            QT, rhs0 = [], []
            cT = a_sb.tile([1, HEADS, NCq], F32, name=f"cT_{dirn}")
            for h in range(HEADS):
                tp = smallps.tile([128, 512], F32, name="small_ps")
                nc.tensor.transpose(tp[0:16, 0:64],
                                    Wk_raw[:, 16 * h:16 * h + 16],
                                    ident_sb[0:64, 0:64])
                wkt = a_sb.tile([16, 64], F32, name="wkt", bufs=2, tag="wkt")
                nc.vector.tensor_copy(wkt[:], tp[0:16, 0:64])

                ps = smallps.tile([128, 512], F32, name="small_ps")
                nc.tensor.matmul(ps[0:16, 0:NCq],
                                 Wq[:, 16 * h:16 * h + 16], xT_cur[qs][:],
                                 start=True, stop=True)
                qt_ = a_sb.tile([16, NCq], F32, name=f"QT_{dirn}_{h}")
                nc.scalar.activation(qt_[:], ps[0:16, 0:NCq], AF.Copy,
                                     scale=0.25)
                QT.append(qt_)

                psR = smallps.tile([128, 512], F32, name="small_ps")
                nc.tensor.matmul(psR[0:64, 0:NCq], wkt[:], qt_[:],
                                 start=True, stop=True)
                psC = smallps.tile([128, 512], F32, name="small_ps")
                nc.tensor.matmul(psC[0:1, 0:NCq], bk_cols[:, h:h + 1],
                                 qt_[:], start=True, stop=True)
                r0 = a_sb.tile([65, NCq], F32, name=f"rhs0_{dirn}_{h}")
                nc.vector.tensor_copy(r0[0:64, :], psR[0:64, 0:NCq])
                nc.vector.tensor_copy(r0[64:65, :], psC[0:1, 0:NCq])
                nc.vector.tensor_copy(cT[0:1, h, :], psC[0:1, 0:NCq])
                rhs0.append(r0)

            # V' [128, n_k128, 4, 17] with ones col
            Vp = a_sb.tile([128, n_k128, HEADS, 17], F32, name=f"Vp_{dirn}")
            nc.vector.memset(Vp[:, :, :, 16:17], 1.0)
            for kt in range(n_k128):
                ps = smallps.tile([128, 512], F32, name="small_ps")
                nc.tensor.matmul(ps[0:128, 0:64],
                                 xT_full[ks][:, kt * 128:(kt + 1) * 128],
                                 Wv[:], start=True, stop=True)
                nc.vector.tensor_copy(
                    Vp[:, kt, :, 0:16],
                    ps[0:128, 0:64].rearrange("p (h d) -> p h d", h=HEADS))

            # pass 1: exact row max m_h [1, NCq] per head ([q, k] layout)
            mT = a_sb.tile([1, HEADS, NCq], F32, name=f"mT_{dirn}")
            for h in range(HEADS):
                for qt in range(n_qt):
                    mx = a_sb.tile([128, n_k512], F32, name="mx", bufs=2,
                                   tag="mx")
                    for cch in range(n_k512):
                        s1 = s12ps.tile([128, 512], F32, name="s12_ps")
                        nc.tensor.matmul(
                            s1[:],
                            rhs0[h][:, qt * 128:(qt + 1) * 128],
                            xT_full[ks][:, cch * 512:(cch + 1) * 512],
                            start=True, stop=True)
                        nc.vector.reduce_max(mx[:, cch:cch + 1], s1[:],
                                             axis=mybir.AxisListType.X)
                    mqt = a_sb.tile([128, 1], F32, name="mqt", bufs=2, tag="mqt")
                    nc.vector.reduce_max(mqt[:], mx[:], axis=mybir.AxisListType.X)
                    tp = smallps.tile([128, 512], F32, name="small_ps")
                    nc.tensor.transpose(tp[0:1, 0:128], mqt[:], ident_sb[:])
                    nc.vector.tensor_copy(
                        mT[0:1, h, qt * 128:(qt + 1) * 128], tp[0:1, 0:128])

            # pass 2 + wV, processed in waves of WAVE k-chunks
            H = a_sb.tile([128, n_qt, 64], F32, name=f"H_{dirn}")
            for h in range(HEADS):
                rhs = a_sb.tile([65, NCq], F32, name="rhs", bufs=2, tag="rhs")
                nc.vector.tensor_copy(rhs[0:64, :], rhs0[h][0:64, :])
                cm = a_sb.tile([1, NCq], F32, name="cm", bufs=2, tag="cm")
                nc.vector.tensor_sub(cm[:], cT[0:1, h, :], mT[0:1, h, :])
                nc.vector.tensor_copy(rhs[64:65, :], cm[:])
                o_tiles = [ops.tile([128, 17], F32, name="o_ps")
                           for _ in range(n_qt)]
                for w0 in range(0, n_k128, WAVE):
                    nw = min(WAVE, n_k128 - w0)
                    exs = []
                    for j in range(nw):
                        kc = w0 + j
                        s2 = s12ps.tile([128, 512], F32, name="s12_ps")
                        nc.tensor.matmul(
                            s2[:, 0:NCq],
                            xT_full[ks][:, kc * 128:(kc + 1) * 128],
                            rhs[:], start=True, stop=True)
                        ex = exp_pool.tile([128, NCq], F32, name="ex",
                                           tag=f"ex_{dirn}")
                        nc.scalar.activation(ex[:], s2[:, 0:NCq], AF.Exp)
                        exs.append(ex)
                    for qt in range(n_qt):
                        for j in range(nw):
                            kc = w0 + j
                            nc.tensor.matmul(
                                o_tiles[qt][:],
                                exs[j][:, qt * 128:(qt + 1) * 128],
                                Vp[:, kc, h, :],
                                start=(kc == 0), stop=(kc == n_k128 - 1))
                for qt in range(n_qt):
                    inv1 = a_sb.tile([128, 1], F32, name="inv1", bufs=2,
                                     tag="inv1")
                    nc.vector.reciprocal(inv1[:], o_tiles[qt][:, 16:17])
                    nc.vector.tensor_scalar_mul(
                        H[:, qt, 16 * h:16 * (h + 1)], o_tiles[qt][:, 0:16],
                        inv1[:])

            # residual: H += x (node-major shard)
            nc.vector.tensor_add(H[:], H[:], xnf_cur[qs][:])
            H_sb[dirn] = H

        # ---------------- pooling + output MLP
        zt_part_d = dpool.tile([128, B], F32, name="zt_part")
        zt_full_d = dpool.tile([128, B], F32, addr_space="Shared",
                               name="zt_full")
        for dirn, qs in (("mp", "mol"), ("pm", "prot")):
            n_qt = sides[qs]["NC"] // 128
            psz = smallps.tile([128, 512], F32, name="small_ps")
            for qt in range(n_qt):
                nc.tensor.matmul(psz[0:64, 0:B], H_sb[dirn][:, qt, :],
                                 sb_pmat[qs][:, qt, :],
                                 start=(qt == 0), stop=(qt == n_qt - 1))
            zpart = a_sb.tile([64, B], F32, name=f"zpart_{dirn}")
            nc.vector.tensor_copy(zpart[:], psz[0:64, 0:B])
            row0 = 0 if dirn == "mp" else 64
            nc.sync.dma_start(zt_part_d[row0:row0 + 64, :], zpart[:])
        nc.gpsimd.collective_compute(
            "AllReduce", ALU.add, replica_groups=[list(range(R))],
            ins=[zt_part_d[:].opt()], outs=[zt_full_d[:].opt()])
        zT = a_sb.tile([128, B], F32, name="zT")
        nc.sync.dma_start(zT[:], zt_full_d[:])

        fc1W = a_sb.tile([128, 64], F32, name="fc1W")
        nc.sync.dma_start(fc1W[:], dram["fc1_W"][:])
        fc1b = a_sb.tile([64, 1], F32, name="fc1b")
        nc.sync.dma_start(fc1b[:], dram["fc1_b"][:, None])
        fc2W = a_sb.tile([64, 1], F32, name="fc2W")
        nc.sync.dma_start(fc2W[:], dram["fc2_W"][:])
        fc2b = a_sb.tile([1, 1], F32, name="fc2b")
        nc.sync.dma_start(fc2b[:], dram["fc2_b"][:, None])

        ps = smallps.tile([128, 512], F32, name="small_ps")
        nc.tensor.matmul(ps[0:64, 0:B], fc1W[:], zT[:], start=True, stop=True)
        h1 = a_sb.tile([65, B], F32, name="h1")
        nc.scalar.activation(h1[0:64, :], ps[0:64, 0:B], AF.Relu, bias=fc1b[:])
        ps2 = smallps.tile([128, 512], F32, name="small_ps")
        nc.tensor.matmul(ps2[0:1, 0:B], fc2W[:], h1[0:64, :],
                         start=True, stop=True)
        osb = a_sb.tile([1, B], F32, name="osb")
        nc.scalar.activation(osb[:], ps2[0:1, 0:B], AF.Sigmoid, bias=fc2b[:])
        nc.sync.dma_start(out_d[:], osb[:])

        exp_pool.release()
        ops.release()
        s12ps.release()
        smallps.release()
        a_sb.release()
        xnf_pool.release()
        xT_pool.release()
        dpool.release()
        const.release()

    nc.compile()
    return nc



# ----------------------------------------------------------------- entry

def kernel(**inputs):
    global last_results
    meta, percore = _prep_host(inputs)
    key = (meta["mol_T_blk"], meta["prot_T_blk"])
    if key not in _CACHE:
        _CACHE[key] = _build(meta)
    nc = _CACHE[key]
    from concourse.bass_utils import run_bass_kernel_spmd
    res = run_bass_kernel_spmd(nc, percore, list(range(R)))
    last_results = res
    return np.asarray(res.results[0]["out"], np.float32).reshape(B)



# revision 3
# speedup vs baseline: 1.0023x; 1.0023x over previous
"""CrossGraphAttentionModel on 8 Trainium2 NeuronCores (Bass/Tile, SPMD).

Sharding: nodes/edges of both graphs are sharded 8 ways by (dst-sorted) node
range; 64-dim weights replicated. Per GINE layer each core dma_gathers x[src]
for its edge shard from an AllGathered copy of x in HBM, forms messages on
DVE/ACT, and scatter-adds them with one-hot matmuls on the PE (PSUM
accumulation), then runs the node MLP on its node shard and AllGathers the new
x. Cross-graph attention shards the query axis: scores are computed twice on
PE - once [q,k] for an exact row max, once [k,q] with the max folded into the
contraction via an appended ones row - so softmax needs only a single ACT exp
pass, and the exp tiles feed the wV matmul directly as lhsT with a ones column
in V producing the softmax denominator for free. Graph pooling is a one-hot
matmul with 1/count weights, AllReduced, followed by the tiny output MLP.

All floating point math runs on device in fp32; the host only sorts/pads
integer index structures and transposes/replicates input layouts.
"""

import numpy as np

R = 8
HID = 64
B = 32
HEADS = 4
HD = 16
N_MOL, N_PROT = 2048, 4096
E_MOL, E_PROT = 32768, 131072
NC_MOL, NC_PROT = N_MOL // R, N_PROT // R          # 256, 512
NBLK_MOL, NBLK_PROT = NC_MOL // 128, NC_PROT // 128  # 2, 4

_CACHE = {}
last_results = None


# ----------------------------------------------------------------- host prep

def _prep_edges(edge_index, eattr, nblk):
    """Sort edges by dst, partition into R cores x nblk 128-node windows,
    pad every window to T_blk tiles of 128 edges. Returns device layouts."""
    src, dst = np.asarray(edge_index[0]), np.asarray(edge_index[1])
    eattr = np.asarray(eattr, np.float32)
    order = np.argsort(dst, kind="stable")
    src_s, dst_s, ea_s = src[order], dst[order], eattr[order]
    nblocks = R * nblk
    blk = dst_s // 128
    counts = np.bincount(blk, minlength=nblocks)
    T_blk = int(np.ceil(counts.max() / 128))
    T_total = nblk * T_blk
    E_core = T_total * 128
    D = eattr.shape[1]

    gidx = np.zeros((R, E_core), np.int64)
    dstoff = np.full((R, E_core), -1.0, np.float32)
    ea_pad = np.zeros((R, E_core, D), np.float32)
    starts = np.concatenate([[0], np.cumsum(counts)])
    for c in range(R):
        for b in range(nblk):
            g = c * nblk + b
            cnt = counts[g]
            lo = starts[g]
            off = b * T_blk * 128
            gidx[c, off:off + cnt] = src_s[lo:lo + cnt]
            dstoff[c, off:off + cnt] = (dst_s[lo:lo + cnt] - g * 128)
            ea_pad[c, off:off + cnt] = ea_s[lo:lo + cnt]

    # gather indices wrapped [128, E_core/16] (i -> p=i%16, col=i//16), x8 replicated
    cols = E_core // 16
    w = gidx.reshape(R, cols, 16).transpose(0, 2, 1).astype(np.int16)
    gidx_sb = np.tile(w, (1, 8, 1)).copy()
    # dstoff [128, T_total]
    dstoff_sb = np.ascontiguousarray(
        dstoff.reshape(R, T_total, 128).transpose(0, 2, 1))
    # eattr^T with ones row: [11, E_core]
    eaT_packed = np.ascontiguousarray(np.concatenate(
        [ea_pad.transpose(0, 2, 1),
         np.ones((R, 1, E_core), np.float32)], axis=1))
    return dict(T_blk=T_blk, T_total=T_total, E_core=E_core, D=D,
                gidx_sb=gidx_sb, dstoff_sb=dstoff_sb, eaT_packed=eaT_packed)


def _prep_host(inp):
    """All integer/layout preprocessing. Returns (meta, per_core_inputs)."""
    mol = _prep_edges(inp["mol_edge_index"], inp["mol_eattr"], NBLK_MOL)
    prot = _prep_edges(inp["prot_edge_index"], inp["prot_eattr"], NBLK_PROT)

    # pool matrices with 1/count entries
    def pmat(batch, ncore):
        batch = np.asarray(batch)
        cnt = np.bincount(batch, minlength=B).astype(np.float32)
        inv = 1.0 / np.maximum(cnt, 1.0)
        m = np.zeros((R, ncore, B), np.float32)
        for c in range(R):
            sl = batch[c * ncore:(c + 1) * ncore]
            m[c, np.arange(ncore), sl] = inv[sl]
        return m

    mol_pmat = pmat(inp["mol_batch"], NC_MOL)
    prot_pmat = pmat(inp["prot_batch"], NC_PROT)

    # node features transposed per core with ones row
    def xt(x, ncore):
        x = np.asarray(x, np.float32)
        d = x.shape[1]
        out = np.zeros((R, d + 1, ncore), np.float32)
        for c in range(R):
            out[c, :d] = x[c * ncore:(c + 1) * ncore].T
            out[c, d] = 1.0
        return out

    mol_xT = xt(inp["mol_x"], NC_MOL)        # [R, 12, 256]
    prot_xT = xt(inp["prot_x"], NC_PROT)     # [R, 16, 512]

    iota = np.tile(np.arange(128, dtype=np.float32), (128, 1))
    ident = np.eye(128, dtype=np.float32)

    # attn K-bias as per-head columns [16, 4]
    def bcols(b):  # [64] -> [16, 4]
        return np.ascontiguousarray(np.asarray(b, np.float32).reshape(4, 16).T)

    percore = []
    for c in range(R):
        m = {
            "mol_xT": mol_xT[c], "prot_xT": prot_xT[c],
            "mol_eaT": mol["eaT_packed"][c], "prot_eaT": prot["eaT_packed"][c],
            "mol_gidx": mol["gidx_sb"][c], "prot_gidx": prot["gidx_sb"][c],
            "mol_dstoff": mol["dstoff_sb"][c], "prot_dstoff": prot["dstoff_sb"][c],
            "mol_pmat": mol_pmat[c], "prot_pmat": prot_pmat[c],
            "iota": iota, "ident": ident,
            "bk_mp_cols": bcols(np.asarray(inp["attn_mp_b"])[1]),
            "bk_pm_cols": bcols(np.asarray(inp["attn_pm_b"])[1]),
        }
        for k in ("node_lin_mol_W", "node_lin_mol_b", "node_lin_prot_W",
                  "node_lin_prot_b", "edge_lin_mol_W", "edge_lin_mol_b",
                  "edge_lin_prot_W", "edge_lin_prot_b",
                  "mol_conv_W1", "mol_conv_b1", "mol_conv_W2", "mol_conv_b2",
                  "prot_conv_W1", "prot_conv_b1", "prot_conv_W2", "prot_conv_b2",
                  "attn_mp_W", "attn_mp_b", "attn_pm_W", "attn_pm_b",
                  "fc1_W", "fc1_b", "fc2_W", "fc2_b"):
            m[k] = np.asarray(inp[k], np.float32)
        percore.append(m)

    meta = dict(mol_T_blk=mol["T_blk"], mol_T_total=mol["T_total"],
                mol_E_core=mol["E_core"],
                prot_T_blk=prot["T_blk"], prot_T_total=prot["T_total"],
                prot_E_core=prot["E_core"])
    return meta, percore


# ------------------------------------------------------------- device build

def _build(meta):
    import concourse.bacc as bacc
    import concourse.mybir as mybir
    import concourse.tile as tile

    F32 = mybir.dt.float32
    I16 = mybir.dt.int16
    AF = mybir.ActivationFunctionType
    ALU = mybir.AluOpType

    nc = bacc.Bacc("TRN2", target_bir_lowering=False, debug=False,
                   num_devices=R)

    # ---- I/O declarations
    dram = {}

    def din(name, shape, dtype=F32):
        dram[name] = nc.dram_tensor(name, list(shape), dtype,
                                    kind="ExternalInput")
        return dram[name]

    mT, mE = meta["mol_T_total"], meta["mol_E_core"]
    pT, pE = meta["prot_T_total"], meta["prot_E_core"]

    din("mol_xT", [12, NC_MOL]); din("prot_xT", [16, NC_PROT])
    din("mol_eaT", [11, mE]); din("prot_eaT", [11, pE])
    din("mol_gidx", [128, mE // 16], I16); din("prot_gidx", [128, pE // 16], I16)
    din("mol_dstoff", [128, mT]); din("prot_dstoff", [128, pT])
    din("mol_pmat", [NC_MOL, B]); din("prot_pmat", [NC_PROT, B])
    din("iota", [128, 128]); din("ident", [128, 128])
    din("bk_mp_cols", [16, 4]); din("bk_pm_cols", [16, 4])
    din("node_lin_mol_W", [11, 64]); din("node_lin_mol_b", [64])
    din("node_lin_prot_W", [15, 64]); din("node_lin_prot_b", [64])
    din("edge_lin_mol_W", [10, 64]); din("edge_lin_mol_b", [64])
    din("edge_lin_prot_W", [10, 64]); din("edge_lin_prot_b", [64])
    for s in ("mol", "prot"):
        din(f"{s}_conv_W1", [3, 64, 64]); din(f"{s}_conv_b1", [3, 64])
        din(f"{s}_conv_W2", [3, 64, 64]); din(f"{s}_conv_b2", [3, 64])
    din("attn_mp_W", [3, 64, 64]); din("attn_mp_b", [3, 64])
    din("attn_pm_W", [3, 64, 64]); din("attn_pm_b", [3, 64])
    din("fc1_W", [128, 64]); din("fc1_b", [64])
    din("fc2_W", [64, 1]); din("fc2_b", [1])

    out_d = nc.dram_tensor("out", [1, B], F32, kind="ExternalOutput")

    sides = {
        "mol": dict(N=N_MOL, NC=NC_MOL, nblk=NBLK_MOL, T_blk=meta["mol_T_blk"],
                    T_total=mT, E_core=mE, D=10),
        "prot": dict(N=N_PROT, NC=NC_PROT, nblk=NBLK_PROT,
                     T_blk=meta["prot_T_blk"], T_total=pT, E_core=pE, D=10),
    }

    with tile.TileContext(nc) as tc:
        # ---------------- persistent SBUF constants
        const = tc.alloc_tile_pool(name="const", bufs=1)

        def load_const(name, shape, dtype=F32, src=None):
            t = const.tile(list(shape), dtype, name=f"c_{name}")
            nc.sync.dma_start(t[:], (dram[name] if src is None else src)[:])
            return t

        iota_sb = load_const("iota", [128, 128])
        ident_sb = load_const("ident", [128, 128])

        def wcat(name_w, name_b, din_, dout, wslice=None, bslice=None):
            t = const.tile([din_ + 1, dout], F32, name=f"w_{name_w}_{wslice}")
            wsrc = dram[name_w] if wslice is None else dram[name_w][wslice]
            bsrc = dram[name_b] if bslice is None else dram[name_b][bslice]
            nc.sync.dma_start(t[0:din_, :], wsrc[:, :] if wslice is None else wsrc)
            nc.sync.dma_start(t[din_:din_ + 1, :], bsrc[None, :])
            return t

        Wn = {"mol": wcat("node_lin_mol_W", "node_lin_mol_b", 11, 64),
              "prot": wcat("node_lin_prot_W", "node_lin_prot_b", 15, 64)}
        We = {"mol": wcat("edge_lin_mol_W", "edge_lin_mol_b", 10, 64),
              "prot": wcat("edge_lin_prot_W", "edge_lin_prot_b", 10, 64)}
        W1 = {s: [wcat(f"{s}_conv_W1", f"{s}_conv_b1", 64, 64, l, l)
                  for l in range(3)] for s in ("mol", "prot")}
        W2 = {s: [wcat(f"{s}_conv_W2", f"{s}_conv_b2", 64, 64, l, l)
                  for l in range(3)] for s in ("mol", "prot")}

        sb_idx, sb_dstoff = {}, {}
        for s in sides:
            sd = sides[s]
            sb_idx[s] = load_const(f"{s}_gidx", [128, sd["E_core"] // 16], I16)
            sb_dstoff[s] = load_const(f"{s}_dstoff", [128, sd["T_total"]])
        sb_xTin = {"mol": load_const("mol_xT", [12, NC_MOL]),
                   "prot": load_const("prot_xT", [16, NC_PROT])}
        sb_pmat = {}
        for s in sides:
            sd = sides[s]
            t = const.tile([128, sd["nblk"], B], F32, name=f"pmat_{s}")
            nc.sync.dma_start(
                t[:], dram[f"{s}_pmat"].rearrange("(t p) g -> p t g", p=128))
            sb_pmat[s] = t

        # ---------------- DRAM internals
        dpool = tc.alloc_tile_pool(name="dram", bufs=1, space="DRAM")
        x_sh_d = {s: [dpool.tile([sides[s]["NC"], 64], F32,
                                 name=f"xsh_{s}_{l}") for l in range(4)]
                  for s in sides}
        x_full_d = {s: [dpool.tile([sides[s]["N"], 64], F32,
                                   addr_space="Shared", name=f"xfull_{s}_{l}")
                        for l in range(4)] for s in sides}

        # ---------------- long-lived x pools, then GINE-scoped pools
        xT_pool = tc.alloc_tile_pool(name="xT", bufs=2)
        xnf_pool = tc.alloc_tile_pool(name="xnf", bufs=2)
        gmem = tc.alloc_tile_pool(name="gmem", bufs=1)
        empp = tc.alloc_tile_pool(name="empp", bufs=2, space="PSUM")
        aggps = tc.alloc_tile_pool(name="aggps", bufs=2, space="PSUM")
        mlpps = tc.alloc_tile_pool(name="mlpps", bufs=2, space="PSUM")
        trps = tc.alloc_tile_pool(name="trps", bufs=2, space="PSUM")

        # edge features em = [eattr;1] @ [We;be], edge-major [128, T, 64],
        # eattr^T streamed from DRAM per block
        ea_stream = tc.alloc_tile_pool(name="ea_stream", bufs=2)
        em_sb = {}
        for s in sides:
            sd = sides[s]
            T_total, T_blk, nblk, D = (sd["T_total"], sd["T_blk"], sd["nblk"],
                                       sd["D"])
            em = gmem.tile([128, T_total, 64], F32, name=f"em_{s}")
            for b in range(nblk):
                ch = ea_stream.tile([11, T_blk * 128], F32, name="ea_chunk")
                nc.sync.dma_start(
                    ch[:],
                    dram[f"{s}_eaT"][:, b * T_blk * 128:(b + 1) * T_blk * 128])
                for t0 in range(0, T_blk, 8):
                    ng = min(8, T_blk - t0)
                    ps = empp.tile([128, 8, 64], F32, name="em_ps")
                    for j in range(ng):
                        nc.tensor.matmul(
                            ps[:, j, :],
                            ch[0:D + 1, (t0 + j) * 128:(t0 + j + 1) * 128],
                            We[s][:], start=True, stop=True)
                    nc.vector.tensor_copy(
                        em[:, b * T_blk + t0:b * T_blk + t0 + ng, :],
                        ps[:, 0:ng, :])
            em_sb[s] = em
        ea_stream.release()
        xg_pool = tc.alloc_tile_pool(name="xg", bufs=2)
        oh_pool = tc.alloc_tile_pool(name="oh", bufs=2)

        # initial node features x0
        xT_cur = {}
        xnf_cur = {}
        for s in sides:
            sd = sides[s]
            NCs, nblk = sd["NC"], sd["nblk"]
            ps = mlpps.tile([64, 512], F32, name="mlp_ps")
            nc.tensor.matmul(ps[:, 0:NCs], Wn[s][:], sb_xTin[s][:],
                             start=True, stop=True)
            xT = xT_pool.tile([65, NCs], F32, name=f"xT_{s}")
            nc.vector.tensor_copy(xT[0:64, :], ps[:, 0:NCs])
            nc.vector.memset(xT[64:65, :], 1.0)
            xnf = xnf_pool.tile([128, nblk, 64], F32, name=f"xnf_{s}")
            for b in range(nblk):
                tp = trps.tile([128, 64], F32, name="tr_ps")
                nc.tensor.transpose(tp[:], xT[0:64, b * 128:(b + 1) * 128],
                                    ident_sb[0:64, 0:64])
                nc.vector.tensor_copy(xnf[:, b, :], tp[:])
            nc.sync.dma_start(
                x_sh_d[s][0][:].rearrange("(t p) f -> p t f", p=128), xnf[:])
            nc.gpsimd.collective_compute(
                "AllGather", ALU.bypass, replica_groups=[list(range(R))],
                ins=[x_sh_d[s][0][:].opt()], outs=[x_full_d[s][0][:].opt()])
            xT_cur[s] = xT
            xnf_cur[s] = xnf

        # GINE layers
        for l in range(3):
            for s in ("prot", "mol"):
                sd = sides[s]
                NCs, nblk, T_blk = sd["NC"], sd["nblk"], sd["T_blk"]
                xT_prev = xT_cur[s]
                hT = gmem.tile([65, NCs], F32, name=f"hT_{s}_{l}", bufs=2,
                               tag=f"hT_{s}")
                for b in range(nblk):
                    nE = T_blk * 128
                    xg = xg_pool.tile([128, T_blk, 64], F32, name="xg")
                    nc.gpsimd.dma_gather(
                        xg[:], x_full_d[s][l][:],
                        sb_idx[s][:, b * T_blk * 8:(b + 1) * T_blk * 8],
                        nE, nE, 64, single_packet=False)
                    msg = xg_pool.tile([128, T_blk, 64], F32, name="msg")
                    nc.vector.tensor_add(
                        msg[:], xg[:], em_sb[s][:, b * T_blk:(b + 1) * T_blk, :])
                    nc.scalar.activation(msg[:], msg[:], AF.Relu)
                    oh = oh_pool.tile([128, T_blk, 128], F32, name="oh")
                    nc.vector.tensor_tensor(
                        oh[:],
                        iota_sb[:, :].unsqueeze(1).broadcast_to([128, T_blk, 128]),
                        sb_dstoff[s][:, b * T_blk:(b + 1) * T_blk]
                            .unsqueeze(2).broadcast_to([128, T_blk, 128]),
                        ALU.is_equal)
                    agg = aggps.tile([64, 128], F32, name="agg_ps")
                    for t in range(T_blk):
                        nc.tensor.matmul(agg[:], msg[:, t, :], oh[:, t, :],
                                         start=(t == 0), stop=(t == T_blk - 1))
                    nc.vector.tensor_add(hT[0:64, b * 128:(b + 1) * 128],
                                         xT_prev[0:64, b * 128:(b + 1) * 128],
                                         agg[:])
                nc.vector.memset(hT[64:65, :], 1.0)
                ps1 = mlpps.tile([64, 512], F32, name="mlp_ps")
                nc.tensor.matmul(ps1[:, 0:NCs], W1[s][l][:], hT[:],
                                 start=True, stop=True)
                r1 = gmem.tile([65, NCs], F32, name=f"r1_{s}_{l}", bufs=2,
                               tag=f"r1_{s}")
                nc.scalar.activation(r1[0:64, :], ps1[:, 0:NCs], AF.Relu)
                nc.vector.memset(r1[64:65, :], 1.0)
                ps2 = mlpps.tile([64, 512], F32, name="mlp_ps")
                nc.tensor.matmul(ps2[:, 0:NCs], W2[s][l][:], r1[:],
                                 start=True, stop=True)
                xT = xT_pool.tile([65, NCs], F32, name=f"xT_{s}")
                nc.scalar.activation(xT[0:64, :], ps2[:, 0:NCs], AF.Relu)
                nc.vector.memset(xT[64:65, :], 1.0)
                xnf = xnf_pool.tile([128, nblk, 64], F32, name=f"xnf_{s}")
                for b in range(nblk):
                    tp = trps.tile([128, 64], F32, name="tr_ps")
                    nc.tensor.transpose(tp[:], xT[0:64, b * 128:(b + 1) * 128],
                                        ident_sb[0:64, 0:64])
                    nc.vector.tensor_copy(xnf[:, b, :], tp[:])
                nc.sync.dma_start(
                    x_sh_d[s][l + 1][:].rearrange("(t p) f -> p t f", p=128),
                    xnf[:])
                nc.gpsimd.collective_compute(
                    "AllGather", ALU.bypass, replica_groups=[list(range(R))],
                    ins=[x_sh_d[s][l + 1][:].opt()],
                    outs=[x_full_d[s][l + 1][:].opt()])
                xT_cur[s] = xT
                xnf_cur[s] = xnf

        # close GINE-scoped pools (LIFO per space)
        oh_pool.release()
        xg_pool.release()
        for p in (trps, mlpps, aggps, empp):
            p.release()
        gmem.release()

        # ---------------- attention phase
        a_sb = tc.alloc_tile_pool(name="attn_sb", bufs=1)
        smallps = tc.alloc_tile_pool(name="smallps", bufs=2, space="PSUM")
        s12ps = tc.alloc_tile_pool(name="s12ps", bufs=2, space="PSUM")
        ops = tc.alloc_tile_pool(name="ops", bufs=4, space="PSUM")
        exp_pool = tc.alloc_tile_pool(name="expt", bufs=10)
        WAVE = 8

        # full x (both sides), transposed with ones row
        xT_full = {}
        for s in sides:
            sd = sides[s]
            Ns = sd["N"]
            nt = Ns // 128
            xT_f = a_sb.tile([65, Ns], F32, name=f"xTfull_{s}")
            for t in range(nt):
                xf_nf = a_sb.tile([128, 64], F32, name="xf_nf", bufs=3,
                                  tag="xf_nf")
                nc.sync.dma_start(
                    xf_nf[:], x_full_d[s][3][t * 128:(t + 1) * 128, :])
                tp = smallps.tile([128, 512], F32, name="small_ps")
                nc.tensor.transpose(tp[0:64, 0:128], xf_nf[:], ident_sb[:])
                nc.vector.tensor_copy(xT_f[0:64, t * 128:(t + 1) * 128],
                                      tp[0:64, 0:128])
            nc.vector.memset(xT_f[64:65, :], 1.0)
            xT_full[s] = xT_f

        H_sb = {}
        for dirn, (qs, ks) in (("mp", ("mol", "prot")), ("pm", ("prot", "mol"))):
            qd, kd = sides[qs], sides[ks]
            NCq, Nk = qd["NC"], kd["N"]
            n_qt = NCq // 128
            n_k512 = Nk // 512
            n_k128 = Nk // 128
            Wd = dram[f"attn_{dirn}_W"]
            bd = dram[f"attn_{dirn}_b"]

            Wq = a_sb.tile([65, 64], F32, name=f"Wq_{dirn}")
            nc.sync.dma_start(Wq[0:64, :], Wd[0])
            nc.sync.dma_start(Wq[64:65, :], bd[0][None, :])
            Wv = a_sb.tile([65, 64], F32, name=f"Wv_{dirn}")
            nc.sync.dma_start(Wv[0:64, :], Wd[2])
            nc.sync.dma_start(Wv[64:65, :], bd[2][None, :])
            Wk_raw = a_sb.tile([64, 64], F32, name=f"Wkraw_{dirn}")
            nc.sync.dma_start(Wk_raw[:], Wd[1])
            bk_cols = a_sb.tile([16, 4], F32, name=f"bkcols_{dirn}")
            nc.sync.dma_start(bk_cols[:], dram[f"bk_{dirn}_cols"][:])

            # rhs0_h = [R_h ; c_h]: folded K-side coefficients per head.
            # s^T chunk = rhs0_h^T @ xT_full  gives [q, k] scores (pass 1);
            # with row 64 -= m_h it gives s~^T in [k, q] (pass 2).
            QT, rhs0 = [], []
            cT = a_sb.tile([1, HEADS, NCq], F32, name=f"cT_{dirn}")
            for h in range(HEADS):
                tp = smallps.tile([128, 512], F32, name="small_ps")
                nc.tensor.transpose(tp[0:16, 0:64],
                                    Wk_raw[:, 16 * h:16 * h + 16],
                                    ident_sb[0:64, 0:64])
                wkt = a_sb.tile([16, 64], F32, name="wkt", bufs=2, tag="wkt")
                nc.vector.tensor_copy(wkt[:], tp[0:16, 0:64])

                ps = smallps.tile([128, 512], F32, name="small_ps")
                nc.tensor.matmul(ps[0:16, 0:NCq],
                                 Wq[:, 16 * h:16 * h + 16], xT_cur[qs][:],
                                 start=True, stop=True)
                qt_ = a_sb.tile([16, NCq], F32, name=f"QT_{dirn}_{h}")
                nc.scalar.activation(qt_[:], ps[0:16, 0:NCq], AF.Copy,
                                     scale=0.25)
                QT.append(qt_)

                psR = smallps.tile([128, 512], F32, name="small_ps")
                nc.tensor.matmul(psR[0:64, 0:NCq], wkt[:], qt_[:],
                                 start=True, stop=True)
                psC = smallps.tile([128, 512], F32, name="small_ps")
                nc.tensor.matmul(psC[0:1, 0:NCq], bk_cols[:, h:h + 1],
                                 qt_[:], start=True, stop=True)
                r0 = a_sb.tile([65, NCq], F32, name=f"rhs0_{dirn}_{h}")
                nc.vector.tensor_copy(r0[0:64, :], psR[0:64, 0:NCq])
                nc.vector.tensor_copy(r0[64:65, :], psC[0:1, 0:NCq])
                nc.vector.tensor_copy(cT[0:1, h, :], psC[0:1, 0:NCq])
                rhs0.append(r0)

            # V' [128, n_k128, 4, 17] with ones col
            Vp = a_sb.tile([128, n_k128, HEADS, 17], F32, name=f"Vp_{dirn}")
            nc.vector.memset(Vp[:, :, :, 16:17], 1.0)
            for kt in range(n_k128):
                ps = smallps.tile([128, 512], F32, name="small_ps")
                nc.tensor.matmul(ps[0:128, 0:64],
                                 xT_full[ks][:, kt * 128:(kt + 1) * 128],
                                 Wv[:], start=True, stop=True)
                nc.vector.tensor_copy(
                    Vp[:, kt, :, 0:16],
                    ps[0:128, 0:64].rearrange("p (h d) -> p h d", h=HEADS))

            # pass 1: exact row max m_h [1, NCq] per head ([q, k] layout)
            mT = a_sb.tile([1, HEADS, NCq], F32, name=f"mT_{dirn}")
            for h in range(HEADS):
                for qt in range(n_qt):
                    mx = a_sb.tile([128, n_k512], F32, name="mx", bufs=2,
                                   tag="mx")
                    for cch in range(n_k512):
                        s1 = s12ps.tile([128, 512], F32, name="s12_ps")
                        nc.tensor.matmul(
                            s1[:],
                            rhs0[h][:, qt * 128:(qt + 1) * 128],
                            xT_full[ks][:, cch * 512:(cch + 1) * 512],
                            start=True, stop=True)
                        nc.vector.reduce_max(mx[:, cch:cch + 1], s1[:],
                                             axis=mybir.AxisListType.X)
                    mqt = a_sb.tile([128, 1], F32, name="mqt", bufs=2, tag="mqt")
                    nc.vector.reduce_max(mqt[:], mx[:], axis=mybir.AxisListType.X)
                    tp = smallps.tile([128, 512], F32, name="small_ps")
                    nc.tensor.transpose(tp[0:1, 0:128], mqt[:], ident_sb[:])
                    nc.vector.tensor_copy(
                        mT[0:1, h, qt * 128:(qt + 1) * 128], tp[0:1, 0:128])

            # pass 2 + wV, processed in waves of WAVE k-chunks
            H = a_sb.tile([128, n_qt, 64], F32, name=f"H_{dirn}")
            for h in range(HEADS):
                rhs = a_sb.tile([65, NCq], F32, name="rhs", bufs=2, tag="rhs")
                nc.vector.tensor_copy(rhs[0:64, :], rhs0[h][0:64, :])
                cm = a_sb.tile([1, NCq], F32, name="cm", bufs=2, tag="cm")
                nc.vector.tensor_sub(cm[:], cT[0:1, h, :], mT[0:1, h, :])
                nc.vector.tensor_copy(rhs[64:65, :], cm[:])
                o_tiles = [ops.tile([128, 17], F32, name="o_ps")
                           for _ in range(n_qt)]
                for w0 in range(0, n_k128, WAVE):
                    nw = min(WAVE, n_k128 - w0)
                    exs = []
                    for j in range(nw):
                        kc = w0 + j
                        s2 = s12ps.tile([128, 512], F32, name="s12_ps")
                        nc.tensor.matmul(
                            s2[:, 0:NCq],
                            xT_full[ks][:, kc * 128:(kc + 1) * 128],
                            rhs[:], start=True, stop=True)
                        ex = exp_pool.tile([128, NCq], F32, name="ex",
                                           tag=f"ex_{dirn}")
                        nc.scalar.activation(ex[:], s2[:, 0:NCq], AF.Exp)
                        exs.append(ex)
                    for qt in range(n_qt):
                        for j in range(nw):
                            kc = w0 + j
                            nc.tensor.matmul(
                                o_tiles[qt][:],
                                exs[j][:, qt * 128:(qt + 1) * 128],
                                Vp[:, kc, h, :],
                                start=(kc == 0), stop=(kc == n_k128 - 1))
                for qt in range(n_qt):
                    inv1 = a_sb.tile([128, 1], F32, name="inv1", bufs=2,
                                     tag="inv1")
                    nc.vector.reciprocal(inv1[:], o_tiles[qt][:, 16:17])
                    nc.vector.tensor_scalar_mul(
                        H[:, qt, 16 * h:16 * (h + 1)], o_tiles[qt][:, 0:16],
                        inv1[:])

            # residual: H += x (node-major shard)
            nc.vector.tensor_add(H[:], H[:], xnf_cur[qs][:])
            H_sb[dirn] = H

        # ---------------- pooling + output MLP
        zt_part_d = dpool.tile([128, B], F32, name="zt_part")
        zt_full_d = dpool.tile([128, B], F32, addr_space="Shared",
                               name="zt_full")
        for dirn, qs in (("mp", "mol"), ("pm", "prot")):
            n_qt = sides[qs]["NC"] // 128
            psz = smallps.tile([128, 512], F32, name="small_ps")
            for qt in range(n_qt):
                nc.tensor.matmul(psz[0:64, 0:B], H_sb[dirn][:, qt, :],
                                 sb_pmat[qs][:, qt, :],
                                 start=(qt == 0), stop=(qt == n_qt - 1))
            zpart = a_sb.tile([64, B], F32, name=f"zpart_{dirn}")
            nc.vector.tensor_copy(zpart[:], psz[0:64, 0:B])
            row0 = 0 if dirn == "mp" else 64
            nc.sync.dma_start(zt_part_d[row0:row0 + 64, :], zpart[:])
        nc.gpsimd.collective_compute(
            "AllReduce", ALU.add, replica_groups=[list(range(R))],
            ins=[zt_part_d[:].opt()], outs=[zt_full_d[:].opt()])
        zT = a_sb.tile([128, B], F32, name="zT")
        nc.sync.dma_start(zT[:], zt_full_d[:])

        fc1W = a_sb.tile([128, 64], F32, name="fc1W")
        nc.sync.dma_start(fc1W[:], dram["fc1_W"][:])
        fc1b = a_sb.tile([64, 1], F32, name="fc1b")
        nc.sync.dma_start(fc1b[:], dram["fc1_b"][:, None])
        fc2W = a_sb.tile([64, 1], F32, name="fc2W")
        nc.sync.dma_start(fc2W[:], dram["fc2_W"][:])
        fc2b = a_sb.tile([1, 1], F32, name="fc2b")
        nc.sync.dma_start(fc2b[:], dram["fc2_b"][:, None])

        ps = smallps.tile([128, 512], F32, name="small_ps")
        nc.tensor.matmul(ps[0:64, 0:B], fc1W[:], zT[:], start=True, stop=True)
        h1 = a_sb.tile([65, B], F32, name="h1")
        nc.scalar.activation(h1[0:64, :], ps[0:64, 0:B], AF.Relu, bias=fc1b[:])
        ps2 = smallps.tile([128, 512], F32, name="small_ps")
        nc.tensor.matmul(ps2[0:1, 0:B], fc2W[:], h1[0:64, :],
                         start=True, stop=True)
        osb = a_sb.tile([1, B], F32, name="osb")
        nc.scalar.activation(osb[:], ps2[0:1, 0:B], AF.Sigmoid, bias=fc2b[:])
        nc.sync.dma_start(out_d[:], osb[:])

        exp_pool.release()
        ops.release()
        s12ps.release()
        smallps.release()
        a_sb.release()
        xnf_pool.release()
        xT_pool.release()
        dpool.release()
        const.release()

    nc.compile()
    return nc



# ----------------------------------------------------------------- entry

def kernel(**inputs):
    global last_results
    meta, percore = _prep_host(inputs)
    key = (meta["mol_T_blk"], meta["prot_T_blk"])
    if key not in _CACHE:
        _CACHE[key] = _build(meta)
    nc = _CACHE[key]
    from concourse.bass_utils import run_bass_kernel_spmd
    res = run_bass_kernel_spmd(nc, percore, list(range(R)))
    last_results = res
    return np.asarray(res.results[0]["out"], np.float32).reshape(B)



# revision 18
# speedup vs baseline: 1.2854x; 1.2825x over previous
"""CrossGraphAttentionModel on 8 Trainium2 NeuronCores (Bass/Tile, SPMD).

Sharding: nodes/edges of both graphs are sharded 8 ways by (dst-sorted) node
range; 64-dim weights replicated. Per GINE layer each core dma_gathers x[src]
for its edge shard from an AllGathered copy of x in HBM, forms messages on
DVE/ACT, and scatter-adds them with one-hot matmuls on the PE (PSUM
accumulation), then runs the node MLP on its node shard and AllGathers the new
x. Cross-graph attention shards the query axis: scores are computed twice on
PE - once [q,k] for an exact row max, once [k,q] with the max folded into the
contraction via an appended ones row - so softmax needs only a single ACT exp
pass, and the exp tiles feed the wV matmul directly as lhsT with a ones column
in V producing the softmax denominator for free. Graph pooling is a one-hot
matmul with 1/count weights, AllReduced, followed by the tiny output MLP.

All floating point math runs on device in fp32; the host only sorts/pads
integer index structures and transposes/replicates input layouts.
"""

import numpy as np

R = 8
HID = 64
B = 32
HEADS = 4
HD = 16
N_MOL, N_PROT = 2048, 4096
E_MOL, E_PROT = 32768, 131072
NC_MOL, NC_PROT = N_MOL // R, N_PROT // R          # 256, 512
NBLK_MOL, NBLK_PROT = NC_MOL // 128, NC_PROT // 128  # 2, 4

_CACHE = {}
last_results = None


# ----------------------------------------------------------------- host prep

def _prep_edges(edge_index, eattr, nblk):
    """Sort edges by dst, partition into R cores x nblk 128-node windows,
    pad every window to T_blk tiles of 128 edges. Returns device layouts."""
    src, dst = np.asarray(edge_index[0]), np.asarray(edge_index[1])
    eattr = np.asarray(eattr, np.float32)
    order = np.argsort(dst, kind="stable")
    src_s, dst_s, ea_s = src[order], dst[order], eattr[order]
    nblocks = R * nblk
    blk = dst_s // 128
    counts = np.bincount(blk, minlength=nblocks)
    T_blk = int(np.ceil(counts.max() / 128))
    T_total = nblk * T_blk
    E_core = T_total * 128
    D = eattr.shape[1]

    gidx = np.zeros((R, E_core), np.int64)
    dstoff = np.full((R, E_core), -1.0, np.float32)
    ea_pad = np.zeros((R, E_core, D), np.float32)
    starts = np.concatenate([[0], np.cumsum(counts)])
    for c in range(R):
        for b in range(nblk):
            g = c * nblk + b
            cnt = counts[g]
            lo = starts[g]
            off = b * T_blk * 128
            gidx[c, off:off + cnt] = src_s[lo:lo + cnt]
            dstoff[c, off:off + cnt] = (dst_s[lo:lo + cnt] - g * 128)
            ea_pad[c, off:off + cnt] = ea_s[lo:lo + cnt]

    # gather indices wrapped [128, E_core/16] (i -> p=i%16, col=i//16), x8 replicated
    cols = E_core // 16
    w = gidx.reshape(R, cols, 16).transpose(0, 2, 1).astype(np.int16)
    gidx_sb = np.tile(w, (1, 8, 1)).copy()
    # dstoff [128, T_total]
    dstoff_sb = np.ascontiguousarray(
        dstoff.reshape(R, T_total, 128).transpose(0, 2, 1))
    # eattr^T with ones row: [11, E_core]
    eaT_packed = np.ascontiguousarray(np.concatenate(
        [ea_pad.transpose(0, 2, 1),
         np.ones((R, 1, E_core), np.float32)], axis=1))
    return dict(T_blk=T_blk, T_total=T_total, E_core=E_core, D=D,
                gidx_sb=gidx_sb, dstoff_sb=dstoff_sb, eaT_packed=eaT_packed)


def _prep_host(inp):
    """All integer/layout preprocessing. Returns (meta, per_core_inputs)."""
    mol = _prep_edges(inp["mol_edge_index"], inp["mol_eattr"], NBLK_MOL)
    prot = _prep_edges(inp["prot_edge_index"], inp["prot_eattr"], NBLK_PROT)

    # pool matrices with 1/count entries
    def pmat(batch, ncore):
        batch = np.asarray(batch)
        cnt = np.bincount(batch, minlength=B).astype(np.float32)
        inv = 1.0 / np.maximum(cnt, 1.0)
        m = np.zeros((R, ncore, B), np.float32)
        for c in range(R):
            sl = batch[c * ncore:(c + 1) * ncore]
            m[c, np.arange(ncore), sl] = inv[sl]
        return m

    mol_pmat = pmat(inp["mol_batch"], NC_MOL)
    prot_pmat = pmat(inp["prot_batch"], NC_PROT)

    # node features transposed per core with ones row
    def xt(x, ncore):
        x = np.asarray(x, np.float32)
        d = x.shape[1]
        out = np.zeros((R, d + 1, ncore), np.float32)
        for c in range(R):
            out[c, :d] = x[c * ncore:(c + 1) * ncore].T
            out[c, d] = 1.0
        return out

    mol_xT = xt(inp["mol_x"], NC_MOL)        # [R, 12, 256]
    prot_xT = xt(inp["prot_x"], NC_PROT)     # [R, 16, 512]

    iota = np.tile(np.arange(128, dtype=np.float32), (128, 1))
    ident = np.eye(128, dtype=np.float32)
    # merge matrix for attention wV output: rows 0-15 -> cols 0-15 (V hi),
    # rows 16-31 -> cols 0-15 (V lo), row 32 -> col 16 (denominator)
    merge33 = np.zeros((33, 17), np.float32)
    merge33[np.arange(16), np.arange(16)] = 1.0
    merge33[16 + np.arange(16), np.arange(16)] = 1.0
    merge33[32, 16] = 1.0

    # attn K-bias as per-head columns [16, 4]
    def bcols(b):  # [64] -> [16, 4]
        return np.ascontiguousarray(np.asarray(b, np.float32).reshape(4, 16).T)

    percore = []
    for c in range(R):
        m = {
            "mol_xT": mol_xT[c], "prot_xT": prot_xT[c],
            "mol_eaT": mol["eaT_packed"][c], "prot_eaT": prot["eaT_packed"][c],
            "mol_gidx": mol["gidx_sb"][c], "prot_gidx": prot["gidx_sb"][c],
            "mol_dstoff": mol["dstoff_sb"][c], "prot_dstoff": prot["dstoff_sb"][c],
            "mol_pmat": mol_pmat[c], "prot_pmat": prot_pmat[c],
            "iota": iota, "ident": ident, "merge33": merge33,
            "bk_mp_cols": bcols(np.asarray(inp["attn_mp_b"])[1]),
            "bk_pm_cols": bcols(np.asarray(inp["attn_pm_b"])[1]),
        }
        for k in ("node_lin_mol_W", "node_lin_mol_b", "node_lin_prot_W",
                  "node_lin_prot_b", "edge_lin_mol_W", "edge_lin_mol_b",
                  "edge_lin_prot_W", "edge_lin_prot_b",
                  "mol_conv_W1", "mol_conv_b1", "mol_conv_W2", "mol_conv_b2",
                  "prot_conv_W1", "prot_conv_b1", "prot_conv_W2", "prot_conv_b2",
                  "attn_mp_W", "attn_mp_b", "attn_pm_W", "attn_pm_b",
                  "fc1_W", "fc1_b", "fc2_W", "fc2_b"):
            m[k] = np.asarray(inp[k], np.float32)
        percore.append(m)

    meta = dict(mol_T_blk=mol["T_blk"], mol_T_total=mol["T_total"],
                mol_E_core=mol["E_core"],
                prot_T_blk=prot["T_blk"], prot_T_total=prot["T_total"],
                prot_E_core=prot["E_core"])
    return meta, percore


# ------------------------------------------------------------- device build

def _build(meta):
    import concourse.bacc as bacc
    import concourse.mybir as mybir
    import concourse.tile as tile

    F32 = mybir.dt.float32
    BF16 = mybir.dt.bfloat16
    I16 = mybir.dt.int16
    AF = mybir.ActivationFunctionType
    ALU = mybir.AluOpType

    nc = bacc.Bacc("TRN2", target_bir_lowering=False, debug=False,
                   num_devices=R)

    # ---- I/O declarations
    dram = {}

    def din(name, shape, dtype=F32):
        dram[name] = nc.dram_tensor(name, list(shape), dtype,
                                    kind="ExternalInput")
        return dram[name]

    mT, mE = meta["mol_T_total"], meta["mol_E_core"]
    pT, pE = meta["prot_T_total"], meta["prot_E_core"]

    din("mol_xT", [12, NC_MOL]); din("prot_xT", [16, NC_PROT])
    din("mol_eaT", [11, mE]); din("prot_eaT", [11, pE])
    din("mol_gidx", [128, mE // 16], I16); din("prot_gidx", [128, pE // 16], I16)
    din("mol_dstoff", [128, mT]); din("prot_dstoff", [128, pT])
    din("mol_pmat", [NC_MOL, B]); din("prot_pmat", [NC_PROT, B])
    din("iota", [128, 128]); din("ident", [128, 128])
    din("merge33", [33, 17])
    din("bk_mp_cols", [16, 4]); din("bk_pm_cols", [16, 4])
    din("node_lin_mol_W", [11, 64]); din("node_lin_mol_b", [64])
    din("node_lin_prot_W", [15, 64]); din("node_lin_prot_b", [64])
    din("edge_lin_mol_W", [10, 64]); din("edge_lin_mol_b", [64])
    din("edge_lin_prot_W", [10, 64]); din("edge_lin_prot_b", [64])
    for s in ("mol", "prot"):
        din(f"{s}_conv_W1", [3, 64, 64]); din(f"{s}_conv_b1", [3, 64])
        din(f"{s}_conv_W2", [3, 64, 64]); din(f"{s}_conv_b2", [3, 64])
    din("attn_mp_W", [3, 64, 64]); din("attn_mp_b", [3, 64])
    din("attn_pm_W", [3, 64, 64]); din("attn_pm_b", [3, 64])
    din("fc1_W", [128, 64]); din("fc1_b", [64])
    din("fc2_W", [64, 1]); din("fc2_b", [1])

    out_d = nc.dram_tensor("out", [1, B], F32, kind="ExternalOutput")

    sides = {
        "mol": dict(N=N_MOL, NC=NC_MOL, nblk=NBLK_MOL, T_blk=meta["mol_T_blk"],
                    T_total=mT, E_core=mE, D=10),
        "prot": dict(N=N_PROT, NC=NC_PROT, nblk=NBLK_PROT,
                     T_blk=meta["prot_T_blk"], T_total=pT, E_core=pE, D=10),
    }

    with tile.TileContext(nc) as tc:
        # ---------------- persistent SBUF constants
        const = tc.alloc_tile_pool(name="const", bufs=1)

        def load_const(name, shape, dtype=F32, src=None):
            t = const.tile(list(shape), dtype, name=f"c_{name}")
            nc.sync.dma_start(t[:], (dram[name] if src is None else src)[:])
            return t

        iota_sb = load_const("iota", [128, 128])
        ident_sb = load_const("ident", [128, 128])

        def wcat(name_w, name_b, din_, dout, wslice=None, bslice=None):
            t = const.tile([din_ + 1, dout], F32, name=f"w_{name_w}_{wslice}")
            wsrc = dram[name_w] if wslice is None else dram[name_w][wslice]
            bsrc = dram[name_b] if bslice is None else dram[name_b][bslice]
            nc.sync.dma_start(t[0:din_, :], wsrc[:, :] if wslice is None else wsrc)
            nc.sync.dma_start(t[din_:din_ + 1, :], bsrc[None, :])
            return t

        Wn = {"mol": wcat("node_lin_mol_W", "node_lin_mol_b", 11, 64),
              "prot": wcat("node_lin_prot_W", "node_lin_prot_b", 15, 64)}
        We = {"mol": wcat("edge_lin_mol_W", "edge_lin_mol_b", 10, 64),
              "prot": wcat("edge_lin_prot_W", "edge_lin_prot_b", 10, 64)}
        W1 = {s: [wcat(f"{s}_conv_W1", f"{s}_conv_b1", 64, 64, l, l)
                  for l in range(3)] for s in ("mol", "prot")}
        W2 = {s: [wcat(f"{s}_conv_W2", f"{s}_conv_b2", 64, 64, l, l)
                  for l in range(3)] for s in ("mol", "prot")}

        sb_idx, sb_dstoff = {}, {}
        for s in sides:
            sd = sides[s]
            sb_idx[s] = load_const(f"{s}_gidx", [128, sd["E_core"] // 16], I16)
            sb_dstoff[s] = load_const(f"{s}_dstoff", [128, sd["T_total"]])
        sb_xTin = {"mol": load_const("mol_xT", [12, NC_MOL]),
                   "prot": load_const("prot_xT", [16, NC_PROT])}
        sb_pmat = {}
        for s in sides:
            sd = sides[s]
            t = const.tile([128, sd["nblk"], B], F32, name=f"pmat_{s}")
            nc.sync.dma_start(
                t[:], dram[f"{s}_pmat"].rearrange("(t p) g -> p t g", p=128))
            sb_pmat[s] = t

        # ---------------- DRAM internals
        dpool = tc.alloc_tile_pool(name="dram", bufs=1, space="DRAM")
        x_sh_d = {s: [dpool.tile([sides[s]["NC"], 64], F32,
                                 name=f"xsh_{s}_{l}") for l in range(4)]
                  for s in sides}
        x_full_d = {s: [dpool.tile([sides[s]["N"], 64], F32,
                                   addr_space="Shared", name=f"xfull_{s}_{l}")
                        for l in range(4)] for s in sides}

        # ---------------- long-lived x pools, then GINE-scoped pools
        xT_pool = tc.alloc_tile_pool(name="xT", bufs=2)
        xnf_pool = tc.alloc_tile_pool(name="xnf", bufs=2)
        gmem = tc.alloc_tile_pool(name="gmem", bufs=1)
        empp = tc.alloc_tile_pool(name="empp", bufs=2, space="PSUM")
        aggps = tc.alloc_tile_pool(name="aggps", bufs=2, space="PSUM")
        mlpps = tc.alloc_tile_pool(name="mlpps", bufs=2, space="PSUM")
        trps = tc.alloc_tile_pool(name="trps", bufs=2, space="PSUM")

        # pools for the GINE loop allocated up-front so nothing aliases the
        # ea_stream buffers (aliasing would stall the first gathers on a WAR)
        xg_pool = tc.alloc_tile_pool(name="xg", bufs=2)
        oh_pool = tc.alloc_tile_pool(name="oh", bufs=2)
        ea_stream = tc.alloc_tile_pool(name="ea_stream", bufs=2)

        # initial node features x0 first: gets the AllGathers in flight early
        xT_cur = {}
        xnf_cur = {}
        for s in sides:
            sd = sides[s]
            NCs, nblk = sd["NC"], sd["nblk"]
            ps = mlpps.tile([64, 512], F32, name="mlp_ps")
            nc.tensor.matmul(ps[:, 0:NCs], Wn[s][:], sb_xTin[s][:],
                             start=True, stop=True)
            xT = xT_pool.tile([65, NCs], F32, name=f"xT_{s}")
            nc.vector.tensor_copy(xT[0:64, :], ps[:, 0:NCs])
            nc.vector.memset(xT[64:65, :], 1.0)
            xnf = xnf_pool.tile([128, nblk, 64], F32, name=f"xnf_{s}")
            for b in range(nblk):
                tp = trps.tile([128, 64], F32, name="tr_ps")
                nc.tensor.transpose(tp[:], xT[0:64, b * 128:(b + 1) * 128],
                                    ident_sb[0:64, 0:64])
                nc.vector.tensor_copy(xnf[:, b, :], tp[:])
            nc.sync.dma_start(
                x_sh_d[s][0][:].rearrange("(t p) f -> p t f", p=128), xnf[:])
            nc.gpsimd.collective_compute(
                "AllGather", ALU.bypass, replica_groups=[list(range(R))],
                ins=[x_sh_d[s][0][:].opt()], outs=[x_full_d[s][0][:].opt()])
            xT_cur[s] = xT
            xnf_cur[s] = xnf

        # edge features em = [eattr;1] @ [We;be], edge-major [128, T, 64],
        # eattr^T streamed from DRAM per block
        em_sb = {}
        for s in sides:
            sd = sides[s]
            T_total, T_blk, nblk, D = (sd["T_total"], sd["T_blk"], sd["nblk"],
                                       sd["D"])
            em = gmem.tile([128, T_total, 64], F32, name=f"em_{s}")
            for b in range(nblk):
                ch = ea_stream.tile([11, T_blk * 128], F32, name="ea_chunk")
                nc.sync.dma_start(
                    ch[:],
                    dram[f"{s}_eaT"][:, b * T_blk * 128:(b + 1) * T_blk * 128])
                for t0 in range(0, T_blk, 8):
                    ng = min(8, T_blk - t0)
                    ps = empp.tile([128, 8, 64], F32, name="em_ps")
                    for j in range(ng):
                        nc.tensor.matmul(
                            ps[:, j, :],
                            ch[0:D + 1, (t0 + j) * 128:(t0 + j + 1) * 128],
                            We[s][:], start=True, stop=True)
                    nc.vector.tensor_copy(
                        em[:, b * T_blk + t0:b * T_blk + t0 + ng, :],
                        ps[:, 0:ng, :])
            em_sb[s] = em

        # GINE layers
        for l in range(3):
            for s in ("prot", "mol"):
                sd = sides[s]
                NCs, nblk, T_blk = sd["NC"], sd["nblk"], sd["T_blk"]
                xT_prev = xT_cur[s]
                hT = gmem.tile([65, NCs], F32, name=f"hT_{s}_{l}", bufs=2,
                               tag=f"hT_{s}")
                for b in range(nblk):
                    nE = T_blk * 128
                    xg = xg_pool.tile([128, T_blk, 64], F32, name="xg")
                    nc.gpsimd.dma_gather(
                        xg[:], x_full_d[s][l][:],
                        sb_idx[s][:, b * T_blk * 8:(b + 1) * T_blk * 8],
                        nE, nE, 64, single_packet=False)
                    msg = xg_pool.tile([128, T_blk, 64], F32, name="msg")
                    nc.vector.tensor_add(
                        msg[:], xg[:], em_sb[s][:, b * T_blk:(b + 1) * T_blk, :])
                    nc.scalar.activation(msg[:], msg[:], AF.Relu)
                    oh = oh_pool.tile([128, T_blk, 128], F32, name="oh")
                    nc.vector.tensor_tensor(
                        oh[:],
                        iota_sb[:, :].unsqueeze(1).broadcast_to([128, T_blk, 128]),
                        sb_dstoff[s][:, b * T_blk:(b + 1) * T_blk]
                            .unsqueeze(2).broadcast_to([128, T_blk, 128]),
                        ALU.is_equal)
                    agg = aggps.tile([64, 128], F32, name="agg_ps")
                    for t in range(T_blk):
                        nc.tensor.matmul(agg[:], msg[:, t, :], oh[:, t, :],
                                         start=(t == 0), stop=(t == T_blk - 1))
                    nc.vector.tensor_add(hT[0:64, b * 128:(b + 1) * 128],
                                         xT_prev[0:64, b * 128:(b + 1) * 128],
                                         agg[:])
                nc.vector.memset(hT[64:65, :], 1.0)
                ps1 = mlpps.tile([64, 512], F32, name="mlp_ps")
                nc.tensor.matmul(ps1[:, 0:NCs], W1[s][l][:], hT[:],
                                 start=True, stop=True)
                r1 = gmem.tile([65, NCs], F32, name=f"r1_{s}_{l}", bufs=2,
                               tag=f"r1_{s}")
                nc.scalar.activation(r1[0:64, :], ps1[:, 0:NCs], AF.Relu)
                nc.vector.memset(r1[64:65, :], 1.0)
                ps2 = mlpps.tile([64, 512], F32, name="mlp_ps")
                nc.tensor.matmul(ps2[:, 0:NCs], W2[s][l][:], r1[:],
                                 start=True, stop=True)
                xT = xT_pool.tile([65, NCs], F32, name=f"xT_{s}")
                nc.scalar.activation(xT[0:64, :], ps2[:, 0:NCs], AF.Relu)
                nc.vector.memset(xT[64:65, :], 1.0)
                xnf = xnf_pool.tile([128, nblk, 64], F32, name=f"xnf_{s}")
                for b in range(nblk):
                    tp = trps.tile([128, 64], F32, name="tr_ps")
                    nc.tensor.transpose(tp[:], xT[0:64, b * 128:(b + 1) * 128],
                                        ident_sb[0:64, 0:64])
                    nc.vector.tensor_copy(xnf[:, b, :], tp[:])
                nc.sync.dma_start(
                    x_sh_d[s][l + 1][:].rearrange("(t p) f -> p t f", p=128),
                    xnf[:])
                nc.gpsimd.collective_compute(
                    "AllGather", ALU.bypass, replica_groups=[list(range(R))],
                    ins=[x_sh_d[s][l + 1][:].opt()],
                    outs=[x_full_d[s][l + 1][:].opt()])
                xT_cur[s] = xT
                xnf_cur[s] = xnf

        # close GINE-scoped pools (LIFO per space)
        ea_stream.release()
        oh_pool.release()
        xg_pool.release()
        for p in (trps, mlpps, aggps, empp):
            p.release()
        gmem.release()

        # ---------------- attention phase
        # Scores are computed in bf16x3 (hi*hi + lo*hi + hi*lo accumulated in
        # PSUM fp32, ~fp32 accuracy at 3/8 the PE cost); the max pass uses a
        # single bf16 product (any tight upper bound works); wV runs in the
        # transposed domain with [V_hi | V_lo | ones] packed along M and the
        # hi+lo recombination folded into the output transpose matmul.
        a_sb = tc.alloc_tile_pool(name="attn_sb", bufs=1)
        smallps = tc.alloc_tile_pool(name="smallps", bufs=2, space="PSUM")
        s12ps = tc.alloc_tile_pool(name="s12ps", bufs=3, space="PSUM")
        otps = tc.alloc_tile_pool(name="otps", bufs=2, space="PSUM")
        exp_pool = tc.alloc_tile_pool(name="expt", bufs=4)

        merge_sb = a_sb.tile([33, 17], F32, name="merge33")
        nc.sync.dma_start(merge_sb[:], dram["merge33"][:])

        # full x (both sides), transposed with ones row, + bf16 hi/lo split
        xT_full, xh_full, xl_full = {}, {}, {}
        for s in sides:
            sd = sides[s]
            Ns = sd["N"]
            nt = Ns // 128
            xT_f = a_sb.tile([65, Ns], F32, name=f"xTfull_{s}")
            for t in range(nt):
                xf_nf = a_sb.tile([128, 64], F32, name="xf_nf", bufs=3,
                                  tag="xf_nf")
                nc.sync.dma_start(
                    xf_nf[:], x_full_d[s][3][t * 128:(t + 1) * 128, :])
                tp = smallps.tile([128, 512], F32, name="small_ps")
                nc.tensor.transpose(tp[0:64, 0:128], xf_nf[:], ident_sb[:])
                nc.vector.tensor_copy(xT_f[0:64, t * 128:(t + 1) * 128],
                                      tp[0:64, 0:128])
            nc.vector.memset(xT_f[64:65, :], 1.0)
            xh = a_sb.tile([65, Ns], BF16, name=f"xh_{s}")
            nc.vector.tensor_copy(xh[:], xT_f[:])
            xr = a_sb.tile([65, Ns], F32, name="xres", bufs=2, tag="xres")
            nc.vector.tensor_sub(xr[:], xT_f[:], xh[:])
            xl = a_sb.tile([65, Ns], BF16, name=f"xl_{s}")
            nc.vector.tensor_copy(xl[:], xr[:])
            xT_full[s] = xT_f
            xh_full[s] = xh
            xl_full[s] = xl

        # per-direction prep: folded per-head projector P65 [65, 65] with
        # rhs0_h = P65_h^T @ xT_q  (row 64 = c_h, the K-bias term)
        prep = {}
        for dirn, (qs, ks) in (("mp", ("mol", "prot")), ("pm", ("prot", "mol"))):
            NCq = sides[qs]["NC"]
            n_k128 = sides[ks]["N"] // 128
            Wd = dram[f"attn_{dirn}_W"]
            bd = dram[f"attn_{dirn}_b"]
            Wq = a_sb.tile([65, 64], F32, name=f"Wq_{dirn}")
            nc.sync.dma_start(Wq[0:64, :], Wd[0])
            nc.sync.dma_start(Wq[64:65, :], bd[0][None, :])
            Wv = a_sb.tile([65, 64], F32, name=f"Wv_{dirn}")
            nc.sync.dma_start(Wv[0:64, :], Wd[2])
            nc.sync.dma_start(Wv[64:65, :], bd[2][None, :])
            Wk_raw = a_sb.tile([64, 64], F32, name=f"Wkraw_{dirn}")
            nc.sync.dma_start(Wk_raw[:], Wd[1])
            bk_cols = a_sb.tile([16, 4], F32, name=f"bkcols_{dirn}")
            nc.sync.dma_start(bk_cols[:], dram[f"bk_{dirn}_cols"][:])

            rhs0, r0h = [], []
            cT = a_sb.tile([1, HEADS, NCq], F32, name=f"cT_{dirn}")
            for h in range(HEADS):
                tp = smallps.tile([128, 512], F32, name="small_ps")
                nc.tensor.transpose(tp[0:16, 0:64],
                                    Wk_raw[:, 16 * h:16 * h + 16],
                                    ident_sb[0:64, 0:64])
                wkt65 = a_sb.tile([16, 65], F32, name="wkt65", bufs=2,
                                  tag="wkt65")
                nc.vector.tensor_copy(wkt65[:, 0:64], tp[0:16, 0:64])
                nc.vector.tensor_copy(wkt65[:, 64:65], bk_cols[:, h:h + 1])
                tq = smallps.tile([128, 512], F32, name="small_ps")
                nc.tensor.transpose(tq[0:16, 0:65],
                                    Wq[:, 16 * h:16 * h + 16],
                                    ident_sb[0:65, 0:65])
                wqt = a_sb.tile([16, 65], F32, name="wqt", bufs=2, tag="wqt")
                nc.vector.tensor_copy(wqt[:], tq[0:16, 0:65])
                pps = smallps.tile([128, 512], F32, name="small_ps")
                nc.tensor.matmul(pps[0:65, 0:65], wqt[:], wkt65[:],
                                 start=True, stop=True)
                P65 = a_sb.tile([65, 65], F32, name="P65", bufs=2, tag="P65")
                nc.scalar.activation(P65[:], pps[0:65, 0:65], AF.Copy,
                                     scale=0.25)
                rps = smallps.tile([128, 512], F32, name="small_ps")
                nc.tensor.matmul(rps[0:65, 0:NCq], P65[:], xT_cur[qs][:],
                                 start=True, stop=True)
                r0 = a_sb.tile([65, NCq], F32, name=f"rhs0_{dirn}_{h}")
                nc.vector.tensor_copy(r0[:], rps[0:65, 0:NCq])
                nc.vector.tensor_copy(cT[0:1, h, :], rps[64:65, 0:NCq])
                rh = a_sb.tile([65, NCq], BF16, name=f"r0h_{dirn}_{h}")
                nc.vector.tensor_copy(rh[:], r0[:])
                rhs0.append(r0)
                r0h.append(rh)

            # V' [128, n_k128, 4, 33] bf16: [V_hi | V_lo | ones]
            Vp = a_sb.tile([128, n_k128, HEADS, 33], BF16, name=f"Vp_{dirn}")
            nc.vector.memset(Vp[:, :, :, 32:33], 1.0)
            for kt in range(n_k128):
                ps = smallps.tile([128, 512], F32, name="small_ps")
                nc.tensor.matmul(ps[0:128, 0:64],
                                 xT_full[ks][:, kt * 128:(kt + 1) * 128],
                                 Wv[:], start=True, stop=True)
                vh = a_sb.tile([128, 64], BF16, name="vh", bufs=2, tag="vh")
                nc.vector.tensor_copy(vh[:], ps[0:128, 0:64])
                vr = a_sb.tile([128, 64], F32, name="vr", bufs=2, tag="vr")
                nc.vector.tensor_sub(vr[:], ps[0:128, 0:64], vh[:])
                nc.vector.tensor_copy(
                    Vp[:, kt, :, 0:16],
                    vh[:].rearrange("p (h d) -> p h d", h=HEADS))
                nc.vector.tensor_copy(
                    Vp[:, kt, :, 16:32],
                    vr[:].rearrange("p (h d) -> p h d", h=HEADS))
            prep[dirn] = dict(rhs0=rhs0, r0h=r0h, Vp=Vp, cT=cT)

        # pass 1: row max m_h [1, NCq] per head ([q, k] layout, single bf16)
        mT = {}
        for dirn, (qs, ks) in (("mp", ("mol", "prot")), ("pm", ("prot", "mol"))):
            NCq = sides[qs]["NC"]
            n_qt = NCq // 128
            n_k512 = sides[ks]["N"] // 512
            r0h = prep[dirn]["r0h"]
            mTd = a_sb.tile([1, HEADS, NCq], F32, name=f"mT_{dirn}")
            for h in range(HEADS):
                for qt in range(n_qt):
                    mx = a_sb.tile([128, n_k512], F32, name="mx", bufs=2,
                                   tag="mx")
                    for cch in range(n_k512):
                        s1 = s12ps.tile([128, 512], F32, name="s12_ps")
                        nc.tensor.matmul(
                            s1[:],
                            r0h[h][:, qt * 128:(qt + 1) * 128],
                            xh_full[ks][:, cch * 512:(cch + 1) * 512],
                            start=True, stop=True)
                        nc.vector.reduce_max(mx[:, cch:cch + 1], s1[:],
                                             axis=mybir.AxisListType.X)
                    mqt = a_sb.tile([128, 1], F32, name="mqt", bufs=2,
                                    tag="mqt")
                    nc.vector.reduce_max(mqt[:], mx[:],
                                         axis=mybir.AxisListType.X)
                    tp = smallps.tile([128, 512], F32, name="small_ps")
                    nc.tensor.transpose(tp[0:1, 0:128], mqt[:], ident_sb[:])
                    nc.vector.tensor_copy(
                        mTd[0:1, h, qt * 128:(qt + 1) * 128], tp[0:1, 0:128])
            mT[dirn] = mTd

        # pass 2 + wV
        H_sb = {}
        for dirn, (qs, ks) in (("mp", ("mol", "prot")), ("pm", ("prot", "mol"))):
            NCq = sides[qs]["NC"]
            n_qt = NCq // 128
            n_k128 = sides[ks]["N"] // 128
            rhs0 = prep[dirn]["rhs0"]
            Vp = prep[dirn]["Vp"]
            xh, xl = xh_full[ks], xl_full[ks]
            H = a_sb.tile([128, n_qt, 64], F32, name=f"H_{dirn}")
            # head groups: pack 2 heads side-by-side when NCq == 256 so the
            # score matmul streams a full 512-wide PSUM tile
            hgroups = ([(0, 1), (2, 3)] if NCq == 256
                       else [(0,), (1,), (2,), (3,)])
            for hg in hgroups:
                gw = NCq * len(hg)
                rhm = a_sb.tile([65, gw], F32, name="rhm", bufs=2, tag="rhm")
                cm = a_sb.tile([1, gw], F32, name="cm", bufs=2, tag="cm")
                for j, h in enumerate(hg):
                    nc.vector.tensor_copy(
                        rhm[0:64, j * NCq:(j + 1) * NCq], rhs0[h][0:64, :])
                    nc.vector.tensor_sub(
                        cm[0:1, j * NCq:(j + 1) * NCq],
                        prep[dirn]["cT"][0:1, h, :], mT[dirn][0:1, h, :])
                nc.vector.tensor_copy(rhm[64:65, :], cm[:])
                rh2 = a_sb.tile([65, gw], BF16, name="rh2", bufs=2, tag="rh2")
                nc.vector.tensor_copy(rh2[:], rhm[:])
                rres = a_sb.tile([65, gw], F32, name="rres", bufs=2,
                                 tag="rres")
                nc.vector.tensor_sub(rres[:], rhm[:], rh2[:])
                rl2 = a_sb.tile([65, gw], BF16, name="rl2", bufs=2, tag="rl2")
                nc.vector.tensor_copy(rl2[:], rres[:])

                oT = {h: otps.tile([33, NCq], F32, name="oT") for h in hg}
                pend = None
                for kc in range(n_k128):
                    ps = s12ps.tile([128, 512], F32, name="s12_ps")
                    xhc = xh[:, kc * 128:(kc + 1) * 128]
                    xlc = xl[:, kc * 128:(kc + 1) * 128]
                    nc.tensor.matmul(ps[:, 0:gw], xhc, rh2[:],
                                     start=True, stop=False)
                    nc.tensor.matmul(ps[:, 0:gw], xlc, rh2[:],
                                     start=False, stop=False)
                    nc.tensor.matmul(ps[:, 0:gw], xhc, rl2[:],
                                     start=False, stop=True)
                    ex = exp_pool.tile([128, gw], BF16, name="ex",
                                       tag=f"ex_{dirn}")
                    nc.scalar.activation(ex[:], ps[:, 0:gw], AF.Exp)
                    if pend is not None:
                        pkc, pex = pend
                        for j, h in enumerate(hg):
                            nc.tensor.matmul(
                                oT[h][:], Vp[:, pkc, h, :],
                                pex[:, j * NCq:(j + 1) * NCq],
                                start=(pkc == 0), stop=False)
                    pend = (kc, ex)
                pkc, pex = pend
                for j, h in enumerate(hg):
                    nc.tensor.matmul(
                        oT[h][:], Vp[:, pkc, h, :],
                        pex[:, j * NCq:(j + 1) * NCq],
                        start=(pkc == 0), stop=True)
                # recombine: copy oT to SBUF, transpose via merge matrix
                # (cols 0-15 = V_hi + V_lo, col 16 = denominator)
                for j, h in enumerate(hg):
                    oT_sb = a_sb.tile([33, NCq], F32, name="oT_sb", bufs=2,
                                      tag="oT_sb")
                    nc.vector.tensor_copy(oT_sb[:], oT[h][:])
                    for qt in range(n_qt):
                        mps = smallps.tile([128, 512], F32, name="small_ps")
                        nc.tensor.matmul(
                            mps[0:128, 0:17],
                            oT_sb[:, qt * 128:(qt + 1) * 128],
                            merge_sb[:], start=True, stop=True)
                        inv1 = a_sb.tile([128, 1], F32, name="inv1", bufs=2,
                                         tag="inv1")
                        nc.vector.reciprocal(inv1[:], mps[0:128, 16:17])
                        nc.vector.tensor_scalar_mul(
                            H[:, qt, 16 * h:16 * (h + 1)],
                            mps[0:128, 0:16], inv1[:])

            # residual: H += x (node-major shard)
            nc.vector.tensor_add(H[:], H[:], xnf_cur[qs][:])
            H_sb[dirn] = H

        # ---------------- pooling + output MLP
        zt_part_d = dpool.tile([128, B], F32, name="zt_part")
        zt_full_d = dpool.tile([128, B], F32, addr_space="Shared",
                               name="zt_full")
        for dirn, qs in (("mp", "mol"), ("pm", "prot")):
            n_qt = sides[qs]["NC"] // 128
            psz = smallps.tile([128, 512], F32, name="small_ps")
            for qt in range(n_qt):
                nc.tensor.matmul(psz[0:64, 0:B], H_sb[dirn][:, qt, :],
                                 sb_pmat[qs][:, qt, :],
                                 start=(qt == 0), stop=(qt == n_qt - 1))
            zpart = a_sb.tile([64, B], F32, name=f"zpart_{dirn}")
            nc.vector.tensor_copy(zpart[:], psz[0:64, 0:B])
            row0 = 0 if dirn == "mp" else 64
            nc.sync.dma_start(zt_part_d[row0:row0 + 64, :], zpart[:])
        nc.gpsimd.collective_compute(
            "AllReduce", ALU.add, replica_groups=[list(range(R))],
            ins=[zt_part_d[:].opt()], outs=[zt_full_d[:].opt()])
        zT = a_sb.tile([128, B], F32, name="zT")
        nc.sync.dma_start(zT[:], zt_full_d[:])

        fc1W = a_sb.tile([128, 64], F32, name="fc1W")
        nc.sync.dma_start(fc1W[:], dram["fc1_W"][:])
        fc1b = a_sb.tile([64, 1], F32, name="fc1b")
        nc.sync.dma_start(fc1b[:], dram["fc1_b"][:, None])
        fc2W = a_sb.tile([64, 1], F32, name="fc2W")
        nc.sync.dma_start(fc2W[:], dram["fc2_W"][:])
        fc2b = a_sb.tile([1, 1], F32, name="fc2b")
        nc.sync.dma_start(fc2b[:], dram["fc2_b"][:, None])

        ps = smallps.tile([128, 512], F32, name="small_ps")
        nc.tensor.matmul(ps[0:64, 0:B], fc1W[:], zT[:], start=True, stop=True)
        h1 = a_sb.tile([65, B], F32, name="h1")
        nc.scalar.activation(h1[0:64, :], ps[0:64, 0:B], AF.Relu, bias=fc1b[:])
        ps2 = smallps.tile([128, 512], F32, name="small_ps")
        nc.tensor.matmul(ps2[0:1, 0:B], fc2W[:], h1[0:64, :],
                         start=True, stop=True)
        osb = a_sb.tile([1, B], F32, name="osb")
        nc.scalar.activation(osb[:], ps2[0:1, 0:B], AF.Sigmoid, bias=fc2b[:])
        nc.sync.dma_start(out_d[:], osb[:])

        exp_pool.release()
        otps.release()
        s12ps.release()
        smallps.release()
        a_sb.release()
        xnf_pool.release()
        xT_pool.release()
        dpool.release()
        const.release()

    nc.compile()
    return nc



# ----------------------------------------------------------------- entry

def kernel(**inputs):
    global last_results
    meta, percore = _prep_host(inputs)
    key = (meta["mol_T_blk"], meta["prot_T_blk"])
    if key not in _CACHE:
        _CACHE[key] = _build(meta)
    nc = _CACHE[key]
    from concourse.bass_utils import run_bass_kernel_spmd
    res = run_bass_kernel_spmd(nc, percore, list(range(R)))
    last_results = res
    return np.asarray(res.results[0]["out"], np.float32).reshape(B)



# revision 21
# speedup vs baseline: 1.5458x; 1.2026x over previous
"""CrossGraphAttentionModel on 8 Trainium2 NeuronCores (Bass/Tile, SPMD).

Sharding: nodes/edges of both graphs are sharded 8 ways by (dst-sorted) node
range; 64-dim weights replicated. Per GINE layer each core dma_gathers x[src]
for its edge shard from an AllGathered copy of x in HBM, forms messages on
DVE/ACT, and scatter-adds them with one-hot matmuls on the PE (PSUM
accumulation), then runs the node MLP on its node shard and AllGathers the new
x. Cross-graph attention shards the query axis: scores are computed twice on
PE - once [q,k] for an exact row max, once [k,q] with the max folded into the
contraction via an appended ones row - so softmax needs only a single ACT exp
pass, and the exp tiles feed the wV matmul directly as lhsT with a ones column
in V producing the softmax denominator for free. Graph pooling is a one-hot
matmul with 1/count weights, AllReduced, followed by the tiny output MLP.

All floating point math runs on device in fp32; the host only sorts/pads
integer index structures and transposes/replicates input layouts.
"""

import numpy as np

R = 8
HID = 64
B = 32
HEADS = 4
HD = 16
N_MOL, N_PROT = 2048, 4096
E_MOL, E_PROT = 32768, 131072
NC_MOL, NC_PROT = N_MOL // R, N_PROT // R          # 256, 512
NBLK_MOL, NBLK_PROT = NC_MOL // 128, NC_PROT // 128  # 2, 4

_CACHE = {}
last_results = None


# ----------------------------------------------------------------- host prep

def _prep_edges(edge_index, eattr, nblk):
    """Sort edges by dst, partition into R cores x nblk 128-node windows,
    pad every window to T_blk tiles of 128 edges. Returns device layouts."""
    src, dst = np.asarray(edge_index[0]), np.asarray(edge_index[1])
    eattr = np.asarray(eattr, np.float32)
    order = np.argsort(dst, kind="stable")
    src_s, dst_s, ea_s = src[order], dst[order], eattr[order]
    nblocks = R * nblk
    blk = dst_s // 128
    counts = np.bincount(blk, minlength=nblocks)
    T_blk = int(np.ceil(counts.max() / 128))
    T_total = nblk * T_blk
    E_core = T_total * 128
    D = eattr.shape[1]

    gidx = np.zeros((R, E_core), np.int64)
    dstoff = np.full((R, E_core), -1.0, np.float32)
    ea_pad = np.zeros((R, E_core, D), np.float32)
    starts = np.concatenate([[0], np.cumsum(counts)])
    for c in range(R):
        for b in range(nblk):
            g = c * nblk + b
            cnt = counts[g]
            lo = starts[g]
            off = b * T_blk * 128
            gidx[c, off:off + cnt] = src_s[lo:lo + cnt]
            dstoff[c, off:off + cnt] = (dst_s[lo:lo + cnt] - g * 128)
            ea_pad[c, off:off + cnt] = ea_s[lo:lo + cnt]

    # gather indices wrapped [128, E_core/16] (i -> p=i%16, col=i//16), x8 replicated
    cols = E_core // 16
    w = gidx.reshape(R, cols, 16).transpose(0, 2, 1).astype(np.int16)
    gidx_sb = np.tile(w, (1, 8, 1)).copy()
    # dstoff [128, T_total]
    dstoff_sb = np.ascontiguousarray(
        dstoff.reshape(R, T_total, 128).transpose(0, 2, 1))
    # eattr^T with ones row: [11, E_core]
    eaT_packed = np.ascontiguousarray(np.concatenate(
        [ea_pad.transpose(0, 2, 1),
         np.ones((R, 1, E_core), np.float32)], axis=1))
    return dict(T_blk=T_blk, T_total=T_total, E_core=E_core, D=D,
                gidx_sb=gidx_sb, dstoff_sb=dstoff_sb, eaT_packed=eaT_packed)


def _prep_host(inp):
    """All integer/layout preprocessing. Returns (meta, per_core_inputs)."""
    mol = _prep_edges(inp["mol_edge_index"], inp["mol_eattr"], NBLK_MOL)
    prot = _prep_edges(inp["prot_edge_index"], inp["prot_eattr"], NBLK_PROT)

    # pool matrices with 1/count entries
    def pmat(batch, ncore):
        batch = np.asarray(batch)
        cnt = np.bincount(batch, minlength=B).astype(np.float32)
        inv = 1.0 / np.maximum(cnt, 1.0)
        m = np.zeros((R, ncore, B), np.float32)
        for c in range(R):
            sl = batch[c * ncore:(c + 1) * ncore]
            m[c, np.arange(ncore), sl] = inv[sl]
        return m

    mol_pmat = pmat(inp["mol_batch"], NC_MOL)
    prot_pmat = pmat(inp["prot_batch"], NC_PROT)

    # node features transposed per core with ones row
    def xt(x, ncore):
        x = np.asarray(x, np.float32)
        d = x.shape[1]
        out = np.zeros((R, d + 1, ncore), np.float32)
        for c in range(R):
            out[c, :d] = x[c * ncore:(c + 1) * ncore].T
            out[c, d] = 1.0
        return out

    mol_xT = xt(inp["mol_x"], NC_MOL)        # [R, 12, 256]
    prot_xT = xt(inp["prot_x"], NC_PROT)     # [R, 16, 512]

    iota = np.tile(np.arange(128, dtype=np.float32), (128, 1))
    ident = np.eye(128, dtype=np.float32)
    # merge matrix for attention wV output: rows 0-15 -> cols 0-15 (V hi),
    # rows 16-31 -> cols 0-15 (V lo), row 32 -> col 16 (denominator)
    merge33 = np.zeros((33, 17), np.float32)
    merge33[np.arange(16), np.arange(16)] = 1.0
    merge33[16 + np.arange(16), np.arange(16)] = 1.0
    merge33[32, 16] = 1.0

    # attn K-bias as per-head columns [16, 4]
    def bcols(b):  # [64] -> [16, 4]
        return np.ascontiguousarray(np.asarray(b, np.float32).reshape(4, 16).T)

    percore = []
    for c in range(R):
        m = {
            "mol_xT": mol_xT[c], "prot_xT": prot_xT[c],
            "mol_eaT": mol["eaT_packed"][c], "prot_eaT": prot["eaT_packed"][c],
            "mol_gidx": mol["gidx_sb"][c], "prot_gidx": prot["gidx_sb"][c],
            "mol_dstoff": mol["dstoff_sb"][c], "prot_dstoff": prot["dstoff_sb"][c],
            "mol_pmat": mol_pmat[c], "prot_pmat": prot_pmat[c],
            "iota": iota, "ident": ident, "merge33": merge33,
            "bk_mp_cols": bcols(np.asarray(inp["attn_mp_b"])[1]),
            "bk_pm_cols": bcols(np.asarray(inp["attn_pm_b"])[1]),
        }
        for k in ("node_lin_mol_W", "node_lin_mol_b", "node_lin_prot_W",
                  "node_lin_prot_b", "edge_lin_mol_W", "edge_lin_mol_b",
                  "edge_lin_prot_W", "edge_lin_prot_b",
                  "mol_conv_W1", "mol_conv_b1", "mol_conv_W2", "mol_conv_b2",
                  "prot_conv_W1", "prot_conv_b1", "prot_conv_W2", "prot_conv_b2",
                  "attn_mp_W", "attn_mp_b", "attn_pm_W", "attn_pm_b",
                  "fc1_W", "fc1_b", "fc2_W", "fc2_b"):
            m[k] = np.asarray(inp[k], np.float32)
        percore.append(m)

    meta = dict(mol_T_blk=mol["T_blk"], mol_T_total=mol["T_total"],
                mol_E_core=mol["E_core"],
                prot_T_blk=prot["T_blk"], prot_T_total=prot["T_total"],
                prot_E_core=prot["E_core"])
    return meta, percore


# ------------------------------------------------------------- device build

def _build(meta):
    import concourse.bacc as bacc
    import concourse.mybir as mybir
    import concourse.tile as tile

    F32 = mybir.dt.float32
    F32R = mybir.dt.float32r
    BF16 = mybir.dt.bfloat16
    I16 = mybir.dt.int16
    AF = mybir.ActivationFunctionType
    ALU = mybir.AluOpType

    nc = bacc.Bacc("TRN2", target_bir_lowering=False, debug=False,
                   num_devices=R)

    # ---- I/O declarations
    dram = {}

    def din(name, shape, dtype=F32):
        dram[name] = nc.dram_tensor(name, list(shape), dtype,
                                    kind="ExternalInput")
        return dram[name]

    mT, mE = meta["mol_T_total"], meta["mol_E_core"]
    pT, pE = meta["prot_T_total"], meta["prot_E_core"]

    din("mol_xT", [12, NC_MOL]); din("prot_xT", [16, NC_PROT])
    din("mol_eaT", [11, mE]); din("prot_eaT", [11, pE])
    din("mol_gidx", [128, mE // 16], I16); din("prot_gidx", [128, pE // 16], I16)
    din("mol_dstoff", [128, mT]); din("prot_dstoff", [128, pT])
    din("mol_pmat", [NC_MOL, B]); din("prot_pmat", [NC_PROT, B])
    din("iota", [128, 128]); din("ident", [128, 128])
    din("merge33", [33, 17])
    din("bk_mp_cols", [16, 4]); din("bk_pm_cols", [16, 4])
    din("node_lin_mol_W", [11, 64]); din("node_lin_mol_b", [64])
    din("node_lin_prot_W", [15, 64]); din("node_lin_prot_b", [64])
    din("edge_lin_mol_W", [10, 64]); din("edge_lin_mol_b", [64])
    din("edge_lin_prot_W", [10, 64]); din("edge_lin_prot_b", [64])
    for s in ("mol", "prot"):
        din(f"{s}_conv_W1", [3, 64, 64]); din(f"{s}_conv_b1", [3, 64])
        din(f"{s}_conv_W2", [3, 64, 64]); din(f"{s}_conv_b2", [3, 64])
    din("attn_mp_W", [3, 64, 64]); din("attn_mp_b", [3, 64])
    din("attn_pm_W", [3, 64, 64]); din("attn_pm_b", [3, 64])
    din("fc1_W", [128, 64]); din("fc1_b", [64])
    din("fc2_W", [64, 1]); din("fc2_b", [1])

    out_d = nc.dram_tensor("out", [1, B], F32, kind="ExternalOutput")

    sides = {
        "mol": dict(N=N_MOL, NC=NC_MOL, nblk=NBLK_MOL, T_blk=meta["mol_T_blk"],
                    T_total=mT, E_core=mE, D=10),
        "prot": dict(N=N_PROT, NC=NC_PROT, nblk=NBLK_PROT,
                     T_blk=meta["prot_T_blk"], T_total=pT, E_core=pE, D=10),
    }

    with tile.TileContext(nc) as tc:
        # ---------------- persistent SBUF constants
        const = tc.alloc_tile_pool(name="const", bufs=1)

        def load_const(name, shape, dtype=F32, src=None):
            t = const.tile(list(shape), dtype, name=f"c_{name}")
            nc.sync.dma_start(t[:], (dram[name] if src is None else src)[:])
            return t

        iota_sb = load_const("iota", [128, 128])
        ident_sb = load_const("ident", [128, 128])

        def wcat(name_w, name_b, din_, dout, wslice=None, bslice=None):
            t = const.tile([din_ + 1, dout], F32, name=f"w_{name_w}_{wslice}")
            wsrc = dram[name_w] if wslice is None else dram[name_w][wslice]
            bsrc = dram[name_b] if bslice is None else dram[name_b][bslice]
            nc.sync.dma_start(t[0:din_, :], wsrc[:, :] if wslice is None else wsrc)
            nc.sync.dma_start(t[din_:din_ + 1, :], bsrc[None, :])
            return t

        Wn = {"mol": wcat("node_lin_mol_W", "node_lin_mol_b", 11, 64),
              "prot": wcat("node_lin_prot_W", "node_lin_prot_b", 15, 64)}
        We = {"mol": wcat("edge_lin_mol_W", "edge_lin_mol_b", 10, 64),
              "prot": wcat("edge_lin_prot_W", "edge_lin_prot_b", 10, 64)}
        W1 = {s: [wcat(f"{s}_conv_W1", f"{s}_conv_b1", 64, 64, l, l)
                  for l in range(3)] for s in ("mol", "prot")}
        W2 = {s: [wcat(f"{s}_conv_W2", f"{s}_conv_b2", 64, 64, l, l)
                  for l in range(3)] for s in ("mol", "prot")}

        sb_idx, sb_dstoff = {}, {}
        for s in sides:
            sd = sides[s]
            sb_idx[s] = load_const(f"{s}_gidx", [128, sd["E_core"] // 16], I16)
            sb_dstoff[s] = load_const(f"{s}_dstoff", [128, sd["T_total"]])
        sb_xTin = {"mol": load_const("mol_xT", [12, NC_MOL]),
                   "prot": load_const("prot_xT", [16, NC_PROT])}
        sb_pmat = {}
        for s in sides:
            sd = sides[s]
            t = const.tile([128, sd["nblk"], B], F32, name=f"pmat_{s}")
            nc.sync.dma_start(
                t[:], dram[f"{s}_pmat"].rearrange("(t p) g -> p t g", p=128))
            sb_pmat[s] = t

        # ---------------- DRAM internals
        dpool = tc.alloc_tile_pool(name="dram", bufs=1, space="DRAM")
        x_sh_d = {s: [dpool.tile([sides[s]["NC"], 64], F32,
                                 name=f"xsh_{s}_{l}") for l in range(4)]
                  for s in sides}
        x_full_d = {s: [dpool.tile([sides[s]["N"], 64], F32,
                                   addr_space="Shared", name=f"xfull_{s}_{l}")
                        for l in range(4)] for s in sides}

        # ---------------- long-lived x pools, then GINE-scoped pools
        xT_pool = tc.alloc_tile_pool(name="xT", bufs=2)
        xnf_pool = tc.alloc_tile_pool(name="xnf", bufs=2)
        gmem = tc.alloc_tile_pool(name="gmem", bufs=1)
        empp = tc.alloc_tile_pool(name="empp", bufs=2, space="PSUM")
        aggps = tc.alloc_tile_pool(name="aggps", bufs=2, space="PSUM")
        mlpps = tc.alloc_tile_pool(name="mlpps", bufs=2, space="PSUM")
        trps = tc.alloc_tile_pool(name="trps", bufs=2, space="PSUM")

        # pools for the GINE loop allocated up-front so nothing aliases the
        # ea_stream buffers (aliasing would stall the first gathers on a WAR)
        xg_pool = tc.alloc_tile_pool(name="xg", bufs=2)
        oh_pool = tc.alloc_tile_pool(name="oh", bufs=2)
        ea_stream = tc.alloc_tile_pool(name="ea_stream", bufs=2)

        # initial node features x0 first: gets the AllGathers in flight early
        xT_cur = {}
        xnf_cur = {}
        for s in sides:
            sd = sides[s]
            NCs, nblk = sd["NC"], sd["nblk"]
            ps = mlpps.tile([64, 512], F32, name="mlp_ps")
            nc.tensor.matmul(ps[:, 0:NCs], Wn[s][:], sb_xTin[s][:],
                             start=True, stop=True)
            xT = xT_pool.tile([65, NCs], F32, name=f"xT_{s}")
            nc.vector.tensor_copy(xT[0:64, :], ps[:, 0:NCs])
            nc.vector.memset(xT[64:65, :], 1.0)
            xnf = xnf_pool.tile([128, nblk, 64], F32, name=f"xnf_{s}")
            for b in range(nblk):
                tp = trps.tile([128, 64], F32, name="tr_ps")
                nc.tensor.transpose(tp[:], xT[0:64, b * 128:(b + 1) * 128],
                                    ident_sb[0:64, 0:64])
                nc.vector.tensor_copy(xnf[:, b, :], tp[:])
            nc.sync.dma_start(
                x_sh_d[s][0][:].rearrange("(t p) f -> p t f", p=128), xnf[:])
            nc.gpsimd.collective_compute(
                "AllGather", ALU.bypass, replica_groups=[list(range(R))],
                ins=[x_sh_d[s][0][:].opt()], outs=[x_full_d[s][0][:].opt()])
            xT_cur[s] = xT
            xnf_cur[s] = xnf

        # edge features em = [eattr;1] @ [We;be], edge-major [128, T, 64],
        # eattr^T streamed from DRAM per block
        em_sb = {}
        for s in sides:
            sd = sides[s]
            T_total, T_blk, nblk, D = (sd["T_total"], sd["T_blk"], sd["nblk"],
                                       sd["D"])
            em = gmem.tile([128, T_total, 64], F32, name=f"em_{s}")
            for b in range(nblk):
                ch = ea_stream.tile([11, T_blk * 128], F32, name="ea_chunk")
                nc.sync.dma_start(
                    ch[:],
                    dram[f"{s}_eaT"][:, b * T_blk * 128:(b + 1) * T_blk * 128])
                for t0 in range(0, T_blk, 8):
                    ng = min(8, T_blk - t0)
                    ps = empp.tile([128, 8, 64], F32, name="em_ps")
                    for j in range(ng):
                        nc.tensor.matmul(
                            ps[:, j, :],
                            ch[0:D + 1, (t0 + j) * 128:(t0 + j + 1) * 128],
                            We[s][:], start=True, stop=True)
                    nc.vector.tensor_copy(
                        em[:, b * T_blk + t0:b * T_blk + t0 + ng, :],
                        ps[:, 0:ng, :])
            em_sb[s] = em

        # GINE layers
        for l in range(3):
            for s in ("prot", "mol"):
                sd = sides[s]
                NCs, nblk, T_blk = sd["NC"], sd["nblk"], sd["T_blk"]
                xT_prev = xT_cur[s]
                hT = gmem.tile([65, NCs], F32, name=f"hT_{s}_{l}", bufs=2,
                               tag=f"hT_{s}")
                for b in range(nblk):
                    nE = T_blk * 128
                    xg = xg_pool.tile([128, T_blk, 64], F32, name="xg")
                    nc.gpsimd.dma_gather(
                        xg[:], x_full_d[s][l][:],
                        sb_idx[s][:, b * T_blk * 8:(b + 1) * T_blk * 8],
                        nE, nE, 64, single_packet=False)
                    msg = xg_pool.tile([128, T_blk, 64], F32, name="msg")
                    nc.vector.tensor_add(
                        msg[:], xg[:], em_sb[s][:, b * T_blk:(b + 1) * T_blk, :])
                    nc.scalar.activation(msg[:], msg[:], AF.Relu)
                    oh = oh_pool.tile([128, T_blk, 128], F32, name="oh")
                    nc.vector.tensor_tensor(
                        oh[:],
                        iota_sb[:, :].unsqueeze(1).broadcast_to([128, T_blk, 128]),
                        sb_dstoff[s][:, b * T_blk:(b + 1) * T_blk]
                            .unsqueeze(2).broadcast_to([128, T_blk, 128]),
                        ALU.is_equal)
                    agg = aggps.tile([64, 128], F32, name="agg_ps")
                    for t in range(T_blk):
                        nc.tensor.matmul(agg[:], msg[:, t, :], oh[:, t, :],
                                         start=(t == 0), stop=(t == T_blk - 1))
                    nc.vector.tensor_add(hT[0:64, b * 128:(b + 1) * 128],
                                         xT_prev[0:64, b * 128:(b + 1) * 128],
                                         agg[:])
                nc.vector.memset(hT[64:65, :], 1.0)
                ps1 = mlpps.tile([64, 512], F32, name="mlp_ps")
                nc.tensor.matmul(ps1[:, 0:NCs], W1[s][l][:], hT[:],
                                 start=True, stop=True)
                r1 = gmem.tile([65, NCs], F32, name=f"r1_{s}_{l}", bufs=2,
                               tag=f"r1_{s}")
                nc.scalar.activation(r1[0:64, :], ps1[:, 0:NCs], AF.Relu)
                nc.vector.memset(r1[64:65, :], 1.0)
                ps2 = mlpps.tile([64, 512], F32, name="mlp_ps")
                nc.tensor.matmul(ps2[:, 0:NCs], W2[s][l][:], r1[:],
                                 start=True, stop=True)
                xT = xT_pool.tile([65, NCs], F32, name=f"xT_{s}")
                nc.scalar.activation(xT[0:64, :], ps2[:, 0:NCs], AF.Relu)
                nc.vector.memset(xT[64:65, :], 1.0)
                xnf = xnf_pool.tile([128, nblk, 64], F32, name=f"xnf_{s}")
                for b in range(nblk):
                    tp = trps.tile([128, 64], F32, name="tr_ps")
                    nc.tensor.transpose(tp[:], xT[0:64, b * 128:(b + 1) * 128],
                                        ident_sb[0:64, 0:64])
                    nc.vector.tensor_copy(xnf[:, b, :], tp[:])
                nc.sync.dma_start(
                    x_sh_d[s][l + 1][:].rearrange("(t p) f -> p t f", p=128),
                    xnf[:])
                nc.gpsimd.collective_compute(
                    "AllGather", ALU.bypass, replica_groups=[list(range(R))],
                    ins=[x_sh_d[s][l + 1][:].opt()],
                    outs=[x_full_d[s][l + 1][:].opt()])
                xT_cur[s] = xT
                xnf_cur[s] = xnf

        # close GINE-scoped pools (LIFO per space)
        ea_stream.release()
        oh_pool.release()
        xg_pool.release()
        for p in (trps, mlpps, aggps, empp):
            p.release()
        gmem.release()

        # ---------------- attention phase
        # All large matmuls stream in float32r (fp32 data, ~1 cycle/row on
        # the PE for moving dim >= 256, measured ~1.6e-4 rel err). Scores are
        # computed twice: pass 1 in [q, k] layout for the row max, pass 2 in
        # [k, q] with the max folded in via the ones row; the exp tiles feed
        # the wV matmuls directly as moving data in the transposed domain
        # with a ones column in V producing the softmax denominator.
        a_sb = tc.alloc_tile_pool(name="attn_sb", bufs=1)
        smallps = tc.alloc_tile_pool(name="smallps", bufs=2, space="PSUM")
        s12ps = tc.alloc_tile_pool(name="s12ps", bufs=3, space="PSUM")
        otps = tc.alloc_tile_pool(name="otps", bufs=2, space="PSUM")
        exp_pool = tc.alloc_tile_pool(name="expt", bufs=4)

        # full x (both sides), transposed with ones row (float32r: DVE/ACT
        # writes round to the fp32r mantissa the PE streams at full rate)
        xT_full = {}
        for s in sides:
            sd = sides[s]
            Ns = sd["N"]
            nt = Ns // 128
            xT_f = a_sb.tile([65, Ns], F32R, name=f"xTfull_{s}")
            for t in range(nt):
                xf_nf = a_sb.tile([128, 64], F32, name="xf_nf", bufs=3,
                                  tag="xf_nf")
                nc.sync.dma_start(
                    xf_nf[:], x_full_d[s][3][t * 128:(t + 1) * 128, :])
                tp = smallps.tile([128, 512], F32, name="small_ps")
                nc.tensor.transpose(tp[0:64, 0:128], xf_nf[:], ident_sb[:])
                nc.vector.tensor_copy(xT_f[0:64, t * 128:(t + 1) * 128],
                                      tp[0:64, 0:128])
            ones_r = a_sb.tile([1, Ns], F32, name="ones_r", bufs=2,
                               tag="ones_r")
            nc.vector.memset(ones_r[:], 1.0)
            nc.vector.tensor_copy(xT_f[64:65, :], ones_r[:])
            xT_full[s] = xT_f

        # per-direction prep: folded per-head projector P65 [65, 65] with
        # rhs0_h = P65_h^T @ xT_q  (row 64 = c_h, the K-bias term)
        prep = {}
        for dirn, (qs, ks) in (("mp", ("mol", "prot")), ("pm", ("prot", "mol"))):
            NCq = sides[qs]["NC"]
            n_k128 = sides[ks]["N"] // 128
            Wd = dram[f"attn_{dirn}_W"]
            bd = dram[f"attn_{dirn}_b"]
            Wq = a_sb.tile([65, 64], F32, name=f"Wq_{dirn}")
            nc.sync.dma_start(Wq[0:64, :], Wd[0])
            nc.sync.dma_start(Wq[64:65, :], bd[0][None, :])
            Wv_f = a_sb.tile([65, 64], F32, name=f"Wvf_{dirn}")
            nc.sync.dma_start(Wv_f[0:64, :], Wd[2])
            nc.sync.dma_start(Wv_f[64:65, :], bd[2][None, :])
            Wv = a_sb.tile([65, 64], F32R, name=f"Wv_{dirn}")
            nc.vector.tensor_copy(Wv[:], Wv_f[:])
            Wk_raw = a_sb.tile([64, 64], F32, name=f"Wkraw_{dirn}")
            nc.sync.dma_start(Wk_raw[:], Wd[1])
            bk_cols = a_sb.tile([16, 4], F32, name=f"bkcols_{dirn}")
            nc.sync.dma_start(bk_cols[:], dram[f"bk_{dirn}_cols"][:])

            rhs0 = []
            cT = a_sb.tile([1, HEADS, NCq], F32, name=f"cT_{dirn}")
            xTq_r = a_sb.tile([65, NCq], F32R, name=f"xTq_{dirn}")
            nc.vector.tensor_copy(xTq_r[:], xT_cur[qs][:])
            for h in range(HEADS):
                tp = smallps.tile([128, 512], F32, name="small_ps")
                nc.tensor.transpose(tp[0:16, 0:64],
                                    Wk_raw[:, 16 * h:16 * h + 16],
                                    ident_sb[0:64, 0:64])
                wkt65 = a_sb.tile([16, 65], F32, name="wkt65", bufs=2,
                                  tag="wkt65")
                nc.vector.tensor_copy(wkt65[:, 0:64], tp[0:16, 0:64])
                nc.vector.tensor_copy(wkt65[:, 64:65], bk_cols[:, h:h + 1])
                tq = smallps.tile([128, 512], F32, name="small_ps")
                nc.tensor.transpose(tq[0:16, 0:65],
                                    Wq[:, 16 * h:16 * h + 16],
                                    ident_sb[0:65, 0:65])
                wqt = a_sb.tile([16, 65], F32, name="wqt", bufs=2, tag="wqt")
                nc.vector.tensor_copy(wqt[:], tq[0:16, 0:65])
                pps = smallps.tile([128, 512], F32, name="small_ps")
                nc.tensor.matmul(pps[0:65, 0:65], wqt[:], wkt65[:],
                                 start=True, stop=True)
                P65 = a_sb.tile([65, 65], F32R, name="P65", bufs=2, tag="P65")
                nc.scalar.activation(P65[:], pps[0:65, 0:65], AF.Copy,
                                     scale=0.25)
                rps = smallps.tile([128, 512], F32, name="small_ps")
                nc.tensor.matmul(rps[0:65, 0:NCq], P65[:], xTq_r[:],
                                 start=True, stop=True)
                r0 = a_sb.tile([65, NCq], F32R, name=f"rhs0_{dirn}_{h}")
                nc.vector.tensor_copy(r0[:], rps[0:65, 0:NCq])
                nc.vector.tensor_copy(cT[0:1, h, :], rps[64:65, 0:NCq])
                rhs0.append(r0)

            # V' [128, n_k128, 4, 17] with ones col
            Vp = a_sb.tile([128, n_k128, HEADS, 17], F32R, name=f"Vp_{dirn}")
            ones_v = a_sb.tile([128, n_k128, HEADS, 1], F32, name="ones_v",
                               bufs=2, tag="ones_v")
            nc.vector.memset(ones_v[:], 1.0)
            nc.vector.tensor_copy(Vp[:, :, :, 16:17], ones_v[:])
            for kt in range(n_k128):
                ps = smallps.tile([128, 512], F32, name="small_ps")
                nc.tensor.matmul(ps[0:128, 0:64],
                                 xT_full[ks][:, kt * 128:(kt + 1) * 128],
                                 Wv[:], start=True, stop=True)
                nc.vector.tensor_copy(
                    Vp[:, kt, :, 0:16],
                    ps[0:128, 0:64].rearrange("p (h d) -> p h d", h=HEADS))
            prep[dirn] = dict(rhs0=rhs0, Vp=Vp, cT=cT)

        # pass 1: row max m_h [1, NCq] per head ([q, k] layout)
        mT = {}
        for dirn, (qs, ks) in (("mp", ("mol", "prot")), ("pm", ("prot", "mol"))):
            NCq = sides[qs]["NC"]
            n_qt = NCq // 128
            n_k512 = sides[ks]["N"] // 512
            rhs0 = prep[dirn]["rhs0"]
            mTd = a_sb.tile([1, HEADS, NCq], F32, name=f"mT_{dirn}")
            for h in range(HEADS):
                for qt in range(n_qt):
                    mx = a_sb.tile([128, n_k512], F32, name="mx", bufs=2,
                                   tag="mx")
                    for cch in range(n_k512):
                        s1 = s12ps.tile([128, 512], F32, name="s12_ps")
                        nc.tensor.matmul(
                            s1[:],
                            rhs0[h][:, qt * 128:(qt + 1) * 128],
                            xT_full[ks][:, cch * 512:(cch + 1) * 512],
                            start=True, stop=True)
                        nc.vector.reduce_max(mx[:, cch:cch + 1], s1[:],
                                             axis=mybir.AxisListType.X)
                    mqt = a_sb.tile([128, 1], F32, name="mqt", bufs=2,
                                    tag="mqt")
                    nc.vector.reduce_max(mqt[:], mx[:],
                                         axis=mybir.AxisListType.X)
                    tp = smallps.tile([128, 512], F32, name="small_ps")
                    nc.tensor.transpose(tp[0:1, 0:128], mqt[:], ident_sb[:])
                    nc.vector.tensor_copy(
                        mTd[0:1, h, qt * 128:(qt + 1) * 128], tp[0:1, 0:128])
            mT[dirn] = mTd

        # pass 2 + wV
        H_sb = {}
        for dirn, (qs, ks) in (("mp", ("mol", "prot")), ("pm", ("prot", "mol"))):
            NCq = sides[qs]["NC"]
            n_qt = NCq // 128
            n_k128 = sides[ks]["N"] // 128
            rhs0 = prep[dirn]["rhs0"]
            Vp = prep[dirn]["Vp"]
            xTk = xT_full[ks]
            H = a_sb.tile([128, n_qt, 64], F32, name=f"H_{dirn}")
            # head groups: pack 2 heads side-by-side when NCq == 256 so the
            # score matmul streams a full 512-wide PSUM tile
            hgroups = ([(0, 1), (2, 3)] if NCq == 256
                       else [(0,), (1,), (2,), (3,)])
            for hg in hgroups:
                gw = NCq * len(hg)
                rhm = a_sb.tile([65, gw], F32R, name="rhm", bufs=2, tag="rhm")
                cm = a_sb.tile([1, gw], F32, name="cm", bufs=2, tag="cm")
                for j, h in enumerate(hg):
                    nc.vector.tensor_copy(
                        rhm[0:64, j * NCq:(j + 1) * NCq], rhs0[h][0:64, :])
                    nc.vector.tensor_sub(
                        cm[0:1, j * NCq:(j + 1) * NCq],
                        prep[dirn]["cT"][0:1, h, :], mT[dirn][0:1, h, :])
                nc.vector.tensor_copy(rhm[64:65, :], cm[:])

                oT = {h: otps.tile([17, NCq], F32, name="oT") for h in hg}
                pend = None
                for kc in range(n_k128):
                    ps = s12ps.tile([128, 512], F32, name="s12_ps")
                    nc.tensor.matmul(ps[:, 0:gw],
                                     xTk[:, kc * 128:(kc + 1) * 128],
                                     rhm[:], start=True, stop=True)
                    ex = exp_pool.tile([128, gw], F32R, name="ex",
                                       tag=f"ex_{dirn}")
                    nc.scalar.activation(ex[:], ps[:, 0:gw], AF.Exp)
                    if pend is not None:
                        pkc, pex = pend
                        for j, h in enumerate(hg):
                            nc.tensor.matmul(
                                oT[h][:], Vp[:, pkc, h, :],
                                pex[:, j * NCq:(j + 1) * NCq],
                                start=(pkc == 0), stop=False)
                    pend = (kc, ex)
                pkc, pex = pend
                for j, h in enumerate(hg):
                    nc.tensor.matmul(
                        oT[h][:], Vp[:, pkc, h, :],
                        pex[:, j * NCq:(j + 1) * NCq],
                        start=(pkc == 0), stop=True)
                # transpose oT back to node-major; col 16 = denominator
                for j, h in enumerate(hg):
                    oT_sb = a_sb.tile([17, NCq], F32, name="oT_sb", bufs=2,
                                      tag="oT_sb")
                    nc.vector.tensor_copy(oT_sb[:], oT[h][:])
                    for qt in range(n_qt):
                        mps = smallps.tile([128, 512], F32, name="small_ps")
                        nc.tensor.transpose(
                            mps[0:128, 0:17],
                            oT_sb[:, qt * 128:(qt + 1) * 128],
                            ident_sb[0:17, 0:17])
                        inv1 = a_sb.tile([128, 1], F32, name="inv1", bufs=2,
                                         tag="inv1")
                        nc.vector.reciprocal(inv1[:], mps[0:128, 16:17])
                        nc.vector.tensor_scalar_mul(
                            H[:, qt, 16 * h:16 * (h + 1)],
                            mps[0:128, 0:16], inv1[:])

            # residual: H += x (node-major shard)
            nc.vector.tensor_add(H[:], H[:], xnf_cur[qs][:])
            H_sb[dirn] = H

        # ---------------- pooling + output MLP
        zt_part_d = dpool.tile([128, B], F32, name="zt_part")
        zt_full_d = dpool.tile([128, B], F32, addr_space="Shared",
                               name="zt_full")
        for dirn, qs in (("mp", "mol"), ("pm", "prot")):
            n_qt = sides[qs]["NC"] // 128
            psz = smallps.tile([128, 512], F32, name="small_ps")
            for qt in range(n_qt):
                nc.tensor.matmul(psz[0:64, 0:B], H_sb[dirn][:, qt, :],
                                 sb_pmat[qs][:, qt, :],
                                 start=(qt == 0), stop=(qt == n_qt - 1))
            zpart = a_sb.tile([64, B], F32, name=f"zpart_{dirn}")
            nc.vector.tensor_copy(zpart[:], psz[0:64, 0:B])
            row0 = 0 if dirn == "mp" else 64
            nc.sync.dma_start(zt_part_d[row0:row0 + 64, :], zpart[:])
        nc.gpsimd.collective_compute(
            "AllReduce", ALU.add, replica_groups=[list(range(R))],
            ins=[zt_part_d[:].opt()], outs=[zt_full_d[:].opt()])
        zT = a_sb.tile([128, B], F32, name="zT")
        nc.sync.dma_start(zT[:], zt_full_d[:])

        fc1W = a_sb.tile([128, 64], F32, name="fc1W")
        nc.sync.dma_start(fc1W[:], dram["fc1_W"][:])
        fc1b = a_sb.tile([64, 1], F32, name="fc1b")
        nc.sync.dma_start(fc1b[:], dram["fc1_b"][:, None])
        fc2W = a_sb.tile([64, 1], F32, name="fc2W")
        nc.sync.dma_start(fc2W[:], dram["fc2_W"][:])
        fc2b = a_sb.tile([1, 1], F32, name="fc2b")
        nc.sync.dma_start(fc2b[:], dram["fc2_b"][:, None])

        ps = smallps.tile([128, 512], F32, name="small_ps")
        nc.tensor.matmul(ps[0:64, 0:B], fc1W[:], zT[:], start=True, stop=True)
        h1 = a_sb.tile([65, B], F32, name="h1")
        nc.scalar.activation(h1[0:64, :], ps[0:64, 0:B], AF.Relu, bias=fc1b[:])
        ps2 = smallps.tile([128, 512], F32, name="small_ps")
        nc.tensor.matmul(ps2[0:1, 0:B], fc2W[:], h1[0:64, :],
                         start=True, stop=True)
        osb = a_sb.tile([1, B], F32, name="osb")
        nc.scalar.activation(osb[:], ps2[0:1, 0:B], AF.Sigmoid, bias=fc2b[:])
        nc.sync.dma_start(out_d[:], osb[:])

        exp_pool.release()
        otps.release()
        s12ps.release()
        smallps.release()
        a_sb.release()
        xnf_pool.release()
        xT_pool.release()
        dpool.release()
        const.release()

    nc.compile()
    return nc



# ----------------------------------------------------------------- entry

def kernel(**inputs):
    global last_results
    meta, percore = _prep_host(inputs)
    key = (meta["mol_T_blk"], meta["prot_T_blk"])
    if key not in _CACHE:
        _CACHE[key] = _build(meta)
    nc = _CACHE[key]
    from concourse.bass_utils import run_bass_kernel_spmd
    res = run_bass_kernel_spmd(nc, percore, list(range(R)))
    last_results = res
    return np.asarray(res.results[0]["out"], np.float32).reshape(B)



# revision 24
# speedup vs baseline: 1.6831x; 1.0888x over previous
"""CrossGraphAttentionModel on 8 Trainium2 NeuronCores (Bass/Tile, SPMD).

Sharding: nodes/edges of both graphs are sharded 8 ways by (dst-sorted) node
range; 64-dim weights replicated. Per GINE layer each core dma_gathers x[src]
for its edge shard from an AllGathered copy of x in HBM, forms messages on
DVE/ACT, and scatter-adds them with one-hot matmuls on the PE (PSUM
accumulation), then runs the node MLP on its node shard and AllGathers the new
x. Cross-graph attention shards the query axis: scores are computed twice on
PE - once [q,k] for an exact row max, once [k,q] with the max folded into the
contraction via an appended ones row - so softmax needs only a single ACT exp
pass, and the exp tiles feed the wV matmul directly as lhsT with a ones column
in V producing the softmax denominator for free. Graph pooling is a one-hot
matmul with 1/count weights, AllReduced, followed by the tiny output MLP.

All floating point math runs on device in fp32; the host only sorts/pads
integer index structures and transposes/replicates input layouts.
"""

import numpy as np

R = 8
HID = 64
B = 32
HEADS = 4
HD = 16
N_MOL, N_PROT = 2048, 4096
E_MOL, E_PROT = 32768, 131072
NC_MOL, NC_PROT = N_MOL // R, N_PROT // R          # 256, 512
NBLK_MOL, NBLK_PROT = NC_MOL // 128, NC_PROT // 128  # 2, 4

_CACHE = {}
last_results = None


# ----------------------------------------------------------------- host prep

def _prep_edges(edge_index, eattr, nblk):
    """Sort edges by dst, partition into R cores x nblk 128-node windows,
    pad every window to T_blk tiles of 128 edges. Returns device layouts."""
    src, dst = np.asarray(edge_index[0]), np.asarray(edge_index[1])
    eattr = np.asarray(eattr, np.float32)
    order = np.argsort(dst, kind="stable")
    src_s, dst_s, ea_s = src[order], dst[order], eattr[order]
    nblocks = R * nblk
    blk = dst_s // 128
    counts = np.bincount(blk, minlength=nblocks)
    T_blk = int(np.ceil(counts.max() / 128))
    T_total = nblk * T_blk
    E_core = T_total * 128
    D = eattr.shape[1]

    gidx = np.zeros((R, E_core), np.int64)
    dstoff = np.full((R, E_core), -1.0, np.float32)
    ea_pad = np.zeros((R, E_core, D), np.float32)
    starts = np.concatenate([[0], np.cumsum(counts)])
    for c in range(R):
        for b in range(nblk):
            g = c * nblk + b
            cnt = counts[g]
            lo = starts[g]
            off = b * T_blk * 128
            gidx[c, off:off + cnt] = src_s[lo:lo + cnt]
            dstoff[c, off:off + cnt] = (dst_s[lo:lo + cnt] - g * 128)
            ea_pad[c, off:off + cnt] = ea_s[lo:lo + cnt]

    # gather indices wrapped [128, E_core/16] (i -> p=i%16, col=i//16), x8 replicated
    cols = E_core // 16
    w = gidx.reshape(R, cols, 16).transpose(0, 2, 1).astype(np.int16)
    gidx_sb = np.tile(w, (1, 8, 1)).copy()
    # dstoff [128, T_total]
    dstoff_sb = np.ascontiguousarray(
        dstoff.reshape(R, T_total, 128).transpose(0, 2, 1))
    # eattr^T with ones row: [11, E_core]
    eaT_packed = np.ascontiguousarray(np.concatenate(
        [ea_pad.transpose(0, 2, 1),
         np.ones((R, 1, E_core), np.float32)], axis=1))
    return dict(T_blk=T_blk, T_total=T_total, E_core=E_core, D=D,
                gidx_sb=gidx_sb, dstoff_sb=dstoff_sb, eaT_packed=eaT_packed)


def _prep_host(inp):
    """All integer/layout preprocessing. Returns (meta, per_core_inputs)."""
    mol = _prep_edges(inp["mol_edge_index"], inp["mol_eattr"], NBLK_MOL)
    prot = _prep_edges(inp["prot_edge_index"], inp["prot_eattr"], NBLK_PROT)

    # pool matrices with 1/count entries
    def pmat(batch, ncore):
        batch = np.asarray(batch)
        cnt = np.bincount(batch, minlength=B).astype(np.float32)
        inv = 1.0 / np.maximum(cnt, 1.0)
        m = np.zeros((R, ncore, B), np.float32)
        for c in range(R):
            sl = batch[c * ncore:(c + 1) * ncore]
            m[c, np.arange(ncore), sl] = inv[sl]
        return m

    mol_pmat = pmat(inp["mol_batch"], NC_MOL)
    prot_pmat = pmat(inp["prot_batch"], NC_PROT)

    # node features transposed per core with ones row
    def xt(x, ncore):
        x = np.asarray(x, np.float32)
        d = x.shape[1]
        out = np.zeros((R, d + 1, ncore), np.float32)
        for c in range(R):
            out[c, :d] = x[c * ncore:(c + 1) * ncore].T
            out[c, d] = 1.0
        return out

    mol_xT = xt(inp["mol_x"], NC_MOL)        # [R, 12, 256]
    prot_xT = xt(inp["prot_x"], NC_PROT)     # [R, 16, 512]

    iota = np.tile(np.arange(128, dtype=np.float32), (128, 1))
    ident = np.eye(128, dtype=np.float32)
    # merge matrix for attention wV output: rows 0-15 -> cols 0-15 (V hi),
    # rows 16-31 -> cols 0-15 (V lo), row 32 -> col 16 (denominator)
    merge33 = np.zeros((33, 17), np.float32)
    merge33[np.arange(16), np.arange(16)] = 1.0
    merge33[16 + np.arange(16), np.arange(16)] = 1.0
    merge33[32, 16] = 1.0

    # attn K-bias as per-head columns [16, 4]
    def bcols(b):  # [64] -> [16, 4]
        return np.ascontiguousarray(np.asarray(b, np.float32).reshape(4, 16).T)

    percore = []
    for c in range(R):
        m = {
            "mol_xT": mol_xT[c], "prot_xT": prot_xT[c],
            "mol_eaT": mol["eaT_packed"][c], "prot_eaT": prot["eaT_packed"][c],
            "mol_gidx": mol["gidx_sb"][c], "prot_gidx": prot["gidx_sb"][c],
            "mol_dstoff": mol["dstoff_sb"][c], "prot_dstoff": prot["dstoff_sb"][c],
            "mol_pmat": mol_pmat[c], "prot_pmat": prot_pmat[c],
            "iota": iota, "ident": ident, "merge33": merge33,
            "bk_mp_cols": bcols(np.asarray(inp["attn_mp_b"])[1]),
            "bk_pm_cols": bcols(np.asarray(inp["attn_pm_b"])[1]),
        }
        for k in ("node_lin_mol_W", "node_lin_mol_b", "node_lin_prot_W",
                  "node_lin_prot_b", "edge_lin_mol_W", "edge_lin_mol_b",
                  "edge_lin_prot_W", "edge_lin_prot_b",
                  "mol_conv_W1", "mol_conv_b1", "mol_conv_W2", "mol_conv_b2",
                  "prot_conv_W1", "prot_conv_b1", "prot_conv_W2", "prot_conv_b2",
                  "attn_mp_W", "attn_mp_b", "attn_pm_W", "attn_pm_b",
                  "fc1_W", "fc1_b", "fc2_W", "fc2_b"):
            m[k] = np.asarray(inp[k], np.float32)
        percore.append(m)

    meta = dict(mol_T_blk=mol["T_blk"], mol_T_total=mol["T_total"],
                mol_E_core=mol["E_core"],
                prot_T_blk=prot["T_blk"], prot_T_total=prot["T_total"],
                prot_E_core=prot["E_core"])
    return meta, percore


# ------------------------------------------------------------- device build

def _build(meta):
    import concourse.bacc as bacc
    import concourse.mybir as mybir
    import concourse.tile as tile

    F32 = mybir.dt.float32
    F32R = mybir.dt.float32r
    BF16 = mybir.dt.bfloat16
    I16 = mybir.dt.int16
    AF = mybir.ActivationFunctionType
    ALU = mybir.AluOpType

    nc = bacc.Bacc("TRN2", target_bir_lowering=False, debug=False,
                   num_devices=R)

    # ---- I/O declarations
    dram = {}

    def din(name, shape, dtype=F32):
        dram[name] = nc.dram_tensor(name, list(shape), dtype,
                                    kind="ExternalInput")
        return dram[name]

    mT, mE = meta["mol_T_total"], meta["mol_E_core"]
    pT, pE = meta["prot_T_total"], meta["prot_E_core"]

    din("mol_xT", [12, NC_MOL]); din("prot_xT", [16, NC_PROT])
    din("mol_eaT", [11, mE]); din("prot_eaT", [11, pE])
    din("mol_gidx", [128, mE // 16], I16); din("prot_gidx", [128, pE // 16], I16)
    din("mol_dstoff", [128, mT]); din("prot_dstoff", [128, pT])
    din("mol_pmat", [NC_MOL, B]); din("prot_pmat", [NC_PROT, B])
    din("iota", [128, 128]); din("ident", [128, 128])
    din("merge33", [33, 17])
    din("bk_mp_cols", [16, 4]); din("bk_pm_cols", [16, 4])
    din("node_lin_mol_W", [11, 64]); din("node_lin_mol_b", [64])
    din("node_lin_prot_W", [15, 64]); din("node_lin_prot_b", [64])
    din("edge_lin_mol_W", [10, 64]); din("edge_lin_mol_b", [64])
    din("edge_lin_prot_W", [10, 64]); din("edge_lin_prot_b", [64])
    for s in ("mol", "prot"):
        din(f"{s}_conv_W1", [3, 64, 64]); din(f"{s}_conv_b1", [3, 64])
        din(f"{s}_conv_W2", [3, 64, 64]); din(f"{s}_conv_b2", [3, 64])
    din("attn_mp_W", [3, 64, 64]); din("attn_mp_b", [3, 64])
    din("attn_pm_W", [3, 64, 64]); din("attn_pm_b", [3, 64])
    din("fc1_W", [128, 64]); din("fc1_b", [64])
    din("fc2_W", [64, 1]); din("fc2_b", [1])

    out_d = nc.dram_tensor("out", [1, B], F32, kind="ExternalOutput")

    sides = {
        "mol": dict(N=N_MOL, NC=NC_MOL, nblk=NBLK_MOL, T_blk=meta["mol_T_blk"],
                    T_total=mT, E_core=mE, D=10),
        "prot": dict(N=N_PROT, NC=NC_PROT, nblk=NBLK_PROT,
                     T_blk=meta["prot_T_blk"], T_total=pT, E_core=pE, D=10),
    }

    with tile.TileContext(nc) as tc:
        # ---------------- persistent SBUF constants
        const = tc.alloc_tile_pool(name="const", bufs=1)

        def load_const(name, shape, dtype=F32, src=None):
            t = const.tile(list(shape), dtype, name=f"c_{name}")
            nc.sync.dma_start(t[:], (dram[name] if src is None else src)[:])
            return t

        iota_sb = load_const("iota", [128, 128])
        ident_sb = load_const("ident", [128, 128])

        def wcat(name_w, name_b, din_, dout, wslice=None, bslice=None):
            t = const.tile([din_ + 1, dout], F32, name=f"w_{name_w}_{wslice}")
            wsrc = dram[name_w] if wslice is None else dram[name_w][wslice]
            bsrc = dram[name_b] if bslice is None else dram[name_b][bslice]
            nc.sync.dma_start(t[0:din_, :], wsrc[:, :] if wslice is None else wsrc)
            nc.sync.dma_start(t[din_:din_ + 1, :], bsrc[None, :])
            return t

        Wn = {"mol": wcat("node_lin_mol_W", "node_lin_mol_b", 11, 64),
              "prot": wcat("node_lin_prot_W", "node_lin_prot_b", 15, 64)}
        We = {"mol": wcat("edge_lin_mol_W", "edge_lin_mol_b", 10, 64),
              "prot": wcat("edge_lin_prot_W", "edge_lin_prot_b", 10, 64)}
        W1 = {s: [wcat(f"{s}_conv_W1", f"{s}_conv_b1", 64, 64, l, l)
                  for l in range(3)] for s in ("mol", "prot")}
        W2 = {s: [wcat(f"{s}_conv_W2", f"{s}_conv_b2", 64, 64, l, l)
                  for l in range(3)] for s in ("mol", "prot")}

        sb_idx, sb_dstoff = {}, {}
        for s in sides:
            sd = sides[s]
            sb_idx[s] = load_const(f"{s}_gidx", [128, sd["E_core"] // 16], I16)
            sb_dstoff[s] = load_const(f"{s}_dstoff", [128, sd["T_total"]])
        sb_xTin = {"mol": load_const("mol_xT", [12, NC_MOL]),
                   "prot": load_const("prot_xT", [16, NC_PROT])}
        sb_pmat = {}
        for s in sides:
            sd = sides[s]
            t = const.tile([128, sd["nblk"], B], F32, name=f"pmat_{s}")
            nc.sync.dma_start(
                t[:], dram[f"{s}_pmat"].rearrange("(t p) g -> p t g", p=128))
            sb_pmat[s] = t

        # ---------------- DRAM internals
        dpool = tc.alloc_tile_pool(name="dram", bufs=1, space="DRAM")
        x_sh_d = {s: [dpool.tile([sides[s]["NC"], 64], F32,
                                 name=f"xsh_{s}_{l}") for l in range(3)]
                  for s in sides}
        x_full_d = {s: [dpool.tile([sides[s]["N"], 64], F32,
                                   addr_space="Shared", name=f"xfull_{s}_{l}")
                        for l in range(3)] for s in sides}
        x_shT_d = {s: dpool.tile([65, sides[s]["NC"]], F32,
                                 name=f"xshT_{s}") for s in sides}
        x_fullT_d = {s: dpool.tile([R, 65, sides[s]["NC"]], F32,
                                   addr_space="Shared", name=f"xfullT_{s}")
                     for s in sides}

        # ---------------- long-lived x pools, then GINE-scoped pools
        xT_pool = tc.alloc_tile_pool(name="xT", bufs=2)
        xnf_pool = tc.alloc_tile_pool(name="xnf", bufs=2)
        gmem = tc.alloc_tile_pool(name="gmem", bufs=1)
        empp = tc.alloc_tile_pool(name="empp", bufs=2, space="PSUM")
        aggps = tc.alloc_tile_pool(name="aggps", bufs=2, space="PSUM")
        mlpps = tc.alloc_tile_pool(name="mlpps", bufs=2, space="PSUM")
        trps = tc.alloc_tile_pool(name="trps", bufs=2, space="PSUM")

        # pools for the GINE loop allocated up-front so nothing aliases the
        # ea_stream buffers (aliasing would stall the first gathers on a WAR)
        xg_pool = tc.alloc_tile_pool(name="xg", bufs=2)
        oh_pool = tc.alloc_tile_pool(name="oh", bufs=2)
        ea_stream = tc.alloc_tile_pool(name="ea_stream", bufs=2)

        # initial node features x0 first: gets the AllGathers in flight early
        xT_cur = {}
        xnf_cur = {}
        for s in ("prot", "mol"):
            sd = sides[s]
            NCs, nblk = sd["NC"], sd["nblk"]
            ps = mlpps.tile([64, 512], F32, name="mlp_ps")
            nc.tensor.matmul(ps[:, 0:NCs], Wn[s][:], sb_xTin[s][:],
                             start=True, stop=True)
            xT = xT_pool.tile([65, NCs], F32, name=f"xT_{s}")
            nc.vector.tensor_copy(xT[0:64, :], ps[:, 0:NCs])
            nc.vector.memset(xT[64:65, :], 1.0)
            xnf = xnf_pool.tile([128, nblk, 64], F32, name=f"xnf_{s}")
            for b in range(nblk):
                tp = trps.tile([128, 64], F32, name="tr_ps")
                nc.tensor.transpose(tp[:], xT[0:64, b * 128:(b + 1) * 128],
                                    ident_sb[0:64, 0:64])
                nc.vector.tensor_copy(xnf[:, b, :], tp[:])
            nc.sync.dma_start(
                x_sh_d[s][0][:].rearrange("(t p) f -> p t f", p=128), xnf[:])
            nc.gpsimd.collective_compute(
                "AllGather", ALU.bypass, replica_groups=[list(range(R))],
                ins=[x_sh_d[s][0][:].opt()], outs=[x_full_d[s][0][:].opt()])
            xT_cur[s] = xT
            xnf_cur[s] = xnf

        # attention weight prep: P65_h = 0.25 * Wq_h^T (Wk_h | bk_h) folded
        # per-head projector; depends only on weights so it overlaps the
        # first gathers
        atw = {}
        for dirn in ("mp", "pm"):
            Wd = dram[f"attn_{dirn}_W"]
            bd = dram[f"attn_{dirn}_b"]
            Wq = const.tile([65, 64], F32, name=f"Wq_{dirn}")
            nc.sync.dma_start(Wq[0:64, :], Wd[0])
            nc.sync.dma_start(Wq[64:65, :], bd[0][None, :])
            Wv_f = const.tile([65, 64], F32, name=f"Wvf_{dirn}")
            nc.sync.dma_start(Wv_f[0:64, :], Wd[2])
            nc.sync.dma_start(Wv_f[64:65, :], bd[2][None, :])
            Wv = const.tile([65, 64], F32R, name=f"Wv_{dirn}")
            nc.vector.tensor_copy(Wv[:], Wv_f[:])
            Wk_raw = const.tile([64, 64], F32, name=f"Wkraw_{dirn}")
            nc.sync.dma_start(Wk_raw[:], Wd[1])
            bk_cols = const.tile([16, 4], F32, name=f"bkcols_{dirn}")
            nc.sync.dma_start(bk_cols[:], dram[f"bk_{dirn}_cols"][:])
            P65s = []
            for h in range(HEADS):
                pv = empp.tile([128, 8, 64], F32, name="em_ps")[:].rearrange(
                    "p a b -> p (a b)")
                nc.tensor.transpose(pv[0:16, 0:64],
                                    Wk_raw[:, 16 * h:16 * h + 16],
                                    ident_sb[0:64, 0:64])
                wkt65 = const.tile([16, 65], F32, name=f"wkt65_{dirn}_{h}")
                nc.vector.tensor_copy(wkt65[:, 0:64], pv[0:16, 0:64])
                nc.vector.tensor_copy(wkt65[:, 64:65], bk_cols[:, h:h + 1])
                qv = empp.tile([128, 8, 64], F32, name="em_ps")[:].rearrange(
                    "p a b -> p (a b)")
                nc.tensor.transpose(qv[0:16, 0:65],
                                    Wq[:, 16 * h:16 * h + 16],
                                    ident_sb[0:65, 0:65])
                wqt = const.tile([16, 65], F32, name=f"wqt_{dirn}_{h}")
                nc.vector.tensor_copy(wqt[:], qv[0:16, 0:65])
                ppv = empp.tile([128, 8, 64], F32, name="em_ps")[:].rearrange(
                    "p a b -> p (a b)")
                nc.tensor.matmul(ppv[0:65, 0:65], wqt[:], wkt65[:],
                                 start=True, stop=True)
                P65 = const.tile([65, 65], F32R, name=f"P65_{dirn}_{h}")
                nc.scalar.activation(P65[:], ppv[0:65, 0:65], AF.Copy,
                                     scale=0.25)
                P65s.append(P65)
            atw[dirn] = dict(Wv=Wv, P65s=P65s)

        # edge features em = [eattr;1] @ [We;be], edge-major [128, T, 64],
        # eattr^T streamed from DRAM per block
        em_sb = {}
        for s in sides:
            sd = sides[s]
            T_total, T_blk, nblk, D = (sd["T_total"], sd["T_blk"], sd["nblk"],
                                       sd["D"])
            em = gmem.tile([128, T_total, 64], F32, name=f"em_{s}")
            for b in range(nblk):
                ch = ea_stream.tile([11, T_blk * 128], F32, name="ea_chunk")
                nc.sync.dma_start(
                    ch[:],
                    dram[f"{s}_eaT"][:, b * T_blk * 128:(b + 1) * T_blk * 128])
                for t0 in range(0, T_blk, 8):
                    ng = min(8, T_blk - t0)
                    ps = empp.tile([128, 8, 64], F32, name="em_ps")
                    for j in range(ng):
                        nc.tensor.matmul(
                            ps[:, j, :],
                            ch[0:D + 1, (t0 + j) * 128:(t0 + j + 1) * 128],
                            We[s][:], start=True, stop=True)
                    nc.vector.tensor_copy(
                        em[:, b * T_blk + t0:b * T_blk + t0 + ng, :],
                        ps[:, 0:ng, :])
            em_sb[s] = em

        # GINE layers
        for l in range(3):
            for s in ("prot", "mol"):
                sd = sides[s]
                NCs, nblk, T_blk = sd["NC"], sd["nblk"], sd["T_blk"]
                xT_prev = xT_cur[s]
                hT = gmem.tile([65, NCs], F32, name=f"hT_{s}_{l}", bufs=2,
                               tag=f"hT_{s}")
                for b in range(nblk):
                    nE = T_blk * 128
                    xg = xg_pool.tile([128, T_blk, 64], F32, name="xg")
                    nc.gpsimd.dma_gather(
                        xg[:], x_full_d[s][l][:],
                        sb_idx[s][:, b * T_blk * 8:(b + 1) * T_blk * 8],
                        nE, nE, 64, single_packet=False)
                    oh = oh_pool.tile([128, T_blk, 128], F32, name="oh")
                    nc.vector.tensor_tensor(
                        oh[:],
                        iota_sb[:, :].unsqueeze(1).broadcast_to([128, T_blk, 128]),
                        sb_dstoff[s][:, b * T_blk:(b + 1) * T_blk]
                            .unsqueeze(2).broadcast_to([128, T_blk, 128]),
                        ALU.is_equal)
                    msg = xg_pool.tile([128, T_blk, 64], F32, name="msg")
                    nc.vector.tensor_add(
                        msg[:], xg[:], em_sb[s][:, b * T_blk:(b + 1) * T_blk, :])
                    nc.scalar.activation(msg[:], msg[:], AF.Relu)
                    agg = aggps.tile([64, 128], F32, name="agg_ps")
                    for t in range(T_blk):
                        nc.tensor.matmul(agg[:], msg[:, t, :], oh[:, t, :],
                                         start=(t == 0), stop=(t == T_blk - 1))
                    nc.vector.tensor_add(hT[0:64, b * 128:(b + 1) * 128],
                                         xT_prev[0:64, b * 128:(b + 1) * 128],
                                         agg[:])
                nc.vector.memset(hT[64:65, :], 1.0)
                ps1 = mlpps.tile([64, 512], F32, name="mlp_ps")
                nc.tensor.matmul(ps1[:, 0:NCs], W1[s][l][:], hT[:],
                                 start=True, stop=True)
                r1 = gmem.tile([65, NCs], F32, name=f"r1_{s}_{l}", bufs=2,
                               tag=f"r1_{s}")
                nc.scalar.activation(r1[0:64, :], ps1[:, 0:NCs], AF.Relu)
                nc.vector.memset(r1[64:65, :], 1.0)
                ps2 = mlpps.tile([64, 512], F32, name="mlp_ps")
                nc.tensor.matmul(ps2[:, 0:NCs], W2[s][l][:], r1[:],
                                 start=True, stop=True)
                xT = xT_pool.tile([65, NCs], F32, name=f"xT_{s}")
                nc.scalar.activation(xT[0:64, :], ps2[:, 0:NCs], AF.Relu)
                nc.vector.memset(xT[64:65, :], 1.0)
                if l == 2:
                    # final layer: AllGather the transposed x directly for
                    # attention (skips the node-major transpose round trip)
                    nc.sync.dma_start(x_shT_d[s][:], xT[:])
                    nc.gpsimd.collective_compute(
                        "AllGather", ALU.bypass,
                        replica_groups=[list(range(R))],
                        ins=[x_shT_d[s][:].opt()],
                        outs=[x_fullT_d[s][:].opt()])
                xnf = xnf_pool.tile([128, nblk, 64], F32, name=f"xnf_{s}")
                for b in range(nblk):
                    tp = trps.tile([128, 64], F32, name="tr_ps")
                    nc.tensor.transpose(tp[:], xT[0:64, b * 128:(b + 1) * 128],
                                        ident_sb[0:64, 0:64])
                    nc.vector.tensor_copy(xnf[:, b, :], tp[:])
                if l < 2:
                    nc.sync.dma_start(
                        x_sh_d[s][l + 1][:].rearrange("(t p) f -> p t f",
                                                      p=128),
                        xnf[:])
                    nc.gpsimd.collective_compute(
                        "AllGather", ALU.bypass,
                        replica_groups=[list(range(R))],
                        ins=[x_sh_d[s][l + 1][:].opt()],
                        outs=[x_full_d[s][l + 1][:].opt()])
                xT_cur[s] = xT
                xnf_cur[s] = xnf

        # close GINE-scoped pools (LIFO per space)
        ea_stream.release()
        oh_pool.release()
        xg_pool.release()
        for p in (trps, mlpps, aggps, empp):
            p.release()
        gmem.release()

        # ---------------- attention phase
        # All large matmuls stream in float32r (fp32 data, ~1 cycle/row on
        # the PE for moving dim >= 256, measured ~1.6e-4 rel err). Scores are
        # computed twice: pass 1 in [q, k] layout for the row max, pass 2 in
        # [k, q] with the max folded in via the ones row; the exp tiles feed
        # the wV matmuls directly as moving data in the transposed domain
        # with a ones column in V producing the softmax denominator.
        a_sb = tc.alloc_tile_pool(name="attn_sb", bufs=1)
        smallps = tc.alloc_tile_pool(name="smallps", bufs=2, space="PSUM")
        s12ps = tc.alloc_tile_pool(name="s12ps", bufs=3, space="PSUM")
        otps = tc.alloc_tile_pool(name="otps", bufs=2, space="PSUM")
        exp_pool = tc.alloc_tile_pool(name="expt", bufs=4)

        # full x (both sides), transposed with ones row, from the transposed
        # AllGather (float32r: the DVE round-copy feeds the PE at full rate)
        xT_full = {}
        for s in ("prot", "mol"):
            Ns = sides[s]["N"]
            xT_ld = a_sb.tile([65, Ns], F32, name=f"xTld_{s}")
            NCs = sides[s]["NC"]
            for r in range(R):
                nc.sync.dma_start(xT_ld[:, r * NCs:(r + 1) * NCs],
                                  x_fullT_d[s][r])
            xT_f = a_sb.tile([65, Ns], F32R, name=f"xTfull_{s}")
            nc.vector.tensor_copy(xT_f[:], xT_ld[:])
            xT_full[s] = xT_f

        # per-direction prep: folded per-head projector P65 [65, 65] with
        # rhs0_h = P65_h^T @ xT_q  (row 64 = c_h, the K-bias term)
        prep = {}
        for dirn, (qs, ks) in (("mp", ("mol", "prot")), ("pm", ("prot", "mol"))):
            NCq = sides[qs]["NC"]
            n_k128 = sides[ks]["N"] // 128
            Wv = atw[dirn]["Wv"]

            rhs0 = []
            cT = a_sb.tile([1, HEADS, NCq], F32, name=f"cT_{dirn}")
            xTq_r = a_sb.tile([65, NCq], F32R, name=f"xTq_{dirn}")
            nc.vector.tensor_copy(xTq_r[:], xT_cur[qs][:])
            for h in range(HEADS):
                rps = smallps.tile([128, 512], F32, name="small_ps")
                nc.tensor.matmul(rps[0:65, 0:NCq], atw[dirn]["P65s"][h][:],
                                 xTq_r[:], start=True, stop=True)
                r0 = a_sb.tile([65, NCq], F32R, name=f"rhs0_{dirn}_{h}")
                nc.vector.tensor_copy(r0[:], rps[0:65, 0:NCq])
                nc.vector.tensor_copy(cT[0:1, h, :], rps[64:65, 0:NCq])
                rhs0.append(r0)

            # V' [128, n_k128, 4, 17] with ones col
            Vp = a_sb.tile([128, n_k128, HEADS, 17], F32R, name=f"Vp_{dirn}")
            ones_v = a_sb.tile([128, n_k128, HEADS, 1], F32, name="ones_v",
                               bufs=2, tag="ones_v")
            nc.vector.memset(ones_v[:], 1.0)
            nc.vector.tensor_copy(Vp[:, :, :, 16:17], ones_v[:])
            for kt in range(n_k128):
                ps = smallps.tile([128, 512], F32, name="small_ps")
                nc.tensor.matmul(ps[0:128, 0:64],
                                 xT_full[ks][:, kt * 128:(kt + 1) * 128],
                                 Wv[:], start=True, stop=True)
                nc.vector.tensor_copy(
                    Vp[:, kt, :, 0:16],
                    ps[0:128, 0:64].rearrange("p (h d) -> p h d", h=HEADS))
            prep[dirn] = dict(rhs0=rhs0, Vp=Vp, cT=cT)

        # pass 1: row max m_h [1, NCq] per head ([q, k] layout)
        mT = {}
        for dirn, (qs, ks) in (("mp", ("mol", "prot")), ("pm", ("prot", "mol"))):
            NCq = sides[qs]["NC"]
            n_qt = NCq // 128
            n_k512 = sides[ks]["N"] // 512
            rhs0 = prep[dirn]["rhs0"]
            mTd = a_sb.tile([1, HEADS, NCq], F32, name=f"mT_{dirn}")
            for h in range(HEADS):
                for qt in range(n_qt):
                    mx = a_sb.tile([128, n_k512], F32, name="mx", bufs=2,
                                   tag="mx")
                    for cch in range(n_k512):
                        s1 = s12ps.tile([128, 512], F32, name="s12_ps")
                        nc.tensor.matmul(
                            s1[:],
                            rhs0[h][:, qt * 128:(qt + 1) * 128],
                            xT_full[ks][:, cch * 512:(cch + 1) * 512],
                            start=True, stop=True)
                        nc.vector.reduce_max(mx[:, cch:cch + 1], s1[:],
                                             axis=mybir.AxisListType.X)
                    mqt = a_sb.tile([128, 1], F32, name="mqt", bufs=2,
                                    tag="mqt")
                    nc.vector.reduce_max(mqt[:], mx[:],
                                         axis=mybir.AxisListType.X)
                    tp = smallps.tile([128, 512], F32, name="small_ps")
                    nc.tensor.transpose(tp[0:1, 0:128], mqt[:], ident_sb[:])
                    nc.vector.tensor_copy(
                        mTd[0:1, h, qt * 128:(qt + 1) * 128], tp[0:1, 0:128])
            mT[dirn] = mTd

        # pass 2 + wV
        H_sb = {}
        for dirn, (qs, ks) in (("mp", ("mol", "prot")), ("pm", ("prot", "mol"))):
            NCq = sides[qs]["NC"]
            n_qt = NCq // 128
            n_k128 = sides[ks]["N"] // 128
            rhs0 = prep[dirn]["rhs0"]
            Vp = prep[dirn]["Vp"]
            xTk = xT_full[ks]
            H = a_sb.tile([128, n_qt, 64], F32, name=f"H_{dirn}")
            # head groups: pack 2 heads side-by-side when NCq == 256 so the
            # score matmul streams a full 512-wide PSUM tile
            hgroups = ([(0, 1), (2, 3)] if NCq == 256
                       else [(0,), (1,), (2,), (3,)])
            for hg in hgroups:
                gw = NCq * len(hg)
                rhm = a_sb.tile([65, gw], F32R, name="rhm", bufs=2, tag="rhm")
                cm = a_sb.tile([1, gw], F32, name="cm", bufs=2, tag="cm")
                for j, h in enumerate(hg):
                    nc.vector.tensor_copy(
                        rhm[0:64, j * NCq:(j + 1) * NCq], rhs0[h][0:64, :])
                    nc.vector.tensor_sub(
                        cm[0:1, j * NCq:(j + 1) * NCq],
                        prep[dirn]["cT"][0:1, h, :], mT[dirn][0:1, h, :])
                nc.vector.tensor_copy(rhm[64:65, :], cm[:])

                oT = {h: otps.tile([17, NCq], F32, name="oT") for h in hg}
                pend = None
                for kc in range(n_k128):
                    ps = s12ps.tile([128, 512], F32, name="s12_ps")
                    nc.tensor.matmul(ps[:, 0:gw],
                                     xTk[:, kc * 128:(kc + 1) * 128],
                                     rhm[:], start=True, stop=True)
                    ex = exp_pool.tile([128, gw], F32R, name="ex",
                                       tag=f"ex_{dirn}")
                    nc.scalar.activation(ex[:], ps[:, 0:gw], AF.Exp)
                    if pend is not None:
                        pkc, pex = pend
                        for j, h in enumerate(hg):
                            nc.tensor.matmul(
                                oT[h][:], Vp[:, pkc, h, :],
                                pex[:, j * NCq:(j + 1) * NCq],
                                start=(pkc == 0), stop=False)
                    pend = (kc, ex)
                pkc, pex = pend
                for j, h in enumerate(hg):
                    nc.tensor.matmul(
                        oT[h][:], Vp[:, pkc, h, :],
                        pex[:, j * NCq:(j + 1) * NCq],
                        start=(pkc == 0), stop=True)
                # transpose oT back to node-major; col 16 = denominator
                for j, h in enumerate(hg):
                    oT_sb = a_sb.tile([17, NCq], F32, name="oT_sb", bufs=2,
                                      tag="oT_sb")
                    nc.vector.tensor_copy(oT_sb[:], oT[h][:])
                    for qt in range(n_qt):
                        mps = smallps.tile([128, 512], F32, name="small_ps")
                        nc.tensor.transpose(
                            mps[0:128, 0:17],
                            oT_sb[:, qt * 128:(qt + 1) * 128],
                            ident_sb[0:17, 0:17])
                        inv1 = a_sb.tile([128, 1], F32, name="inv1", bufs=2,
                                         tag="inv1")
                        nc.vector.reciprocal(inv1[:], mps[0:128, 16:17])
                        nc.vector.tensor_scalar_mul(
                            H[:, qt, 16 * h:16 * (h + 1)],
                            mps[0:128, 0:16], inv1[:])

            # residual: H += x (node-major shard)
            nc.vector.tensor_add(H[:], H[:], xnf_cur[qs][:])
            H_sb[dirn] = H

        # ---------------- pooling + output MLP
        zt_part_d = dpool.tile([128, B], F32, name="zt_part")
        zt_full_d = dpool.tile([128, B], F32, addr_space="Shared",
                               name="zt_full")
        for dirn, qs in (("mp", "mol"), ("pm", "prot")):
            n_qt = sides[qs]["NC"] // 128
            psz = smallps.tile([128, 512], F32, name="small_ps")
            for qt in range(n_qt):
                nc.tensor.matmul(psz[0:64, 0:B], H_sb[dirn][:, qt, :],
                                 sb_pmat[qs][:, qt, :],
                                 start=(qt == 0), stop=(qt == n_qt - 1))
            zpart = a_sb.tile([64, B], F32, name=f"zpart_{dirn}")
            nc.vector.tensor_copy(zpart[:], psz[0:64, 0:B])
            row0 = 0 if dirn == "mp" else 64
            nc.sync.dma_start(zt_part_d[row0:row0 + 64, :], zpart[:])
        nc.gpsimd.collective_compute(
            "AllReduce", ALU.add, replica_groups=[list(range(R))],
            ins=[zt_part_d[:].opt()], outs=[zt_full_d[:].opt()])
        zT = a_sb.tile([128, B], F32, name="zT")
        nc.sync.dma_start(zT[:], zt_full_d[:])

        fc1W = a_sb.tile([128, 64], F32, name="fc1W")
        nc.sync.dma_start(fc1W[:], dram["fc1_W"][:])
        fc1b = a_sb.tile([64, 1], F32, name="fc1b")
        nc.sync.dma_start(fc1b[:], dram["fc1_b"][:, None])
        fc2W = a_sb.tile([64, 1], F32, name="fc2W")
        nc.sync.dma_start(fc2W[:], dram["fc2_W"][:])
        fc2b = a_sb.tile([1, 1], F32, name="fc2b")
        nc.sync.dma_start(fc2b[:], dram["fc2_b"][:, None])

        ps = smallps.tile([128, 512], F32, name="small_ps")
        nc.tensor.matmul(ps[0:64, 0:B], fc1W[:], zT[:], start=True, stop=True)
        h1 = a_sb.tile([65, B], F32, name="h1")
        nc.scalar.activation(h1[0:64, :], ps[0:64, 0:B], AF.Relu, bias=fc1b[:])
        ps2 = smallps.tile([128, 512], F32, name="small_ps")
        nc.tensor.matmul(ps2[0:1, 0:B], fc2W[:], h1[0:64, :],
                         start=True, stop=True)
        osb = a_sb.tile([1, B], F32, name="osb")
        nc.scalar.activation(osb[:], ps2[0:1, 0:B], AF.Sigmoid, bias=fc2b[:])
        nc.sync.dma_start(out_d[:], osb[:])

        exp_pool.release()
        otps.release()
        s12ps.release()
        smallps.release()
        a_sb.release()
        xnf_pool.release()
        xT_pool.release()
        dpool.release()
        const.release()

    nc.compile()
    return nc



# ----------------------------------------------------------------- entry

def kernel(**inputs):
    global last_results
    meta, percore = _prep_host(inputs)
    key = (meta["mol_T_blk"], meta["prot_T_blk"])
    if key not in _CACHE:
        _CACHE[key] = _build(meta)
    nc = _CACHE[key]
    from concourse.bass_utils import run_bass_kernel_spmd
    res = run_bass_kernel_spmd(nc, percore, list(range(R)))
    last_results = res
    return np.asarray(res.results[0]["out"], np.float32).reshape(B)



# revision 27
# speedup vs baseline: 1.6896x; 1.0039x over previous
"""CrossGraphAttentionModel on 8 Trainium2 NeuronCores (Bass/Tile, SPMD).

Sharding: nodes/edges of both graphs are sharded 8 ways by (dst-sorted) node
range; 64-dim weights replicated. Per GINE layer each core dma_gathers x[src]
for its edge shard from an AllGathered copy of x in HBM, forms messages on
DVE/ACT, and scatter-adds them with one-hot matmuls on the PE (PSUM
accumulation), then runs the node MLP on its node shard and AllGathers the new
x. Cross-graph attention shards the query axis: scores are computed twice on
PE - once [q,k] for an exact row max, once [k,q] with the max folded into the
contraction via an appended ones row - so softmax needs only a single ACT exp
pass, and the exp tiles feed the wV matmul directly as lhsT with a ones column
in V producing the softmax denominator for free. Graph pooling is a one-hot
matmul with 1/count weights, AllReduced, followed by the tiny output MLP.

All floating point math runs on device in fp32; the host only sorts/pads
integer index structures and transposes/replicates input layouts.
"""

import numpy as np

R = 8
HID = 64
B = 32
HEADS = 4
HD = 16
N_MOL, N_PROT = 2048, 4096
E_MOL, E_PROT = 32768, 131072
NC_MOL, NC_PROT = N_MOL // R, N_PROT // R          # 256, 512
NBLK_MOL, NBLK_PROT = NC_MOL // 128, NC_PROT // 128  # 2, 4

_CACHE = {}
last_results = None


# ----------------------------------------------------------------- host prep

def _prep_edges(edge_index, eattr, nblk):
    """Sort edges by dst, partition into R cores x nblk 128-node windows,
    pad every window to T_blk tiles of 128 edges. Returns device layouts."""
    src, dst = np.asarray(edge_index[0]), np.asarray(edge_index[1])
    eattr = np.asarray(eattr, np.float32)
    order = np.argsort(dst, kind="stable")
    src_s, dst_s, ea_s = src[order], dst[order], eattr[order]
    nblocks = R * nblk
    blk = dst_s // 128
    counts = np.bincount(blk, minlength=nblocks)
    T_blk = int(np.ceil(counts.max() / 128))
    T_total = nblk * T_blk
    E_core = T_total * 128
    D = eattr.shape[1]

    gidx = np.zeros((R, E_core), np.int64)
    dstoff = np.full((R, E_core), -1.0, np.float32)
    ea_pad = np.zeros((R, E_core, D), np.float32)
    starts = np.concatenate([[0], np.cumsum(counts)])
    for c in range(R):
        for b in range(nblk):
            g = c * nblk + b
            cnt = counts[g]
            lo = starts[g]
            off = b * T_blk * 128
            gidx[c, off:off + cnt] = src_s[lo:lo + cnt]
            dstoff[c, off:off + cnt] = (dst_s[lo:lo + cnt] - g * 128)
            ea_pad[c, off:off + cnt] = ea_s[lo:lo + cnt]

    # gather indices wrapped [128, E_core/16] (i -> p=i%16, col=i//16), x8 replicated
    cols = E_core // 16
    w = gidx.reshape(R, cols, 16).transpose(0, 2, 1).astype(np.int16)
    gidx_sb = np.tile(w, (1, 8, 1)).copy()
    # dstoff [128, T_total]
    dstoff_sb = np.ascontiguousarray(
        dstoff.reshape(R, T_total, 128).transpose(0, 2, 1))
    # eattr^T with ones row: [11, E_core]
    eaT_packed = np.ascontiguousarray(np.concatenate(
        [ea_pad.transpose(0, 2, 1),
         np.ones((R, 1, E_core), np.float32)], axis=1))
    return dict(T_blk=T_blk, T_total=T_total, E_core=E_core, D=D,
                gidx_sb=gidx_sb, dstoff_sb=dstoff_sb, eaT_packed=eaT_packed)


def _prep_host(inp):
    """All integer/layout preprocessing. Returns (meta, per_core_inputs)."""
    mol = _prep_edges(inp["mol_edge_index"], inp["mol_eattr"], NBLK_MOL)
    prot = _prep_edges(inp["prot_edge_index"], inp["prot_eattr"], NBLK_PROT)

    # pool matrices with 1/count entries
    def pmat(batch, ncore):
        batch = np.asarray(batch)
        cnt = np.bincount(batch, minlength=B).astype(np.float32)
        inv = 1.0 / np.maximum(cnt, 1.0)
        m = np.zeros((R, ncore, B), np.float32)
        for c in range(R):
            sl = batch[c * ncore:(c + 1) * ncore]
            m[c, np.arange(ncore), sl] = inv[sl]
        return m

    mol_pmat = pmat(inp["mol_batch"], NC_MOL)
    prot_pmat = pmat(inp["prot_batch"], NC_PROT)

    # node features transposed per core with ones row
    def xt(x, ncore):
        x = np.asarray(x, np.float32)
        d = x.shape[1]
        out = np.zeros((R, d + 1, ncore), np.float32)
        for c in range(R):
            out[c, :d] = x[c * ncore:(c + 1) * ncore].T
            out[c, d] = 1.0
        return out

    mol_xT = xt(inp["mol_x"], NC_MOL)        # [R, 12, 256]
    prot_xT = xt(inp["prot_x"], NC_PROT)     # [R, 16, 512]

    iota = np.tile(np.arange(128, dtype=np.float32), (128, 1))
    ident = np.eye(128, dtype=np.float32)
    # merge matrix for attention wV output: rows 0-15 -> cols 0-15 (V hi),
    # rows 16-31 -> cols 0-15 (V lo), row 32 -> col 16 (denominator)
    merge33 = np.zeros((33, 17), np.float32)
    merge33[np.arange(16), np.arange(16)] = 1.0
    merge33[16 + np.arange(16), np.arange(16)] = 1.0
    merge33[32, 16] = 1.0

    # attn K-bias as per-head columns [16, 4]
    def bcols(b):  # [64] -> [16, 4]
        return np.ascontiguousarray(np.asarray(b, np.float32).reshape(4, 16).T)

    percore = []
    for c in range(R):
        m = {
            "mol_xT": mol_xT[c], "prot_xT": prot_xT[c],
            "mol_eaT": mol["eaT_packed"][c], "prot_eaT": prot["eaT_packed"][c],
            "mol_gidx": mol["gidx_sb"][c], "prot_gidx": prot["gidx_sb"][c],
            "mol_dstoff": mol["dstoff_sb"][c], "prot_dstoff": prot["dstoff_sb"][c],
            "mol_pmat": mol_pmat[c], "prot_pmat": prot_pmat[c],
            "iota": iota, "ident": ident, "merge33": merge33,
            "bk_mp_cols": bcols(np.asarray(inp["attn_mp_b"])[1]),
            "bk_pm_cols": bcols(np.asarray(inp["attn_pm_b"])[1]),
        }
        for k in ("node_lin_mol_W", "node_lin_mol_b", "node_lin_prot_W",
                  "node_lin_prot_b", "edge_lin_mol_W", "edge_lin_mol_b",
                  "edge_lin_prot_W", "edge_lin_prot_b",
                  "mol_conv_W1", "mol_conv_b1", "mol_conv_W2", "mol_conv_b2",
                  "prot_conv_W1", "prot_conv_b1", "prot_conv_W2", "prot_conv_b2",
                  "attn_mp_W", "attn_mp_b", "attn_pm_W", "attn_pm_b",
                  "fc1_W", "fc1_b", "fc2_W", "fc2_b"):
            m[k] = np.asarray(inp[k], np.float32)
        percore.append(m)

    meta = dict(mol_T_blk=mol["T_blk"], mol_T_total=mol["T_total"],
                mol_E_core=mol["E_core"],
                prot_T_blk=prot["T_blk"], prot_T_total=prot["T_total"],
                prot_E_core=prot["E_core"])
    return meta, percore


# ------------------------------------------------------------- device build

def _build(meta):
    import concourse.bacc as bacc
    import concourse.mybir as mybir
    import concourse.tile as tile

    F32 = mybir.dt.float32
    F32R = mybir.dt.float32r
    BF16 = mybir.dt.bfloat16
    I16 = mybir.dt.int16
    AF = mybir.ActivationFunctionType
    ALU = mybir.AluOpType

    nc = bacc.Bacc("TRN2", target_bir_lowering=False, debug=False,
                   num_devices=R)

    # ---- I/O declarations
    dram = {}

    def din(name, shape, dtype=F32):
        dram[name] = nc.dram_tensor(name, list(shape), dtype,
                                    kind="ExternalInput")
        return dram[name]

    mT, mE = meta["mol_T_total"], meta["mol_E_core"]
    pT, pE = meta["prot_T_total"], meta["prot_E_core"]

    din("mol_xT", [12, NC_MOL]); din("prot_xT", [16, NC_PROT])
    din("mol_eaT", [11, mE]); din("prot_eaT", [11, pE])
    din("mol_gidx", [128, mE // 16], I16); din("prot_gidx", [128, pE // 16], I16)
    din("mol_dstoff", [128, mT]); din("prot_dstoff", [128, pT])
    din("mol_pmat", [NC_MOL, B]); din("prot_pmat", [NC_PROT, B])
    din("iota", [128, 128]); din("ident", [128, 128])
    din("merge33", [33, 17])
    din("bk_mp_cols", [16, 4]); din("bk_pm_cols", [16, 4])
    din("node_lin_mol_W", [11, 64]); din("node_lin_mol_b", [64])
    din("node_lin_prot_W", [15, 64]); din("node_lin_prot_b", [64])
    din("edge_lin_mol_W", [10, 64]); din("edge_lin_mol_b", [64])
    din("edge_lin_prot_W", [10, 64]); din("edge_lin_prot_b", [64])
    for s in ("mol", "prot"):
        din(f"{s}_conv_W1", [3, 64, 64]); din(f"{s}_conv_b1", [3, 64])
        din(f"{s}_conv_W2", [3, 64, 64]); din(f"{s}_conv_b2", [3, 64])
    din("attn_mp_W", [3, 64, 64]); din("attn_mp_b", [3, 64])
    din("attn_pm_W", [3, 64, 64]); din("attn_pm_b", [3, 64])
    din("fc1_W", [128, 64]); din("fc1_b", [64])
    din("fc2_W", [64, 1]); din("fc2_b", [1])

    out_d = nc.dram_tensor("out", [1, B], F32, kind="ExternalOutput")

    sides = {
        "mol": dict(N=N_MOL, NC=NC_MOL, nblk=NBLK_MOL, T_blk=meta["mol_T_blk"],
                    T_total=mT, E_core=mE, D=10),
        "prot": dict(N=N_PROT, NC=NC_PROT, nblk=NBLK_PROT,
                     T_blk=meta["prot_T_blk"], T_total=pT, E_core=pE, D=10),
    }

    with tile.TileContext(nc) as tc:
        # ---------------- persistent SBUF constants
        const = tc.alloc_tile_pool(name="const", bufs=1)

        def load_const(name, shape, dtype=F32, src=None):
            t = const.tile(list(shape), dtype, name=f"c_{name}")
            nc.sync.dma_start(t[:], (dram[name] if src is None else src)[:])
            return t

        def wcat(name_w, name_b, din_, dout, wslice=None, bslice=None):
            t = const.tile([din_ + 1, dout], F32, name=f"w_{name_w}_{wslice}")
            wsrc = dram[name_w] if wslice is None else dram[name_w][wslice]
            bsrc = dram[name_b] if bslice is None else dram[name_b][bslice]
            nc.sync.dma_start(t[0:din_, :], wsrc[:, :] if wslice is None else wsrc)
            nc.sync.dma_start(t[din_:din_ + 1, :], bsrc[None, :])
            return t

        # x0-critical loads first so the first AllGather launches ASAP
        sb_xTin = {"prot": load_const("prot_xT", [16, NC_PROT]),
                   "mol": load_const("mol_xT", [12, NC_MOL])}
        Wn = {"prot": wcat("node_lin_prot_W", "node_lin_prot_b", 15, 64),
              "mol": wcat("node_lin_mol_W", "node_lin_mol_b", 11, 64)}
        iota_sb = load_const("iota", [128, 128])
        ident_sb = load_const("ident", [128, 128])
        sb_idx, sb_dstoff = {}, {}
        for s in ("prot", "mol"):
            sd = sides[s]
            sb_idx[s] = load_const(f"{s}_gidx", [128, sd["E_core"] // 16], I16)
            sb_dstoff[s] = load_const(f"{s}_dstoff", [128, sd["T_total"]])
        We = {"mol": wcat("edge_lin_mol_W", "edge_lin_mol_b", 10, 64),
              "prot": wcat("edge_lin_prot_W", "edge_lin_prot_b", 10, 64)}
        W1 = {s: [wcat(f"{s}_conv_W1", f"{s}_conv_b1", 64, 64, l, l)
                  for l in range(3)] for s in ("mol", "prot")}
        W2 = {s: [wcat(f"{s}_conv_W2", f"{s}_conv_b2", 64, 64, l, l)
                  for l in range(3)] for s in ("mol", "prot")}
        sb_pmat = {}
        for s in sides:
            sd = sides[s]
            t = const.tile([128, sd["nblk"], B], F32, name=f"pmat_{s}")
            nc.sync.dma_start(
                t[:], dram[f"{s}_pmat"].rearrange("(t p) g -> p t g", p=128))
            sb_pmat[s] = t

        # ---------------- DRAM internals
        dpool = tc.alloc_tile_pool(name="dram", bufs=1, space="DRAM")
        x_sh_d = {s: [dpool.tile([sides[s]["NC"], 64], F32,
                                 name=f"xsh_{s}_{l}") for l in range(3)]
                  for s in sides}
        x_full_d = {s: [dpool.tile([sides[s]["N"], 64], F32,
                                   addr_space="Shared", name=f"xfull_{s}_{l}")
                        for l in range(3)] for s in sides}
        x_shT_d = {s: dpool.tile([65, sides[s]["NC"]], F32,
                                 name=f"xshT_{s}") for s in sides}
        x_fullT_d = {s: dpool.tile([R, 65, sides[s]["NC"]], F32,
                                   addr_space="Shared", name=f"xfullT_{s}")
                     for s in sides}

        # ---------------- long-lived x pools, then GINE-scoped pools
        xT_pool = tc.alloc_tile_pool(name="xT", bufs=2)
        xnf_pool = tc.alloc_tile_pool(name="xnf", bufs=2)
        gmem = tc.alloc_tile_pool(name="gmem", bufs=1)
        empp = tc.alloc_tile_pool(name="empp", bufs=2, space="PSUM")
        aggps = tc.alloc_tile_pool(name="aggps", bufs=2, space="PSUM")
        mlpps = tc.alloc_tile_pool(name="mlpps", bufs=2, space="PSUM")
        trps = tc.alloc_tile_pool(name="trps", bufs=2, space="PSUM")

        # pools for the GINE loop allocated up-front so nothing aliases the
        # ea_stream buffers (aliasing would stall the first gathers on a WAR)
        xg_pool = tc.alloc_tile_pool(name="xg", bufs=2)
        oh_pool = tc.alloc_tile_pool(name="oh", bufs=2)
        ea_stream = tc.alloc_tile_pool(name="ea_stream", bufs=2)

        # initial node features x0 first: gets the AllGathers in flight early
        xT_cur = {}
        xnf_cur = {}
        for s in ("prot", "mol"):
            sd = sides[s]
            NCs, nblk = sd["NC"], sd["nblk"]
            ps = mlpps.tile([64, 512], F32, name="mlp_ps")
            nc.tensor.matmul(ps[:, 0:NCs], Wn[s][:], sb_xTin[s][:],
                             start=True, stop=True)
            xT = xT_pool.tile([65, NCs], F32, name=f"xT_{s}")
            nc.vector.tensor_copy(xT[0:64, :], ps[:, 0:NCs])
            nc.vector.memset(xT[64:65, :], 1.0)
            xnf = xnf_pool.tile([128, nblk, 64], F32, name=f"xnf_{s}")
            for b in range(nblk):
                tp = trps.tile([128, 64], F32, name="tr_ps")
                nc.tensor.transpose(tp[:], xT[0:64, b * 128:(b + 1) * 128],
                                    ident_sb[0:64, 0:64])
                nc.vector.tensor_copy(xnf[:, b, :], tp[:])
            nc.sync.dma_start(
                x_sh_d[s][0][:].rearrange("(t p) f -> p t f", p=128), xnf[:])
            nc.gpsimd.collective_compute(
                "AllGather", ALU.bypass, replica_groups=[list(range(R))],
                ins=[x_sh_d[s][0][:].opt()], outs=[x_full_d[s][0][:].opt()])
            xT_cur[s] = xT
            xnf_cur[s] = xnf

        # attention weight prep: P65_h = 0.25 * Wq_h^T (Wk_h | bk_h) folded
        # per-head projector; depends only on weights so it overlaps the
        # first gathers
        atw = {}
        for dirn in ("mp", "pm"):
            Wd = dram[f"attn_{dirn}_W"]
            bd = dram[f"attn_{dirn}_b"]
            Wq = const.tile([65, 64], F32, name=f"Wq_{dirn}")
            nc.sync.dma_start(Wq[0:64, :], Wd[0])
            nc.sync.dma_start(Wq[64:65, :], bd[0][None, :])
            Wv_f = const.tile([65, 64], F32, name=f"Wvf_{dirn}")
            nc.sync.dma_start(Wv_f[0:64, :], Wd[2])
            nc.sync.dma_start(Wv_f[64:65, :], bd[2][None, :])
            Wv = const.tile([65, 64], F32R, name=f"Wv_{dirn}")
            nc.vector.tensor_copy(Wv[:], Wv_f[:])
            Wk_raw = const.tile([64, 64], F32, name=f"Wkraw_{dirn}")
            nc.sync.dma_start(Wk_raw[:], Wd[1])
            bk_cols = const.tile([16, 4], F32, name=f"bkcols_{dirn}")
            nc.sync.dma_start(bk_cols[:], dram[f"bk_{dirn}_cols"][:])
            P65s = []
            for h in range(HEADS):
                pv = empp.tile([128, 8, 64], F32, name="em_ps")[:].rearrange(
                    "p a b -> p (a b)")
                nc.tensor.transpose(pv[0:16, 0:64],
                                    Wk_raw[:, 16 * h:16 * h + 16],
                                    ident_sb[0:64, 0:64])
                wkt65 = const.tile([16, 65], F32, name=f"wkt65_{dirn}_{h}")
                nc.vector.tensor_copy(wkt65[:, 0:64], pv[0:16, 0:64])
                nc.vector.tensor_copy(wkt65[:, 64:65], bk_cols[:, h:h + 1])
                qv = empp.tile([128, 8, 64], F32, name="em_ps")[:].rearrange(
                    "p a b -> p (a b)")
                nc.tensor.transpose(qv[0:16, 0:65],
                                    Wq[:, 16 * h:16 * h + 16],
                                    ident_sb[0:65, 0:65])
                wqt = const.tile([16, 65], F32, name=f"wqt_{dirn}_{h}")
                nc.vector.tensor_copy(wqt[:], qv[0:16, 0:65])
                ppv = empp.tile([128, 8, 64], F32, name="em_ps")[:].rearrange(
                    "p a b -> p (a b)")
                nc.tensor.matmul(ppv[0:65, 0:65], wqt[:], wkt65[:],
                                 start=True, stop=True)
                P65 = const.tile([65, 65], F32R, name=f"P65_{dirn}_{h}")
                nc.scalar.activation(P65[:], ppv[0:65, 0:65], AF.Copy,
                                     scale=0.25)
                P65s.append(P65)
            atw[dirn] = dict(Wv=Wv, P65s=P65s)

        # edge features em = [eattr;1] @ [We;be], edge-major [128, T, 64],
        # eattr^T streamed from DRAM per block
        em_sb = {}
        for s in sides:
            sd = sides[s]
            T_total, T_blk, nblk, D = (sd["T_total"], sd["T_blk"], sd["nblk"],
                                       sd["D"])
            em = gmem.tile([128, T_total, 64], F32, name=f"em_{s}")
            for b in range(nblk):
                ch = ea_stream.tile([11, T_blk * 128], F32, name="ea_chunk")
                nc.sync.dma_start(
                    ch[:],
                    dram[f"{s}_eaT"][:, b * T_blk * 128:(b + 1) * T_blk * 128])
                for t0 in range(0, T_blk, 8):
                    ng = min(8, T_blk - t0)
                    ps = empp.tile([128, 8, 64], F32, name="em_ps")
                    for j in range(ng):
                        nc.tensor.matmul(
                            ps[:, j, :],
                            ch[0:D + 1, (t0 + j) * 128:(t0 + j + 1) * 128],
                            We[s][:], start=True, stop=True)
                    nc.vector.tensor_copy(
                        em[:, b * T_blk + t0:b * T_blk + t0 + ng, :],
                        ps[:, 0:ng, :])
            em_sb[s] = em

        # GINE layers
        for l in range(3):
            ag_jobs = []
            for s in ("prot", "mol"):
                sd = sides[s]
                NCs, nblk, T_blk = sd["NC"], sd["nblk"], sd["T_blk"]
                xT_prev = xT_cur[s]
                hT = gmem.tile([65, NCs], F32, name=f"hT_{s}_{l}", bufs=2,
                               tag=f"hT_{s}")
                for b in range(nblk):
                    nE = T_blk * 128
                    xg = xg_pool.tile([128, T_blk, 64], F32, name="xg")
                    nc.gpsimd.dma_gather(
                        xg[:], x_full_d[s][l][:],
                        sb_idx[s][:, b * T_blk * 8:(b + 1) * T_blk * 8],
                        nE, nE, 64, single_packet=False)
                    oh = oh_pool.tile([128, T_blk, 128], F32, name="oh")
                    nc.vector.tensor_tensor(
                        oh[:],
                        iota_sb[:, :].unsqueeze(1).broadcast_to([128, T_blk, 128]),
                        sb_dstoff[s][:, b * T_blk:(b + 1) * T_blk]
                            .unsqueeze(2).broadcast_to([128, T_blk, 128]),
                        ALU.is_equal)
                    msg = xg_pool.tile([128, T_blk, 64], F32, name="msg")
                    nc.vector.tensor_add(
                        msg[:], xg[:], em_sb[s][:, b * T_blk:(b + 1) * T_blk, :])
                    nc.scalar.activation(msg[:], msg[:], AF.Relu)
                    agg = aggps.tile([64, 128], F32, name="agg_ps")
                    for t in range(T_blk):
                        nc.tensor.matmul(agg[:], msg[:, t, :], oh[:, t, :],
                                         start=(t == 0), stop=(t == T_blk - 1))
                    nc.vector.tensor_add(hT[0:64, b * 128:(b + 1) * 128],
                                         xT_prev[0:64, b * 128:(b + 1) * 128],
                                         agg[:])
                nc.vector.memset(hT[64:65, :], 1.0)
                ps1 = mlpps.tile([64, 512], F32, name="mlp_ps")
                nc.tensor.matmul(ps1[:, 0:NCs], W1[s][l][:], hT[:],
                                 start=True, stop=True)
                r1 = gmem.tile([65, NCs], F32, name=f"r1_{s}_{l}", bufs=2,
                               tag=f"r1_{s}")
                nc.scalar.activation(r1[0:64, :], ps1[:, 0:NCs], AF.Relu)
                nc.vector.memset(r1[64:65, :], 1.0)
                ps2 = mlpps.tile([64, 512], F32, name="mlp_ps")
                nc.tensor.matmul(ps2[:, 0:NCs], W2[s][l][:], r1[:],
                                 start=True, stop=True)
                xT = xT_pool.tile([65, NCs], F32, name=f"xT_{s}")
                nc.scalar.activation(xT[0:64, :], ps2[:, 0:NCs], AF.Relu)
                nc.vector.memset(xT[64:65, :], 1.0)
                if l == 2:
                    # final layer: AllGather the transposed x directly for
                    # attention (skips the node-major transpose round trip)
                    nc.sync.dma_start(x_shT_d[s][:], xT[:])
                    nc.gpsimd.collective_compute(
                        "AllGather", ALU.bypass,
                        replica_groups=[list(range(R))],
                        ins=[x_shT_d[s][:].opt()],
                        outs=[x_fullT_d[s][:].opt()])
                xnf = xnf_pool.tile([128, nblk, 64], F32, name=f"xnf_{s}")
                for b in range(nblk):
                    tp = trps.tile([128, 64], F32, name="tr_ps")
                    nc.tensor.transpose(tp[:], xT[0:64, b * 128:(b + 1) * 128],
                                        ident_sb[0:64, 0:64])
                    nc.vector.tensor_copy(xnf[:, b, :], tp[:])
                if l < 2:
                    nc.sync.dma_start(
                        x_sh_d[s][l + 1][:].rearrange("(t p) f -> p t f",
                                                      p=128),
                        xnf[:])
                    ag_jobs.append((x_sh_d[s][l + 1], x_full_d[s][l + 1]))
                xT_cur[s] = xT
                xnf_cur[s] = xnf
            # AllGather launches after both sides' gathers are enqueued, so
            # a waiting launch never stalls the next gathers in the GpSimd
            # queue
            for src_d, dst_d in ag_jobs:
                nc.gpsimd.collective_compute(
                    "AllGather", ALU.bypass,
                    replica_groups=[list(range(R))],
                    ins=[src_d[:].opt()], outs=[dst_d[:].opt()])

        # close GINE-scoped pools (LIFO per space)
        ea_stream.release()
        oh_pool.release()
        xg_pool.release()
        for p in (trps, mlpps, aggps, empp):
            p.release()
        gmem.release()

        # ---------------- attention phase
        # All large matmuls stream in float32r (fp32 data, ~1 cycle/row on
        # the PE for moving dim >= 256, measured ~1.6e-4 rel err). Scores are
        # computed twice: pass 1 in [q, k] layout for the row max, pass 2 in
        # [k, q] with the max folded in via the ones row; the exp tiles feed
        # the wV matmuls directly as moving data in the transposed domain
        # with a ones column in V producing the softmax denominator.
        a_sb = tc.alloc_tile_pool(name="attn_sb", bufs=1)
        smallps = tc.alloc_tile_pool(name="smallps", bufs=2, space="PSUM")
        s12ps = tc.alloc_tile_pool(name="s12ps", bufs=3, space="PSUM")
        otps = tc.alloc_tile_pool(name="otps", bufs=2, space="PSUM")
        exp_pool = tc.alloc_tile_pool(name="expt", bufs=4)

        # full x (both sides), transposed with ones row, from the transposed
        # AllGather (float32r: the DVE round-copy feeds the PE at full rate)
        xT_full = {}
        for s in ("prot", "mol"):
            Ns = sides[s]["N"]
            xT_ld = a_sb.tile([65, Ns], F32, name=f"xTld_{s}")
            NCs = sides[s]["NC"]
            for r in range(R):
                nc.sync.dma_start(xT_ld[:, r * NCs:(r + 1) * NCs],
                                  x_fullT_d[s][r])
            xT_f = a_sb.tile([65, Ns], F32R, name=f"xTfull_{s}")
            nc.vector.tensor_copy(xT_f[:], xT_ld[:])
            xT_full[s] = xT_f

        # per-direction prep: folded per-head projector P65 [65, 65] with
        # rhs0_h = P65_h^T @ xT_q  (row 64 = c_h, the K-bias term)
        prep = {}
        for dirn, (qs, ks) in (("mp", ("mol", "prot")), ("pm", ("prot", "mol"))):
            NCq = sides[qs]["NC"]
            n_k128 = sides[ks]["N"] // 128
            Wv = atw[dirn]["Wv"]

            rhs0 = []
            cT = a_sb.tile([1, HEADS, NCq], F32, name=f"cT_{dirn}")
            xTq_r = a_sb.tile([65, NCq], F32R, name=f"xTq_{dirn}")
            nc.vector.tensor_copy(xTq_r[:], xT_cur[qs][:])
            for h in range(HEADS):
                rps = smallps.tile([128, 512], F32, name="small_ps")
                nc.tensor.matmul(rps[0:65, 0:NCq], atw[dirn]["P65s"][h][:],
                                 xTq_r[:], start=True, stop=True)
                r0 = a_sb.tile([65, NCq], F32R, name=f"rhs0_{dirn}_{h}")
                nc.vector.tensor_copy(r0[:], rps[0:65, 0:NCq])
                nc.vector.tensor_copy(cT[0:1, h, :], rps[64:65, 0:NCq])
                rhs0.append(r0)

            # V' [128, n_k128, 4, 17] with ones col
            Vp = a_sb.tile([128, n_k128, HEADS, 17], F32R, name=f"Vp_{dirn}")
            ones_v = a_sb.tile([128, n_k128, HEADS, 1], F32, name="ones_v",
                               bufs=2, tag="ones_v")
            nc.vector.memset(ones_v[:], 1.0)
            nc.vector.tensor_copy(Vp[:, :, :, 16:17], ones_v[:])
            for kt in range(n_k128):
                ps = smallps.tile([128, 512], F32, name="small_ps")
                nc.tensor.matmul(ps[0:128, 0:64],
                                 xT_full[ks][:, kt * 128:(kt + 1) * 128],
                                 Wv[:], start=True, stop=True)
                nc.vector.tensor_copy(
                    Vp[:, kt, :, 0:16],
                    ps[0:128, 0:64].rearrange("p (h d) -> p h d", h=HEADS))
            prep[dirn] = dict(rhs0=rhs0, Vp=Vp, cT=cT)

        # pass 1: row max m_h [1, NCq] per head ([q, k] layout)
        mT = {}
        for dirn, (qs, ks) in (("mp", ("mol", "prot")), ("pm", ("prot", "mol"))):
            NCq = sides[qs]["NC"]
            n_qt = NCq // 128
            n_k512 = sides[ks]["N"] // 512
            rhs0 = prep[dirn]["rhs0"]
            mTd = a_sb.tile([1, HEADS, NCq], F32, name=f"mT_{dirn}")
            for h in range(HEADS):
                for qt in range(n_qt):
                    mx = a_sb.tile([128, n_k512], F32, name="mx", bufs=2,
                                   tag="mx")
                    for cch in range(n_k512):
                        s1 = s12ps.tile([128, 512], F32, name="s12_ps")
                        nc.tensor.matmul(
                            s1[:],
                            rhs0[h][:, qt * 128:(qt + 1) * 128],
                            xT_full[ks][:, cch * 512:(cch + 1) * 512],
                            start=True, stop=True)
                        nc.vector.reduce_max(mx[:, cch:cch + 1], s1[:],
                                             axis=mybir.AxisListType.X)
                    mqt = a_sb.tile([128, 1], F32, name="mqt", bufs=2,
                                    tag="mqt")
                    nc.vector.reduce_max(mqt[:], mx[:],
                                         axis=mybir.AxisListType.X)
                    tp = smallps.tile([128, 512], F32, name="small_ps")
                    nc.tensor.transpose(tp[0:1, 0:128], mqt[:], ident_sb[:])
                    nc.vector.tensor_copy(
                        mTd[0:1, h, qt * 128:(qt + 1) * 128], tp[0:1, 0:128])
            mT[dirn] = mTd

        # pass 2 + wV (pooling for each direction follows its H directly so
        # the mp pooling overlaps the pm pass)
        zt_part_d = dpool.tile([128, B], F32, name="zt_part")
        zt_full_d = dpool.tile([128, B], F32, addr_space="Shared",
                               name="zt_full")
        H_sb = {}
        for dirn, (qs, ks) in (("mp", ("mol", "prot")), ("pm", ("prot", "mol"))):
            NCq = sides[qs]["NC"]
            n_qt = NCq // 128
            n_k128 = sides[ks]["N"] // 128
            rhs0 = prep[dirn]["rhs0"]
            Vp = prep[dirn]["Vp"]
            xTk = xT_full[ks]
            H = a_sb.tile([128, n_qt, 64], F32, name=f"H_{dirn}")
            # head groups: pack 2 heads side-by-side when NCq == 256 so the
            # score matmul streams a full 512-wide PSUM tile
            hgroups = ([(0, 1), (2, 3)] if NCq == 256
                       else [(0,), (1,), (2,), (3,)])
            for hg in hgroups:
                gw = NCq * len(hg)
                rhm = a_sb.tile([65, gw], F32R, name="rhm", bufs=2, tag="rhm")
                cm = a_sb.tile([1, gw], F32, name="cm", bufs=2, tag="cm")
                for j, h in enumerate(hg):
                    nc.vector.tensor_copy(
                        rhm[0:64, j * NCq:(j + 1) * NCq], rhs0[h][0:64, :])
                    nc.vector.tensor_sub(
                        cm[0:1, j * NCq:(j + 1) * NCq],
                        prep[dirn]["cT"][0:1, h, :], mT[dirn][0:1, h, :])
                nc.vector.tensor_copy(rhm[64:65, :], cm[:])

                oT = {h: otps.tile([17, NCq], F32, name="oT") for h in hg}
                pend = None
                for kc in range(n_k128):
                    ps = s12ps.tile([128, 512], F32, name="s12_ps")
                    nc.tensor.matmul(ps[:, 0:gw],
                                     xTk[:, kc * 128:(kc + 1) * 128],
                                     rhm[:], start=True, stop=True)
                    ex = exp_pool.tile([128, gw], F32R, name="ex",
                                       tag=f"ex_{dirn}")
                    nc.scalar.activation(ex[:], ps[:, 0:gw], AF.Exp)
                    if pend is not None:
                        pkc, pex = pend
                        for j, h in enumerate(hg):
                            nc.tensor.matmul(
                                oT[h][:], Vp[:, pkc, h, :],
                                pex[:, j * NCq:(j + 1) * NCq],
                                start=(pkc == 0), stop=False)
                    pend = (kc, ex)
                pkc, pex = pend
                for j, h in enumerate(hg):
                    nc.tensor.matmul(
                        oT[h][:], Vp[:, pkc, h, :],
                        pex[:, j * NCq:(j + 1) * NCq],
                        start=(pkc == 0), stop=True)
                # transpose oT back to node-major; col 16 = denominator
                for j, h in enumerate(hg):
                    oT_sb = a_sb.tile([17, NCq], F32, name="oT_sb", bufs=2,
                                      tag="oT_sb")
                    nc.vector.tensor_copy(oT_sb[:], oT[h][:])
                    for qt in range(n_qt):
                        mps = smallps.tile([128, 512], F32, name="small_ps")
                        nc.tensor.transpose(
                            mps[0:128, 0:17],
                            oT_sb[:, qt * 128:(qt + 1) * 128],
                            ident_sb[0:17, 0:17])
                        inv1 = a_sb.tile([128, 1], F32, name="inv1", bufs=2,
                                         tag="inv1")
                        nc.vector.reciprocal(inv1[:], mps[0:128, 16:17])
                        nc.vector.tensor_scalar_mul(
                            H[:, qt, 16 * h:16 * (h + 1)],
                            mps[0:128, 0:16], inv1[:])

            # residual: H += x (node-major shard)
            nc.vector.tensor_add(H[:], H[:], xnf_cur[qs][:])
            H_sb[dirn] = H
            psz = smallps.tile([128, 512], F32, name="small_ps")
            for qt in range(n_qt):
                nc.tensor.matmul(psz[0:64, 0:B], H[:, qt, :],
                                 sb_pmat[qs][:, qt, :],
                                 start=(qt == 0), stop=(qt == n_qt - 1))
            zpart = a_sb.tile([64, B], F32, name=f"zpart_{dirn}")
            nc.vector.tensor_copy(zpart[:], psz[0:64, 0:B])
            row0 = 0 if dirn == "mp" else 64
            nc.sync.dma_start(zt_part_d[row0:row0 + 64, :], zpart[:])

        # ---------------- output MLP
        nc.gpsimd.collective_compute(
            "AllReduce", ALU.add, replica_groups=[list(range(R))],
            ins=[zt_part_d[:].opt()], outs=[zt_full_d[:].opt()])
        zT = a_sb.tile([128, B], F32, name="zT")
        nc.sync.dma_start(zT[:], zt_full_d[:])

        fc1W = a_sb.tile([128, 64], F32, name="fc1W")
        nc.sync.dma_start(fc1W[:], dram["fc1_W"][:])
        fc1b = a_sb.tile([64, 1], F32, name="fc1b")
        nc.sync.dma_start(fc1b[:], dram["fc1_b"][:, None])
        fc2W = a_sb.tile([64, 1], F32, name="fc2W")
        nc.sync.dma_start(fc2W[:], dram["fc2_W"][:])
        fc2b = a_sb.tile([1, 1], F32, name="fc2b")
        nc.sync.dma_start(fc2b[:], dram["fc2_b"][:, None])

        ps = smallps.tile([128, 512], F32, name="small_ps")
        nc.tensor.matmul(ps[0:64, 0:B], fc1W[:], zT[:], start=True, stop=True)
        h1 = a_sb.tile([65, B], F32, name="h1")
        nc.scalar.activation(h1[0:64, :], ps[0:64, 0:B], AF.Relu, bias=fc1b[:])
        ps2 = smallps.tile([128, 512], F32, name="small_ps")
        nc.tensor.matmul(ps2[0:1, 0:B], fc2W[:], h1[0:64, :],
                         start=True, stop=True)
        osb = a_sb.tile([1, B], F32, name="osb")
        nc.scalar.activation(osb[:], ps2[0:1, 0:B], AF.Sigmoid, bias=fc2b[:])
        nc.sync.dma_start(out_d[:], osb[:])

        exp_pool.release()
        otps.release()
        s12ps.release()
        smallps.release()
        a_sb.release()
        xnf_pool.release()
        xT_pool.release()
        dpool.release()
        const.release()

    nc.compile()
    return nc



# ----------------------------------------------------------------- entry

def kernel(**inputs):
    global last_results
    meta, percore = _prep_host(inputs)
    key = (meta["mol_T_blk"], meta["prot_T_blk"])
    if key not in _CACHE:
        _CACHE[key] = _build(meta)
    nc = _CACHE[key]
    from concourse.bass_utils import run_bass_kernel_spmd
    res = run_bass_kernel_spmd(nc, percore, list(range(R)))
    last_results = res
    return np.asarray(res.results[0]["out"], np.float32).reshape(B)

